# revision 1
# baseline (speedup 1.0000x reference)
"""Circulant matmul for TRN2: 4-level CRT with trinomial split, bf16 matmuls.

out[b, r] = sum_c x[b,c] w[(c-r) mod N] = (x (*) wt)[r], wt = roll(w[::-1],1)
(cyclic convolution along c) — no input flip or output reversal needed.

CRT tree on z^4096-1 (per 128-row block of x):
  cyc4096 -> cyc2048 + nega2048             (binomial, band scale 1/2 each)
  cyc2048 -> cyc1024 + nega1024[LEAF]       (1/2)
  cyc1024 -> cyc512[LEAF] + nega512[LEAF]   (1/2)
  nega2048 -> T1024+ , T1024- [LEAVES]      (z^1024 -+ sqrt2 z^512 + 1, 1/(2sqrt2))
Leaf matmuls: 56 x [K=128, M=128, N=512] bf16 = 28672 PE cycles/block
(vs 88/45056 for the 3-level f32r baseline). Reconstruction scales fold into
the ACT PSUM->SBUF evacuations; sqrt2 scale passes run on ACT. Ring bands are
(piecewise) shear tensors S[q - k]; the host precomputes all 7 leaf strips
from w into one [128, 9344] bf16 table (2.4 MB) — no on-chip band build.

Per-block engine budget: PE 11.95us (critical), DMA 11.65, DVE ~11
(bf16 2x tensor_tensor), Pool ~10.3 (plain tensor_tensor only — the
compiler rejects scalar_tensor_tensor/tensor_scalar on Pool), ACT ~7.
Evacs interleave with the matmul stream (PSUM banks recycle without
stalling PE); the last block runs T matmuls first and nega1024 last in
two half-PSUM tiles so the reconstruction tail streams out per half.
"""

import sys

sys.path.insert(0, "/opt/trn_rl_repo")

import numpy as np

N = 4096
B = 8192
N_CORES = 8
B_SHARD = B // N_CORES  # 1024
NB = B_SHARD // 128     # 8 row-tiles per core
R2 = float(np.sqrt(2.0))
SC_T = float(1.0 / (4.0 * np.sqrt(2.0)))   # T1024 leaves: 1/2 * 1/(2sqrt2)

# strip layout in the packed [128, SBW] host table (name: col0, width, OFF)
STRIPS = {
    "sC5": (0, 896, -384),
    "sN5": (896, 896, -384),
    "sN10": (1792, 1920, -896),
    "sLp": (3712, 1408, -896),
    "sHp": (5120, 1408, -896),
    "sLm": (6528, 1408, -896),
    "sHm": (7936, 1408, -896),
}
SBW = 9344
SB_SPLIT = 3712  # first DMA: C5/N5/N10 strips; second: T strips

_STATE = {}


def _build():
    import concourse.bacc as bacc
    import concourse.mybir as mybir
    import concourse.tile as tile

    f32 = mybir.dt.float32
    bf16 = mybir.dt.bfloat16
    ADD = mybir.AluOpType.add
    SUB = mybir.AluOpType.subtract

    nc = bacc.Bacc("TRN2", target_bir_lowering=False, debug=False)
    xtr_d = nc.declare_dram_parameter("xtr", [N, B_SHARD], f32, isOutput=False)
    sb_d = nc.declare_dram_parameter("sb", [128, SBW], bf16, isOutput=False)
    out_d = nc.declare_dram_parameter("out", [B_SHARD, N], f32, isOutput=True)

    xtr_t = xtr_d[:].rearrange("(a p) b -> p a b", p=128)  # [128, 32, B_SHARD]

    with tile.TileContext(nc) as tc:
        with (
            tc.tile_pool(name="const", bufs=1) as constp,
            tc.tile_pool(name="xbig", bufs=3) as xbigp,
            tc.tile_pool(name="fold", bufs=2) as fp,
            tc.tile_pool(name="evac", bufs=3) as ep,
            tc.tile_pool(name="unf", bufs=2) as up,
            tc.tile_pool(name="outp", bufs=2) as op,
            tc.tile_pool(name="psum", bufs=1, space="PSUM") as pp,
        ):
            SB = constp.tile([128, SBW], bf16, name="SB")
            warm = constp.tile([128, 512], bf16, name="warm")
            strip = {k: SB[:, c0 : c0 + w] for k, (c0, w, _) in STRIPS.items()}

            def emit_dma_in(bt):
                b0 = 128 * bt
                xbig = xbigp.tile([128, 32, 128], f32, tag="xbig", name="xbig")
                nc.sync.dma_start(xbig[:], xtr_t[:, :, b0 : b0 + 128])
                return xbig

            def emit_folds_a(xbig, swap=False):
                """Early folds: xc (Pool), binomial chain (DVE) — these feed
                the first matmuls of the next block. swap=True runs xc on DVE
                instead (fill phase: xc gates the chain and Pool is 2x slower
                at the mixed L1 op)."""
                xc = fp.tile([128, 16, 128], bf16, tag="xc", name="xc")
                xn = fp.tile([128, 16, 128], bf16, tag="xn", name="xn")
                if swap:
                    nc.vector.tensor_tensor(
                        xc[:], xbig[:, 0:32:2, :], xbig[:, 1:32:2, :], ADD
                    )
                    nc.gpsimd.tensor_tensor(
                        xn[:], xbig[:, 0:32:2, :], xbig[:, 1:32:2, :], SUB
                    )
                else:
                    nc.gpsimd.tensor_tensor(
                        xc[:], xbig[:, 0:32:2, :], xbig[:, 1:32:2, :], ADD
                    )
                    nc.vector.tensor_tensor(
                        xn[:], xbig[:, 0:32:2, :], xbig[:, 1:32:2, :], SUB
                    )
                xcc = fp.tile([128, 8, 128], bf16, tag="xcc", name="xcc")
                nc.vector.tensor_tensor(xcc[:], xc[:, 0:8, :], xc[:, 8:16, :], ADD)
                xccc = fp.tile([128, 4, 128], bf16, tag="xccc", name="xccc")
                nc.vector.tensor_tensor(
                    xccc[:], xcc[:, 0:4, :], xcc[:, 4:8, :], ADD
                )
                xccn = fp.tile([128, 4, 128], bf16, tag="xccn", name="xccn")
                nc.vector.tensor_tensor(
                    xccn[:], xcc[:, 0:4, :], xcc[:, 4:8, :], SUB
                )
                xcn = fp.tile([128, 8, 128], bf16, tag="xcn", name="xcn")
                nc.vector.tensor_tensor(xcn[:], xc[:, 0:8, :], xc[:, 8:16, :], SUB)
                return dict(xn=xn, xcn=xcn, xccc=xccc, xccn=xccn)

            def emit_folds_b(fa):
                """Late folds: trinomial xTp/xTm (needed only by the T
                matmuls, ~5us into the block). sqrt2 scales on ACT."""
                xn = fa["xn"]
                xA = fp.tile([128, 4, 128], bf16, tag="xA", name="xA")
                nc.vector.tensor_tensor(xA[:], xn[:, 0:4, :], xn[:, 8:12, :], SUB)
                xB = fp.tile([128, 4, 128], bf16, tag="xB", name="xB")
                nc.vector.tensor_tensor(xB[:], xn[:, 4:8, :], xn[:, 12:16, :], ADD)
                s10 = fp.tile([128, 4, 128], bf16, tag="s10", name="s10")
                nc.scalar.mul(s10[:], xn[:, 8:12, :], R2)
                s11 = fp.tile([128, 4, 128], bf16, tag="s11", name="s11")
                nc.scalar.mul(s11[:], xn[:, 12:16, :], R2)
                xTp = fp.tile([128, 8, 128], bf16, tag="xTp", name="xTp")
                nc.vector.tensor_tensor(xTp[:, 0:4, :], xA[:], s11[:], ADD)
                nc.vector.tensor_tensor(xTp[:, 4:8, :], xB[:], s10[:], SUB)
                xTm = fp.tile([128, 8, 128], bf16, tag="xTm", name="xTm")
                nc.vector.tensor_tensor(xTm[:, 0:4, :], xA[:], s11[:], SUB)
                nc.vector.tensor_tensor(xTm[:, 4:8, :], xB[:], s10[:], ADD)
                fa["xTp"] = xTp
                fa["xTm"] = xTm
                return fa

            def emit_folds(xbig, first=False):
                # Block 0 only: fold each interleaved DMA half as it lands;
                # xc on DVE (the cyc chain gates the first matmuls).
                xc = fp.tile([128, 16, 128], bf16, tag="xc", name="xc")
                xn = fp.tile([128, 16, 128], bf16, tag="xn", name="xn")
                for h in range(2):
                    nc.vector.tensor_tensor(
                        xc[:, 8 * h : 8 * h + 8, :],
                        xbig[:, 16 * h : 16 * h + 16 : 2, :],
                        xbig[:, 16 * h + 1 : 16 * h + 16 : 2, :], ADD
                    )
                    nc.gpsimd.tensor_tensor(
                        xn[:, 8 * h : 8 * h + 8, :],
                        xbig[:, 16 * h : 16 * h + 16 : 2, :],
                        xbig[:, 16 * h + 1 : 16 * h + 16 : 2, :], SUB
                    )
                # L2/L3 binomial on xc (DVE); C5/N5 operands first, xcn
                # (needed only by the mid-block N10 matmuls) last
                xcc = fp.tile([128, 8, 128], bf16, tag="xcc", name="xcc")
                nc.vector.tensor_tensor(xcc[:], xc[:, 0:8, :], xc[:, 8:16, :], ADD)
                xccc = fp.tile([128, 4, 128], bf16, tag="xccc", name="xccc")
                nc.vector.tensor_tensor(
                    xccc[:], xcc[:, 0:4, :], xcc[:, 4:8, :], ADD
                )
                xccn = fp.tile([128, 4, 128], bf16, tag="xccn", name="xccn")
                nc.vector.tensor_tensor(
                    xccn[:], xcc[:, 0:4, :], xcc[:, 4:8, :], SUB
                )
                xcn = fp.tile([128, 8, 128], bf16, tag="xcn", name="xcn")
                nc.vector.tensor_tensor(xcn[:], xc[:, 0:8, :], xc[:, 8:16, :], SUB)
                # trinomial fold of xn -> xTp, xTm (sqrt2 scales on ACT)
                xA = fp.tile([128, 4, 128], bf16, tag="xA", name="xA")
                nc.vector.tensor_tensor(xA[:], xn[:, 0:4, :], xn[:, 8:12, :], SUB)
                xB = fp.tile([128, 4, 128], bf16, tag="xB", name="xB")
                nc.vector.tensor_tensor(xB[:], xn[:, 4:8, :], xn[:, 12:16, :], ADD)
                s10 = fp.tile([128, 4, 128], bf16, tag="s10", name="s10")
                nc.scalar.mul(s10[:], xn[:, 8:12, :], R2)
                s11 = fp.tile([128, 4, 128], bf16, tag="s11", name="s11")
                nc.scalar.mul(s11[:], xn[:, 12:16, :], R2)
                xTp = fp.tile([128, 8, 128], bf16, tag="xTp", name="xTp")
                nc.vector.tensor_tensor(xTp[:, 0:4, :], xA[:], s11[:], ADD)
                nc.vector.tensor_tensor(xTp[:, 4:8, :], xB[:], s10[:], SUB)
                xTm = fp.tile([128, 8, 128], bf16, tag="xTm", name="xTm")
                nc.vector.tensor_tensor(xTm[:, 0:4, :], xA[:], s11[:], SUB)
                nc.vector.tensor_tensor(xTm[:, 4:8, :], xB[:], s10[:], ADD)
                return dict(xcn=xcn, xTp=xTp, xTm=xTm, xccc=xccc, xccn=xccn)

            def mm_ring(psum, xres, sname, nchunks):
                s = strip[sname]
                for a in range(nchunks):
                    v0 = 128 * (nchunks - 1) - 128 * a
                    nc.tensor.matmul(
                        psum[:], xres[:, a, :], s[:, v0 : v0 + 512],
                        start=(a == 0), stop=(a == nchunks - 1),
                    )

            def mms_cn5(f):
                pC5 = pp.tile([128, 512], f32, tag="pc5", name="pc5")
                mm_ring(pC5, f["xccc"], "sC5", 4)
                pN5 = pp.tile([128, 512], f32, tag="pn5", name="pn5")
                mm_ring(pN5, f["xccn"], "sN5", 4)
                rCN5 = ep.tile([128, 1024], bf16, tag="rcn5", name="rcn5")
                nc.scalar.mul(rCN5[:, 0:512], pC5[:], 0.125)
                nc.scalar.mul(rCN5[:, 512:1024], pN5[:], 0.125)
                return rCN5

            def mms_n10(f):
                """nega1024 in two half-PSUM tiles (tail streams per half)."""
                pN10a = pp.tile([128, 512], f32, tag="pn10a", name="pn10a")
                pN10b = pp.tile([128, 512], f32, tag="pn10b", name="pn10b")
                sN10 = strip["sN10"]
                rN10 = ep.tile([128, 1024], bf16, tag="rn10", name="rn10")
                for j, ps in ((0, pN10a), (1, pN10b)):
                    for a in range(8):
                        v0 = 896 - 128 * a + 512 * j
                        nc.tensor.matmul(
                            ps[:], f["xcn"][:, a, :], sN10[:, v0 : v0 + 512],
                            start=(a == 0), stop=(a == 7),
                        )
                    nc.scalar.mul(rN10[:, 512 * j : 512 * j + 512], ps[:], 0.25)
                return rN10

            def mms_t(f):
                rTp = ep.tile([128, 1024], bf16, tag="rtp", name="rtp")
                rTm = ep.tile([128, 1024], bf16, tag="rtm", name="rtm")
                for xres, rr, lo_name, hi_name in (
                    (f["xTp"], rTp, "sLp", "sHp"),
                    (f["xTm"], rTm, "sLm", "sHm"),
                ):
                    for j, sname in ((0, lo_name), (1, hi_name)):
                        ph = pp.tile(
                            [128, 512], f32, tag=f"pt{sname}", name=f"pt{sname}"
                        )
                        mm_ring(ph, xres, sname, 8)
                        nc.scalar.mul(
                            rr[:, 512 * j : 512 * j + 512], ph[:], SC_T
                        )
                return rTp, rTm

            def emit_rn_chain(rTp, rTm):
                """nega2048 reconstruction from T1024+/- (DVE + ACT scale)."""
                rn = up.tile([128, 2048], bf16, tag="rn", name="rn")
                tt01 = up.tile([128, 1024], bf16, tag="tt01", name="tt01")
                st01 = up.tile([128, 1024], bf16, tag="st01", name="st01")
                nc.vector.tensor_tensor(  # o3
                    rn[:, 1536:2048], rTp[:, 0:512], rTm[:, 0:512], SUB
                )
                nc.vector.tensor_tensor(  # t0
                    tt01[:, 0:512], rTp[:, 0:512], rTm[:, 0:512], ADD
                )
                nc.vector.tensor_tensor(  # o2
                    rn[:, 1024:1536], rTm[:, 512:1024], rTp[:, 512:1024], SUB
                )
                nc.vector.tensor_tensor(  # t1
                    tt01[:, 512:1024], rTp[:, 512:1024], rTm[:, 512:1024], ADD
                )
                nc.scalar.mul(st01[:], tt01[:], R2)
                nc.vector.tensor_tensor(  # o0
                    rn[:, 0:512], st01[:, 0:512], rn[:, 1024:1536], ADD
                )
                nc.vector.tensor_tensor(  # o1
                    rn[:, 512:1024], st01[:, 512:1024], rn[:, 1536:2048], SUB
                )
                return rn

            def emit_rc_part(rCN5, rN10):
                rc1 = up.tile([128, 1024], bf16, tag="rc1", name="rc1")
                nc.vector.tensor_tensor(
                    rc1[:, 0:512], rCN5[:, 0:512], rCN5[:, 512:1024], ADD
                )
                nc.vector.tensor_tensor(
                    rc1[:, 512:1024], rCN5[:, 0:512], rCN5[:, 512:1024], SUB
                )
                rc2 = up.tile([128, 2048], bf16, tag="rc2", name="rc2")
                nc.vector.tensor_tensor(rc2[:, 0:1024], rc1[:], rN10[:], ADD)
                nc.vector.tensor_tensor(rc2[:, 1024:2048], rc1[:], rN10[:], SUB)
                return rc2

            def emit_finals(bt, rc2, rn, halves=False):
                b0 = 128 * bt
                specs = ((ADD, 0, "D"), (ADD, 1024, "P"), (SUB, 0, "D"), (SUB, 1024, "P"))
                os_ = [
                    op.tile([128, 1024], f32, tag=f"o{seg}", name=f"o{seg}")
                    for seg in range(4)
                ]
                nh = 2 if halves else 1
                w = 1024 // nh
                for h in range(nh):
                    c0 = w * h
                    for seg, (alu, lo, eng) in enumerate(specs):
                        oh = os_[seg][:, c0 : c0 + w]
                        if eng == "D":
                            nc.vector.tensor_tensor(
                                oh, rc2[:, lo + c0 : lo + c0 + w],
                                rn[:, lo + c0 : lo + c0 + w], alu
                            )
                        else:
                            nc.gpsimd.tensor_tensor(
                                oh, rc2[:, lo + c0 : lo + c0 + w],
                                rn[:, lo + c0 : lo + c0 + w], alu
                            )
                        nc.sync.dma_start(
                            out_d[b0 : b0 + 128, 1024 * seg + c0 : 1024 * seg + c0 + w],
                            oh,
                        )

            def emit_unfold(bt, rCN5, rN10, rTp, rTm, halves=False):
                rn = emit_rn_chain(rTp, rTm)
                rc2 = emit_rc_part(rCN5, rN10)
                emit_finals(bt, rc2, rn, halves=halves)

            def emit_tail_block(f, bt):
                """Last block: T matmuls first (longest reconstruction chain),
                C5/N5 next, nega1024 last in two halves; pre-combines
                preN0 = rc1+rN10h, preN1 = rc1-rN10h feed two finals each, and
                each output half streams out as its rN10 half evacuates."""
                b0 = 128 * bt
                rTp, rTm = mms_t(f)
                rCN5 = mms_cn5(f)
                rn = emit_rn_chain(rTp, rTm)
                rc1 = up.tile([128, 1024], bf16, tag="rc1", name="rc1")
                nc.vector.tensor_tensor(
                    rc1[:, 0:512], rCN5[:, 0:512], rCN5[:, 512:1024], ADD
                )
                nc.vector.tensor_tensor(
                    rc1[:, 512:1024], rCN5[:, 0:512], rCN5[:, 512:1024], SUB
                )
                pN10a = pp.tile([128, 512], f32, tag="pn10a", name="pn10a")
                pN10b = pp.tile([128, 512], f32, tag="pn10b", name="pn10b")
                sN10 = strip["sN10"]
                for j, ps in ((0, pN10a), (1, pN10b)):
                    for a in range(8):
                        v0 = 896 - 128 * a + 512 * j
                        nc.tensor.matmul(
                            ps[:], f["xcn"][:, a, :], sN10[:, v0 : v0 + 512],
                            start=(a == 0), stop=(a == 7),
                        )
                preN = up.tile([128, 2, 1024], bf16, tag="preN", name="preN")
                rN10 = ep.tile([128, 1024], bf16, tag="rn10", name="rn10")
                os_ = [
                    op.tile([128, 1024], f32, tag=f"o{seg}", name=f"o{seg}")
                    for seg in range(4)
                ]
                for h, ps in ((0, pN10a), (1, pN10b)):
                    c0 = 512 * h
                    rh = rN10[:, c0 : c0 + 512]
                    nc.scalar.mul(rh, ps[:], 0.25)
                    nc.vector.tensor_tensor(  # preN0 half
                        preN[:, 0, c0 : c0 + 512], rc1[:, c0 : c0 + 512], rh, ADD
                    )
                    nc.vector.tensor_tensor(  # preN1 half
                        preN[:, 1, c0 : c0 + 512], rc1[:, c0 : c0 + 512], rh, SUB
                    )
                    for seg, eng in ((0, "D"), (1, "P"), (2, "D"), (3, "P")):
                        alu = ADD if seg in (0, 1) else SUB
                        rn_half = rn[:, 1024 * (seg % 2) + c0 : 1024 * (seg % 2) + c0 + 512]
                        oh = os_[seg][:, c0 : c0 + 512]
                        if eng == "D":
                            nc.vector.tensor_tensor(
                                oh, preN[:, seg % 2, c0 : c0 + 512], rn_half, alu
                            )
                        else:
                            nc.gpsimd.tensor_tensor(
                                oh, preN[:, seg % 2, c0 : c0 + 512], rn_half, alu
                            )
                        nc.sync.dma_start(
                            out_d[b0 : b0 + 128, 1024 * seg + c0 : 1024 * seg + c0 + 512],
                            oh,
                        )

            # ---------------- preamble ----------------
            nc.gpsimd.memset(warm[:], 0.0)
            xbig = xbigp.tile([128, 32, 128], f32, tag="xbig", name="xbig")
            nc.sync.dma_start(xbig[:, 0:16, :], xtr_t[:, 0:16, 0:128])
            nc.sync.dma_start(xbig[:, 16:32, :], xtr_t[:, 16:32, 0:128])
            nc.sync.dma_start(SB[:, 0:1792], sb_d[:, 0:1792])           # sC5+sN5
            nc.sync.dma_start(SB[:, 1792:SB_SPLIT], sb_d[:, 1792:SB_SPLIT])  # sN10
            nc.sync.dma_start(SB[:, SB_SPLIT:SBW], sb_d[:, SB_SPLIT:SBW])
            # PE clock warmup: HAM releases 2.4 GHz after ~3us of activity;
            # burn dummies while the first DMAs/folds land.
            pW = pp.tile([128, 512], f32, tag="pc5", name="pwarm")
            for _ in range(25):
                nc.tensor.matmul(
                    pW[:], warm[:, 0:128], warm[:], start=True, stop=True
                )

            # ---------------- main pipeline ----------------
            # Iteration bt interleaves emissions so each engine's in-order
            # stream matches when its work becomes runnable:
            #   mms+evacs(bt) | rn-chain(bt-1) | early folds(bt+1) |
            #   rc-part(bt-1) | T-folds(bt+1) | finals(bt-1)
            f_cur = emit_folds(xbig, first=True)
            xbig_next = emit_dma_in(1)
            r_prev = None
            for bt in range(NB - 1):
                xbig = xbig_next
                if bt + 2 < NB:
                    xbig_next = emit_dma_in(bt + 2)
                rCN5 = mms_cn5(f_cur)
                rN10 = mms_n10(f_cur)
                rTp, rTm = mms_t(f_cur)
                if r_prev is not None:
                    rn = emit_rn_chain(r_prev[2], r_prev[3])
                fa = emit_folds_a(xbig, swap=(bt == 0))
                if r_prev is not None:
                    rc2 = emit_rc_part(r_prev[0], r_prev[1])
                f_cur = emit_folds_b(fa)
                if r_prev is not None:
                    emit_finals(bt - 1, rc2, rn)
                r_prev = (rCN5, rN10, rTp, rTm)
            emit_unfold(NB - 2, *r_prev)
            emit_tail_block(f_cur, NB - 1)

    nc.compile()
    return nc


def _get_nc():
    if "nc" not in _STATE:
        _STATE["nc"] = _build()
    return _STATE["nc"]


def _make_strip_table(w):
    """All 7 leaf band strips, packed [128, SBW] bf16.

    Strip tiles are shears: tile[p, v] = S[OFF + v - p]. Sequences (t any int,
    Ecyc = wt cyclic):
      Ep(t)   = Ecyc(t) + Ecyc(t+2048)          cyc2048
      En(t)   = Ecyc(t) - Ecyc(t+2048)          nega2048
      En10(t) = Ep(t) - Ep(t+1024)              nega1024 leaf
      Epp(t)  = Ep(t) + Ep(t+1024)              cyc1024
      Ec5(t)  = Epp(t) + Epp(t+512)             cyc512 leaf
      En5(t)  = Epp(t) - Epp(t+512)             nega512 leaf
      D(t) = En(t) - En(t+1024); Bt(t) = En(t+512) + En(t+1536)
      L+/-(t) = D(t) +- sqrt2 En(t+1536)        T1024 low strips
      H+/-(t) = Bt(t) -+ sqrt2 En(t+1024)       T1024 high strips
    CRT scales (1/8, 1/8, 1/4, 1/(4sqrt2)) are applied in the ACT evacs.
    """
    import ml_dtypes

    wt = np.roll(w[::-1], 1).astype(np.float64)
    Ecyc = lambda t: wt[np.mod(t, N)]
    Ep = lambda t: Ecyc(t) + Ecyc(t + 2048)
    En = lambda t: Ecyc(t) - Ecyc(t + 2048)
    Epp = lambda t: Ep(t) + Ep(t + 1024)
    seqs = {
        "sC5": lambda t: Epp(t) + Epp(t + 512),
        "sN5": lambda t: Epp(t) - Epp(t + 512),
        "sN10": lambda t: Ep(t) - Ep(t + 1024),
        "sLp": lambda t: (En(t) - En(t + 1024)) + R2 * En(t + 1536),
        "sLm": lambda t: (En(t) - En(t + 1024)) - R2 * En(t + 1536),
        "sHp": lambda t: (En(t + 512) + En(t + 1536)) - R2 * En(t + 1024),
        "sHm": lambda t: (En(t + 512) + En(t + 1536)) + R2 * En(t + 1024),
    }
    p = np.arange(128)[:, None]
    tab = np.zeros((128, SBW), dtype=np.float64)
    for name, (c0, width, off) in STRIPS.items():
        v = np.arange(width)[None, :]
        tab[:, c0 : c0 + width] = seqs[name](off + v - p)
    return np.ascontiguousarray(tab.astype(ml_dtypes.bfloat16))


def _prep_inputs(x, w):
    x = np.ascontiguousarray(x, dtype=np.float32)
    w = np.ascontiguousarray(w, dtype=np.float32)
    sb = _make_strip_table(w)
    # chunk-interleaved layout: position 2t holds ring chunk t, position
    # 2t+1 holds chunk t+16 — L1 fold pairs become stride-2 neighbors, so
    # block 0 can fold each DMA half as it lands.
    perm = [(i >> 1) + 16 * (i & 1) for i in range(32)]
    in_maps = []
    for i in range(N_CORES):
        xtr = np.ascontiguousarray(x[i * B_SHARD : (i + 1) * B_SHARD].T)
        xtr = np.ascontiguousarray(
            xtr.reshape(32, 128, B_SHARD)[perm].reshape(N, B_SHARD)
        )
        in_maps.append({"xtr": xtr, "sb": sb})
    return in_maps


def kernel(x, w, _trace=False):
    from concourse.bass_utils import run_bass_kernel_spmd

    nc = _get_nc()
    in_maps = _prep_inputs(x, w)
    res = run_bass_kernel_spmd(nc, in_maps, list(range(N_CORES)), trace=_trace)
    out = np.concatenate([res.results[i]["out"] for i in range(N_CORES)], axis=0)
    if _trace:
        _STATE["last_result"] = res
    return out



# revision 7
# speedup vs baseline: 1.0752x; 1.0752x over previous
"""Circulant matmul for TRN2: 4-level CRT with trinomial split, bf16 matmuls.

out[b, r] = sum_c x[b,c] w[(c-r) mod N] = (x (*) wt)[r], wt = roll(w[::-1],1)
(cyclic convolution along c) — no input flip or output reversal needed.

CRT tree on z^4096-1 (per 128-row block of x):
  cyc4096 -> cyc2048 + nega2048             (binomial, band scale 1/2 each)
  cyc2048 -> cyc1024 + nega1024[LEAF]       (1/2)
  cyc1024 -> cyc512[LEAF] + nega512[LEAF]   (1/2)
  nega2048 -> T1024+ , T1024- [LEAVES]      (z^1024 -+ sqrt2 z^512 + 1, 1/(2sqrt2))
Leaf matmuls: 56 x [K=128, M=128, N=512] bf16 = 28672 PE cycles/block
(vs 88/45056 for the 3-level f32r baseline). Reconstruction scales fold into
the ACT PSUM->SBUF evacuations; sqrt2 scale passes run on ACT. Ring bands are
(piecewise) shear tensors S[q - k]; the host precomputes all 7 leaf strips
from w into one [128, 9344] bf16 table (2.4 MB) — no on-chip band build.

Per-block engine budget: PE 11.95us (critical), DMA 11.65, DVE ~11
(bf16 2x tensor_tensor), Pool ~10.3 (plain tensor_tensor only — the
compiler rejects scalar_tensor_tensor/tensor_scalar on Pool), ACT ~7.
Evacs interleave with the matmul stream (PSUM banks recycle without
stalling PE); the last block runs T matmuls first and nega1024 last in
two half-PSUM tiles so the reconstruction tail streams out per half.
"""

import sys

sys.path.insert(0, "/opt/trn_rl_repo")

import numpy as np

N = 4096
B = 8192
N_CORES = 8
B_SHARD = B // N_CORES  # 1024
NB = B_SHARD // 128     # 8 row-tiles per core
R2 = float(np.sqrt(2.0))
SC_T = float(1.0 / (4.0 * np.sqrt(2.0)))   # T1024 leaves: 1/2 * 1/(2sqrt2)

# strip layout in the packed [128, SBW] host table (name: col0, width, OFF)
STRIPS = {
    "sC5": (0, 896, -384),
    "sN5": (896, 896, -384),
    "sN10": (1792, 1920, -896),
    "sLp": (3712, 1408, -896),
    "sHp": (5120, 1408, -896),
    "sLm": (6528, 1408, -896),
    "sHm": (7936, 1408, -896),
}
SBW = 9344
SB_SPLIT = 3712  # first DMA: C5/N5/N10 strips; second: T strips

_STATE = {}

N_WARM = 10


def _build():
    import concourse.bacc as bacc
    import concourse.mybir as mybir
    import concourse.tile as tile

    f32 = mybir.dt.float32
    bf16 = mybir.dt.bfloat16
    ADD = mybir.AluOpType.add
    SUB = mybir.AluOpType.subtract

    nc = bacc.Bacc("TRN2", target_bir_lowering=False, debug=False)
    # x transposed+permuted+bf16 on host, laid out [p, bblk, a, bwithin] so
    # each 128-row block is one 8KB-contiguous run per partition.
    xtr_d = nc.declare_dram_parameter("xtr", [128, NB, 32, 128], bf16, isOutput=False)
    sb_d = nc.declare_dram_parameter("sb", [128, SBW], bf16, isOutput=False)
    # bf16 output staging; host upcasts to f32 after gather.
    out_d = nc.declare_dram_parameter("out", [B_SHARD, N], bf16, isOutput=True)

    with tile.TileContext(nc) as tc:
        with (
            tc.tile_pool(name="const", bufs=1) as constp,
            tc.tile_pool(name="xbig", bufs=3) as xbigp,
            tc.tile_pool(name="fold", bufs=2) as fp,
            tc.tile_pool(name="evac", bufs=3) as ep,
            tc.tile_pool(name="unf", bufs=2) as up,
            tc.tile_pool(name="outp", bufs=2) as op,
            tc.tile_pool(name="psum", bufs=1, space="PSUM") as pp,
        ):
            SB = constp.tile([128, SBW], bf16, name="SB")
            warm = constp.tile([128, 512], bf16, name="warm")
            strip = {k: SB[:, c0 : c0 + w] for k, (c0, w, _) in STRIPS.items()}

            def emit_dma_in(bt):
                xbig = xbigp.tile([128, 32, 128], bf16, tag="xbig", name="xbig")
                nc.sync.dma_start(xbig[:], xtr_d[:, bt, :, :])
                return xbig

            def emit_folds_a(xbig, swap=False):
                """Early folds: xc (Pool), binomial chain (DVE) — these feed
                the first matmuls of the next block. swap=True runs xc on DVE
                instead (fill phase: xc gates the chain and Pool is 2x slower
                at the mixed L1 op)."""
                xc = fp.tile([128, 16, 128], bf16, tag="xc", name="xc")
                xn = fp.tile([128, 16, 128], bf16, tag="xn", name="xn")
                if swap:
                    nc.vector.tensor_tensor(
                        xc[:], xbig[:, 0:32:2, :], xbig[:, 1:32:2, :], ADD
                    )
                    nc.gpsimd.tensor_tensor(
                        xn[:], xbig[:, 0:32:2, :], xbig[:, 1:32:2, :], SUB
                    )
                else:
                    nc.gpsimd.tensor_tensor(
                        xc[:], xbig[:, 0:32:2, :], xbig[:, 1:32:2, :], ADD
                    )
                    nc.vector.tensor_tensor(
                        xn[:], xbig[:, 0:32:2, :], xbig[:, 1:32:2, :], SUB
                    )
                xcc = fp.tile([128, 8, 128], bf16, tag="xcc", name="xcc")
                nc.vector.tensor_tensor(xcc[:], xc[:, 0:8, :], xc[:, 8:16, :], ADD)
                xccc = fp.tile([128, 4, 128], bf16, tag="xccc", name="xccc")
                nc.vector.tensor_tensor(
                    xccc[:], xcc[:, 0:4, :], xcc[:, 4:8, :], ADD
                )
                xccn = fp.tile([128, 4, 128], bf16, tag="xccn", name="xccn")
                nc.vector.tensor_tensor(
                    xccn[:], xcc[:, 0:4, :], xcc[:, 4:8, :], SUB
                )
                xcn = fp.tile([128, 8, 128], bf16, tag="xcn", name="xcn")
                nc.vector.tensor_tensor(xcn[:], xc[:, 0:8, :], xc[:, 8:16, :], SUB)
                return dict(xn=xn, xcn=xcn, xccc=xccc, xccn=xccn)

            def emit_folds_b(fa):
                """Late folds: trinomial xTp/xTm (needed only by the T
                matmuls, ~5us into the block). sqrt2 scales on ACT."""
                xn = fa["xn"]
                xA = fp.tile([128, 4, 128], bf16, tag="xA", name="xA")
                nc.vector.tensor_tensor(xA[:], xn[:, 0:4, :], xn[:, 8:12, :], SUB)
                xB = fp.tile([128, 4, 128], bf16, tag="xB", name="xB")
                nc.vector.tensor_tensor(xB[:], xn[:, 4:8, :], xn[:, 12:16, :], ADD)
                s10 = fp.tile([128, 4, 128], bf16, tag="s10", name="s10")
                nc.scalar.mul(s10[:], xn[:, 8:12, :], R2)
                s11 = fp.tile([128, 4, 128], bf16, tag="s11", name="s11")
                nc.scalar.mul(s11[:], xn[:, 12:16, :], R2)
                xTp = fp.tile([128, 8, 128], bf16, tag="xTp", name="xTp")
                nc.vector.tensor_tensor(xTp[:, 0:4, :], xA[:], s11[:], ADD)
                nc.vector.tensor_tensor(xTp[:, 4:8, :], xB[:], s10[:], SUB)
                xTm = fp.tile([128, 8, 128], bf16, tag="xTm", name="xTm")
                nc.vector.tensor_tensor(xTm[:, 0:4, :], xA[:], s11[:], SUB)
                nc.vector.tensor_tensor(xTm[:, 4:8, :], xB[:], s10[:], ADD)
                fa["xTp"] = xTp
                fa["xTm"] = xTm
                return fa

            def emit_folds(xbig, first=False):
                # Block 0 only: fold each interleaved DMA half as it lands;
                # xc on DVE (the cyc chain gates the first matmuls).
                xc = fp.tile([128, 16, 128], bf16, tag="xc", name="xc")
                xn = fp.tile([128, 16, 128], bf16, tag="xn", name="xn")
                for h in range(2):
                    nc.vector.tensor_tensor(
                        xc[:, 8 * h : 8 * h + 8, :],
                        xbig[:, 16 * h : 16 * h + 16 : 2, :],
                        xbig[:, 16 * h + 1 : 16 * h + 16 : 2, :], ADD
                    )
                    nc.gpsimd.tensor_tensor(
                        xn[:, 8 * h : 8 * h + 8, :],
                        xbig[:, 16 * h : 16 * h + 16 : 2, :],
                        xbig[:, 16 * h + 1 : 16 * h + 16 : 2, :], SUB
                    )
                # L2/L3 binomial on xc (DVE); C5/N5 operands first, xcn
                # (needed only by the mid-block N10 matmuls) last
                xcc = fp.tile([128, 8, 128], bf16, tag="xcc", name="xcc")
                nc.vector.tensor_tensor(xcc[:], xc[:, 0:8, :], xc[:, 8:16, :], ADD)
                xccc = fp.tile([128, 4, 128], bf16, tag="xccc", name="xccc")
                nc.vector.tensor_tensor(
                    xccc[:], xcc[:, 0:4, :], xcc[:, 4:8, :], ADD
                )
                xccn = fp.tile([128, 4, 128], bf16, tag="xccn", name="xccn")
                nc.vector.tensor_tensor(
                    xccn[:], xcc[:, 0:4, :], xcc[:, 4:8, :], SUB
                )
                xcn = fp.tile([128, 8, 128], bf16, tag="xcn", name="xcn")
                nc.vector.tensor_tensor(xcn[:], xc[:, 0:8, :], xc[:, 8:16, :], SUB)
                # trinomial fold of xn -> xTp, xTm (sqrt2 scales on ACT)
                xA = fp.tile([128, 4, 128], bf16, tag="xA", name="xA")
                nc.vector.tensor_tensor(xA[:], xn[:, 0:4, :], xn[:, 8:12, :], SUB)
                xB = fp.tile([128, 4, 128], bf16, tag="xB", name="xB")
                nc.vector.tensor_tensor(xB[:], xn[:, 4:8, :], xn[:, 12:16, :], ADD)
                s10 = fp.tile([128, 4, 128], bf16, tag="s10", name="s10")
                nc.scalar.mul(s10[:], xn[:, 8:12, :], R2)
                s11 = fp.tile([128, 4, 128], bf16, tag="s11", name="s11")
                nc.scalar.mul(s11[:], xn[:, 12:16, :], R2)
                xTp = fp.tile([128, 8, 128], bf16, tag="xTp", name="xTp")
                nc.vector.tensor_tensor(xTp[:, 0:4, :], xA[:], s11[:], ADD)
                nc.vector.tensor_tensor(xTp[:, 4:8, :], xB[:], s10[:], SUB)
                xTm = fp.tile([128, 8, 128], bf16, tag="xTm", name="xTm")
                nc.vector.tensor_tensor(xTm[:, 0:4, :], xA[:], s11[:], SUB)
                nc.vector.tensor_tensor(xTm[:, 4:8, :], xB[:], s10[:], ADD)
                return dict(xcn=xcn, xTp=xTp, xTm=xTm, xccc=xccc, xccn=xccn)

            def mm_ring(psum, xres, sname, nchunks):
                s = strip[sname]
                for a in range(nchunks):
                    v0 = 128 * (nchunks - 1) - 128 * a
                    nc.tensor.matmul(
                        psum[:], xres[:, a, :], s[:, v0 : v0 + 512],
                        start=(a == 0), stop=(a == nchunks - 1),
                    )

            def mms_cn5(f):
                pC5 = pp.tile([128, 512], f32, tag="pc5", name="pc5")
                mm_ring(pC5, f["xccc"], "sC5", 4)
                pN5 = pp.tile([128, 512], f32, tag="pn5", name="pn5")
                mm_ring(pN5, f["xccn"], "sN5", 4)
                rCN5 = ep.tile([128, 1024], bf16, tag="rcn5", name="rcn5")
                nc.scalar.mul(rCN5[:, 0:512], pC5[:], 0.125)
                nc.scalar.mul(rCN5[:, 512:1024], pN5[:], 0.125)
                return rCN5

            def mms_n10(f):
                """nega1024 in two half-PSUM tiles (tail streams per half)."""
                pN10a = pp.tile([128, 512], f32, tag="pn10a", name="pn10a")
                pN10b = pp.tile([128, 512], f32, tag="pn10b", name="pn10b")
                sN10 = strip["sN10"]
                rN10 = ep.tile([128, 1024], bf16, tag="rn10", name="rn10")
                for j, ps in ((0, pN10a), (1, pN10b)):
                    for a in range(8):
                        v0 = 896 - 128 * a + 512 * j
                        nc.tensor.matmul(
                            ps[:], f["xcn"][:, a, :], sN10[:, v0 : v0 + 512],
                            start=(a == 0), stop=(a == 7),
                        )
                    nc.scalar.mul(rN10[:, 512 * j : 512 * j + 512], ps[:], 0.25)
                return rN10

            def mms_t(f):
                rTp = ep.tile([128, 1024], bf16, tag="rtp", name="rtp")
                rTm = ep.tile([128, 1024], bf16, tag="rtm", name="rtm")
                for xres, rr, lo_name, hi_name in (
                    (f["xTp"], rTp, "sLp", "sHp"),
                    (f["xTm"], rTm, "sLm", "sHm"),
                ):
                    for j, sname in ((0, lo_name), (1, hi_name)):
                        ph = pp.tile(
                            [128, 512], f32, tag=f"pt{sname}", name=f"pt{sname}"
                        )
                        mm_ring(ph, xres, sname, 8)
                        nc.scalar.mul(
                            rr[:, 512 * j : 512 * j + 512], ph[:], SC_T
                        )
                return rTp, rTm

            def emit_rn_chain(rTp, rTm):
                """nega2048 reconstruction from T1024+/- (DVE + ACT scale)."""
                rn = up.tile([128, 2048], bf16, tag="rn", name="rn")
                tt01 = up.tile([128, 1024], bf16, tag="tt01", name="tt01")
                st01 = up.tile([128, 1024], bf16, tag="st01", name="st01")
                nc.vector.tensor_tensor(  # o3
                    rn[:, 1536:2048], rTp[:, 0:512], rTm[:, 0:512], SUB
                )
                nc.vector.tensor_tensor(  # t0
                    tt01[:, 0:512], rTp[:, 0:512], rTm[:, 0:512], ADD
                )
                nc.vector.tensor_tensor(  # o2
                    rn[:, 1024:1536], rTm[:, 512:1024], rTp[:, 512:1024], SUB
                )
                nc.vector.tensor_tensor(  # t1
                    tt01[:, 512:1024], rTp[:, 512:1024], rTm[:, 512:1024], ADD
                )
                nc.scalar.mul(st01[:], tt01[:], R2)
                nc.vector.tensor_tensor(  # o0
                    rn[:, 0:512], st01[:, 0:512], rn[:, 1024:1536], ADD
                )
                nc.vector.tensor_tensor(  # o1
                    rn[:, 512:1024], st01[:, 512:1024], rn[:, 1536:2048], SUB
                )
                return rn

            def emit_rc_part(rCN5, rN10):
                rc1 = up.tile([128, 1024], bf16, tag="rc1", name="rc1")
                nc.vector.tensor_tensor(
                    rc1[:, 0:512], rCN5[:, 0:512], rCN5[:, 512:1024], ADD
                )
                nc.vector.tensor_tensor(
                    rc1[:, 512:1024], rCN5[:, 0:512], rCN5[:, 512:1024], SUB
                )
                rc2 = up.tile([128, 2048], bf16, tag="rc2", name="rc2")
                nc.vector.tensor_tensor(rc2[:, 0:1024], rc1[:], rN10[:], ADD)
                nc.vector.tensor_tensor(rc2[:, 1024:2048], rc1[:], rN10[:], SUB)
                return rc2

            def emit_finals(bt, rc2, rn, halves=False):
                b0 = 128 * bt
                specs = ((ADD, 0, "D"), (ADD, 1024, "P"), (SUB, 0, "D"), (SUB, 1024, "P"))
                os_ = [
                    op.tile([128, 1024], bf16, tag=f"o{seg}", name=f"o{seg}")
                    for seg in range(4)
                ]
                nh = 2 if halves else 1
                w = 1024 // nh
                for h in range(nh):
                    c0 = w * h
                    for seg, (alu, lo, eng) in enumerate(specs):
                        oh = os_[seg][:, c0 : c0 + w]
                        if eng == "D":
                            nc.vector.tensor_tensor(
                                oh, rc2[:, lo + c0 : lo + c0 + w],
                                rn[:, lo + c0 : lo + c0 + w], alu
                            )
                        else:
                            nc.gpsimd.tensor_tensor(
                                oh, rc2[:, lo + c0 : lo + c0 + w],
                                rn[:, lo + c0 : lo + c0 + w], alu
                            )
                        nc.sync.dma_start(
                            out_d[b0 : b0 + 128, 1024 * seg + c0 : 1024 * seg + c0 + w],
                            oh,
                        )

            def emit_unfold(bt, rCN5, rN10, rTp, rTm, halves=False):
                rn = emit_rn_chain(rTp, rTm)
                rc2 = emit_rc_part(rCN5, rN10)
                emit_finals(bt, rc2, rn, halves=halves)

            def emit_tail_block(f, bt):
                """Last block: T matmuls first (longest reconstruction chain),
                C5/N5 next, nega1024 last in two halves; pre-combines
                preN0 = rc1+rN10h, preN1 = rc1-rN10h feed two finals each, and
                each output half streams out as its rN10 half evacuates."""
                b0 = 128 * bt
                rTp, rTm = mms_t(f)
                rCN5 = mms_cn5(f)
                rn = emit_rn_chain(rTp, rTm)
                rc1 = up.tile([128, 1024], bf16, tag="rc1", name="rc1")
                nc.vector.tensor_tensor(
                    rc1[:, 0:512], rCN5[:, 0:512], rCN5[:, 512:1024], ADD
                )
                nc.vector.tensor_tensor(
                    rc1[:, 512:1024], rCN5[:, 0:512], rCN5[:, 512:1024], SUB
                )
                pN10a = pp.tile([128, 512], f32, tag="pn10a", name="pn10a")
                pN10b = pp.tile([128, 512], f32, tag="pn10b", name="pn10b")
                sN10 = strip["sN10"]
                for j, ps in ((0, pN10a), (1, pN10b)):
                    for a in range(8):
                        v0 = 896 - 128 * a + 512 * j
                        nc.tensor.matmul(
                            ps[:], f["xcn"][:, a, :], sN10[:, v0 : v0 + 512],
                            start=(a == 0), stop=(a == 7),
                        )
                preN = up.tile([128, 2, 1024], bf16, tag="preN", name="preN")
                rN10 = ep.tile([128, 1024], bf16, tag="rn10", name="rn10")
                os_ = [
                    op.tile([128, 1024], bf16, tag=f"o{seg}", name=f"o{seg}")
                    for seg in range(4)
                ]
                for h, ps in ((0, pN10a), (1, pN10b)):
                    c0 = 512 * h
                    rh = rN10[:, c0 : c0 + 512]
                    nc.scalar.mul(rh, ps[:], 0.25)
                    nc.vector.tensor_tensor(  # preN0 half
                        preN[:, 0, c0 : c0 + 512], rc1[:, c0 : c0 + 512], rh, ADD
                    )
                    nc.vector.tensor_tensor(  # preN1 half
                        preN[:, 1, c0 : c0 + 512], rc1[:, c0 : c0 + 512], rh, SUB
                    )
                    for seg, eng in ((0, "D"), (1, "P"), (2, "D"), (3, "P")):
                        alu = ADD if seg in (0, 1) else SUB
                        rn_half = rn[:, 1024 * (seg % 2) + c0 : 1024 * (seg % 2) + c0 + 512]
                        oh = os_[seg][:, c0 : c0 + 512]
                        if eng == "D":
                            nc.vector.tensor_tensor(
                                oh, preN[:, seg % 2, c0 : c0 + 512], rn_half, alu
                            )
                        else:
                            nc.gpsimd.tensor_tensor(
                                oh, preN[:, seg % 2, c0 : c0 + 512], rn_half, alu
                            )
                        nc.sync.dma_start(
                            out_d[b0 : b0 + 128, 1024 * seg + c0 : 1024 * seg + c0 + 512],
                            oh,
                        )

            # ---------------- preamble ----------------
            nc.gpsimd.memset(warm[:], 0.0)
            xbig = xbigp.tile([128, 32, 128], bf16, tag="xbig", name="xbig")
            nc.sync.dma_start(xbig[:, 0:16, :], xtr_d[:, 0, 0:16, :])
            nc.sync.dma_start(xbig[:, 16:32, :], xtr_d[:, 0, 16:32, :])
            nc.sync.dma_start(SB[:, 0:1792], sb_d[:, 0:1792])           # sC5+sN5
            nc.sync.dma_start(SB[:, 1792:SB_SPLIT], sb_d[:, 1792:SB_SPLIT])  # sN10
            nc.sync.dma_start(SB[:, SB_SPLIT:SBW], sb_d[:, SB_SPLIT:SBW])
            # PE clock warmup: HAM releases 2.4 GHz after ~3us of activity;
            # burn dummies while the first DMAs/folds land.
            pW = pp.tile([128, 512], f32, tag="pc5", name="pwarm")
            for _ in range(N_WARM):
                nc.tensor.matmul(
                    pW[:], warm[:, 0:128], warm[:], start=True, stop=True
                )

            # ---------------- main pipeline ----------------
            # Iteration bt interleaves emissions so each engine's in-order
            # stream matches when its work becomes runnable:
            #   mms+evacs(bt) | rn-chain(bt-1) | early folds(bt+1) |
            #   rc-part(bt-1) | T-folds(bt+1) | finals(bt-1)
            f_cur = emit_folds(xbig, first=True)
            xbig_next = emit_dma_in(1)
            r_prev = None
            for bt in range(NB - 1):
                xbig = xbig_next
                if bt + 2 < NB:
                    xbig_next = emit_dma_in(bt + 2)
                rCN5 = mms_cn5(f_cur)
                rN10 = mms_n10(f_cur)
                rTp, rTm = mms_t(f_cur)
                if r_prev is not None:
                    rn = emit_rn_chain(r_prev[2], r_prev[3])
                fa = emit_folds_a(xbig, swap=(bt == 0))
                if r_prev is not None:
                    rc2 = emit_rc_part(r_prev[0], r_prev[1])
                f_cur = emit_folds_b(fa)
                if r_prev is not None:
                    emit_finals(bt - 1, rc2, rn)
                r_prev = (rCN5, rN10, rTp, rTm)
            emit_unfold(NB - 2, *r_prev)
            emit_tail_block(f_cur, NB - 1)

    nc.compile()
    return nc


def _get_nc():
    if "nc" not in _STATE:
        _STATE["nc"] = _build()
    return _STATE["nc"]


def _make_strip_table(w):
    """All 7 leaf band strips, packed [128, SBW] bf16.

    Strip tiles are shears: tile[p, v] = S[OFF + v - p]. Sequences (t any int,
    Ecyc = wt cyclic):
      Ep(t)   = Ecyc(t) + Ecyc(t+2048)          cyc2048
      En(t)   = Ecyc(t) - Ecyc(t+2048)          nega2048
      En10(t) = Ep(t) - Ep(t+1024)              nega1024 leaf
      Epp(t)  = Ep(t) + Ep(t+1024)              cyc1024
      Ec5(t)  = Epp(t) + Epp(t+512)             cyc512 leaf
      En5(t)  = Epp(t) - Epp(t+512)             nega512 leaf
      D(t) = En(t) - En(t+1024); Bt(t) = En(t+512) + En(t+1536)
      L+/-(t) = D(t) +- sqrt2 En(t+1536)        T1024 low strips
      H+/-(t) = Bt(t) -+ sqrt2 En(t+1024)       T1024 high strips
    CRT scales (1/8, 1/8, 1/4, 1/(4sqrt2)) are applied in the ACT evacs.
    """
    import ml_dtypes

    wt = np.roll(w[::-1], 1).astype(np.float64)
    Ecyc = lambda t: wt[np.mod(t, N)]
    Ep = lambda t: Ecyc(t) + Ecyc(t + 2048)
    En = lambda t: Ecyc(t) - Ecyc(t + 2048)
    Epp = lambda t: Ep(t) + Ep(t + 1024)
    seqs = {
        "sC5": lambda t: Epp(t) + Epp(t + 512),
        "sN5": lambda t: Epp(t) - Epp(t + 512),
        "sN10": lambda t: Ep(t) - Ep(t + 1024),
        "sLp": lambda t: (En(t) - En(t + 1024)) + R2 * En(t + 1536),
        "sLm": lambda t: (En(t) - En(t + 1024)) - R2 * En(t + 1536),
        "sHp": lambda t: (En(t + 512) + En(t + 1536)) - R2 * En(t + 1024),
        "sHm": lambda t: (En(t + 512) + En(t + 1536)) + R2 * En(t + 1024),
    }
    p = np.arange(128)[:, None]
    tab = np.zeros((128, SBW), dtype=np.float64)
    for name, (c0, width, off) in STRIPS.items():
        v = np.arange(width)[None, :]
        tab[:, c0 : c0 + width] = seqs[name](off + v - p)
    return np.ascontiguousarray(tab.astype(ml_dtypes.bfloat16))


def _prep_inputs(x, w):
    import ml_dtypes

    x = np.ascontiguousarray(x, dtype=np.float32)
    w = np.ascontiguousarray(w, dtype=np.float32)
    sb = _make_strip_table(w)
    # chunk-interleaved layout: position 2t holds ring chunk t, position
    # 2t+1 holds chunk t+16 — L1 fold pairs become stride-2 neighbors, so
    # block 0 can fold each DMA half as it lands.
    perm = [(i >> 1) + 16 * (i & 1) for i in range(32)]
    in_maps = []
    for i in range(N_CORES):
        xtr = x[i * B_SHARD : (i + 1) * B_SHARD].T  # [N, B_SHARD]
        xtr = xtr.reshape(32, 128, B_SHARD)[perm]   # [a, p, b]
        # [p, bblk, a, bwithin]: per-(p, bblk) an 8KB contiguous bf16 run
        xtr = np.ascontiguousarray(
            xtr.reshape(32, 128, NB, 128).transpose(1, 2, 0, 3).astype(
                ml_dtypes.bfloat16
            )
        )
        in_maps.append({"xtr": xtr, "sb": sb})
    return in_maps


def kernel(x, w, _trace=False):
    from concourse.bass_utils import run_bass_kernel_spmd

    nc = _get_nc()
    in_maps = _prep_inputs(x, w)
    res = run_bass_kernel_spmd(nc, in_maps, list(range(N_CORES)), trace=_trace)
    out = np.concatenate(
        [np.asarray(res.results[i]["out"]).astype(np.float32) for i in range(N_CORES)],
        axis=0,
    )
    if _trace:
        _STATE["last_result"] = res
    return out



# revision 8
# speedup vs baseline: 1.1124x; 1.0346x over previous
"""Circulant matmul for TRN2: 4.5-level CRT with trinomial split, bf16 matmuls.

out[b, r] = sum_c x[b,c] w[(c-r) mod N] = (x (*) wt)[r], wt = roll(w[::-1],1)
(cyclic convolution along c) — no input flip or output reversal needed.

CRT tree on z^4096-1 (per 128-row block of x):
  cyc4096 -> cyc2048 + nega2048             (binomial, band scale 1/2 each)
  cyc2048 -> cyc1024 + nega1024[LEAF]       (1/2)
  cyc1024 -> cyc512 + nega512[LEAF]         (1/2)
  cyc512  -> cyc256[LEAF] + nega256[LEAF]   (1/2)
  nega2048 -> T1024+ , T1024- [LEAVES]      (z^1024 -+ sqrt2 z^512 + 1, 1/(2sqrt2))
Leaf matmuls: 54 x bf16 = 27648 PE cycles/block.  x chunks are stored in
bit-reversed order so EVERY fold level is a stride-2 slice and any quarter
of block 0 folds to completion as its DMA lands (fast pipeline fill).
I/O is bf16 both ways (host converts); per-block HBM traffic = 2 MiB.
Finals are two [128,2048] ops (DVE + Pool) feeding two output DMAs.
"""

import sys

sys.path.insert(0, "/opt/trn_rl_repo")

import numpy as np

N = 4096
B = 8192
N_CORES = 8
B_SHARD = B // N_CORES  # 1024
NB = B_SHARD // 128     # 8 row-tiles per core
R2 = float(np.sqrt(2.0))
SC_T = float(1.0 / (4.0 * np.sqrt(2.0)))   # T1024 leaves: 1/2 * 1/(2sqrt2)

# bit-reversal position maps: leaf-residue chunk a lives at fold-output
# position BRk[a] (k = log2 nchunks)
BR2 = [0, 2, 1, 3]
BR3 = [0, 4, 2, 6, 1, 5, 3, 7]
# x chunk stored at position i is BITREV5[i]
BITREV5 = [
    ((i & 1) << 4) | ((i & 2) << 2) | (i & 4) | ((i & 8) >> 2) | ((i & 16) >> 4)
    for i in range(32)
]

# strip layout in the packed [128, SBW] host table (name: col0, width, OFF)
STRIPS = {
    "sC25": (0, 384, -128),
    "sN25": (384, 384, -128),
    "sN5": (768, 896, -384),
    "sN10": (1664, 1920, -896),
    "sLp": (3584, 1408, -896),
    "sHp": (4992, 1408, -896),
    "sLm": (6400, 1408, -896),
    "sHm": (7808, 1408, -896),
}
SBW = 9216
SB_CN = 1664    # sC25+sN25+sN5
SB_N10 = 3584   # ..sN10

_STATE = {}

N_WARM = 9


def _build():
    import concourse.bacc as bacc
    import concourse.mybir as mybir
    import concourse.tile as tile

    f32 = mybir.dt.float32
    bf16 = mybir.dt.bfloat16
    ADD = mybir.AluOpType.add
    SUB = mybir.AluOpType.subtract

    nc = bacc.Bacc("TRN2", target_bir_lowering=False, debug=False)
    # x transposed, chunk-bit-reversed, bf16 on host; laid out
    # [p, bblk, a, bwithin] so each block is an 8KB contiguous run/partition.
    xtr_d = nc.declare_dram_parameter("xtr", [128, NB, 32, 128], bf16, isOutput=False)
    sb_d = nc.declare_dram_parameter("sb", [128, SBW], bf16, isOutput=False)
    out_d = nc.declare_dram_parameter("out", [B_SHARD, N], bf16, isOutput=True)

    with tile.TileContext(nc) as tc:
        with (
            tc.tile_pool(name="const", bufs=1) as constp,
            tc.tile_pool(name="xbig", bufs=3) as xbigp,
            tc.tile_pool(name="fold", bufs=2) as fp,
            tc.tile_pool(name="evac", bufs=3) as ep,
            tc.tile_pool(name="unf", bufs=2) as up,
            tc.tile_pool(name="outp", bufs=2) as op,
            tc.tile_pool(name="psum", bufs=1, space="PSUM") as pp,
        ):
            SB = constp.tile([128, SBW], bf16, name="SB")
            warm = constp.tile([128, 512], bf16, name="warm")
            strip = {k: SB[:, c0 : c0 + w] for k, (c0, w, _) in STRIPS.items()}

            def emit_dma_in(bt):
                xbig = xbigp.tile([128, 32, 128], bf16, tag="xbig", name="xbig")
                nc.sync.dma_start(xbig[:], xtr_d[:, bt, :, :])
                return xbig

            def emit_folds_a(xbig, swap=False):
                """Early folds: xc (Pool), cyc chain (DVE) — these feed the
                first matmuls of the next block. swap=True runs xc on DVE."""
                xc = fp.tile([128, 16, 128], bf16, tag="xc", name="xc")
                xn = fp.tile([128, 16, 128], bf16, tag="xn", name="xn")
                if swap:
                    nc.vector.tensor_tensor(
                        xc[:], xbig[:, 0:32:2, :], xbig[:, 1:32:2, :], ADD
                    )
                    nc.gpsimd.tensor_tensor(
                        xn[:], xbig[:, 0:32:2, :], xbig[:, 1:32:2, :], SUB
                    )
                else:
                    nc.gpsimd.tensor_tensor(
                        xc[:], xbig[:, 0:32:2, :], xbig[:, 1:32:2, :], ADD
                    )
                    nc.vector.tensor_tensor(
                        xn[:], xbig[:, 0:32:2, :], xbig[:, 1:32:2, :], SUB
                    )
                xcc = fp.tile([128, 8, 128], bf16, tag="xcc", name="xcc")
                nc.vector.tensor_tensor(
                    xcc[:], xc[:, 0:16:2, :], xc[:, 1:16:2, :], ADD
                )
                xccc = fp.tile([128, 4, 128], bf16, tag="xccc", name="xccc")
                nc.vector.tensor_tensor(
                    xccc[:], xcc[:, 0:8:2, :], xcc[:, 1:8:2, :], ADD
                )
                xccn = fp.tile([128, 4, 128], bf16, tag="xccn", name="xccn")
                nc.vector.tensor_tensor(
                    xccn[:], xcc[:, 0:8:2, :], xcc[:, 1:8:2, :], SUB
                )
                xc4 = fp.tile([128, 2, 128], bf16, tag="xc4", name="xc4")
                nc.vector.tensor_tensor(
                    xc4[:], xccc[:, 0:4:2, :], xccc[:, 1:4:2, :], ADD
                )
                xn4 = fp.tile([128, 2, 128], bf16, tag="xn4", name="xn4")
                nc.vector.tensor_tensor(
                    xn4[:], xccc[:, 0:4:2, :], xccc[:, 1:4:2, :], SUB
                )
                xcn = fp.tile([128, 8, 128], bf16, tag="xcn", name="xcn")
                nc.vector.tensor_tensor(
                    xcn[:], xc[:, 0:16:2, :], xc[:, 1:16:2, :], SUB
                )
                return dict(xn=xn, xcn=xcn, xc4=xc4, xn4=xn4, xccn=xccn)

            def emit_folds_b(fa):
                """Late folds: trinomial xTp/xTm (needed only by the T
                matmuls, mid-block). sqrt2 scales on ACT."""
                xn = fa["xn"]
                xA = fp.tile([128, 4, 128], bf16, tag="xA", name="xA")
                nc.vector.tensor_tensor(
                    xA[:], xn[:, 0:16:4, :], xn[:, 1:16:4, :], SUB
                )
                xB = fp.tile([128, 4, 128], bf16, tag="xB", name="xB")
                nc.vector.tensor_tensor(
                    xB[:], xn[:, 2:16:4, :], xn[:, 3:16:4, :], ADD
                )
                s10 = fp.tile([128, 4, 128], bf16, tag="s10", name="s10")
                nc.scalar.mul(s10[:], xn[:, 1:16:4, :], R2)
                s11 = fp.tile([128, 4, 128], bf16, tag="s11", name="s11")
                nc.scalar.mul(s11[:], xn[:, 3:16:4, :], R2)
                xTp = fp.tile([128, 8, 128], bf16, tag="xTp", name="xTp")
                nc.vector.tensor_tensor(xTp[:, 0:4, :], xA[:], s11[:], ADD)
                nc.vector.tensor_tensor(xTp[:, 4:8, :], xB[:], s10[:], SUB)
                xTm = fp.tile([128, 8, 128], bf16, tag="xTm", name="xTm")
                nc.vector.tensor_tensor(xTm[:, 0:4, :], xA[:], s11[:], SUB)
                nc.vector.tensor_tensor(xTm[:, 4:8, :], xB[:], s10[:], ADD)
                fa["xTp"] = xTp
                fa["xTm"] = xTm
                return fa

            def emit_folds_first(xbig):
                """Block 0: fold each quarter's cyc chain as its DMA lands;
                xn/xcn/xc4 pieces follow. All on DVE (Pool too slow to gate)."""
                xc = fp.tile([128, 16, 128], bf16, tag="xc", name="xc")
                xn = fp.tile([128, 16, 128], bf16, tag="xn", name="xn")
                xcc = fp.tile([128, 8, 128], bf16, tag="xcc", name="xcc")
                xccc = fp.tile([128, 4, 128], bf16, tag="xccc", name="xccc")
                xccn = fp.tile([128, 4, 128], bf16, tag="xccn", name="xccn")
                xc4 = fp.tile([128, 2, 128], bf16, tag="xc4", name="xc4")
                xn4 = fp.tile([128, 2, 128], bf16, tag="xn4", name="xn4")
                xcn = fp.tile([128, 8, 128], bf16, tag="xcn", name="xcn")
                for q in range(4):
                    s = 8 * q
                    nc.vector.tensor_tensor(
                        xc[:, 4 * q : 4 * q + 4, :],
                        xbig[:, s : s + 8 : 2, :],
                        xbig[:, s + 1 : s + 8 : 2, :], ADD
                    )
                    nc.vector.tensor_tensor(
                        xcc[:, 2 * q : 2 * q + 2, :],
                        xc[:, 4 * q : 4 * q + 4 : 2, :],
                        xc[:, 4 * q + 1 : 4 * q + 4 : 2, :], ADD
                    )
                    nc.vector.tensor_tensor(
                        xccc[:, q : q + 1, :],
                        xcc[:, 2 * q : 2 * q + 1, :],
                        xcc[:, 2 * q + 1 : 2 * q + 2, :], ADD
                    )
                    nc.vector.tensor_tensor(
                        xccn[:, q : q + 1, :],
                        xcc[:, 2 * q : 2 * q + 1, :],
                        xcc[:, 2 * q + 1 : 2 * q + 2, :], SUB
                    )
                    if q == 1 or q == 3:
                        h = q // 2  # xc4/xn4 entry h from xccc pos 2h, 2h+1
                        nc.vector.tensor_tensor(
                            xc4[:, h : h + 1, :],
                            xccc[:, 2 * h : 2 * h + 1, :],
                            xccc[:, 2 * h + 1 : 2 * h + 2, :], ADD
                        )
                        nc.vector.tensor_tensor(
                            xn4[:, h : h + 1, :],
                            xccc[:, 2 * h : 2 * h + 1, :],
                            xccc[:, 2 * h + 1 : 2 * h + 2, :], SUB
                        )
                        nc.vector.tensor_tensor(
                            xcn[:, 4 * h : 4 * h + 4, :],
                            xc[:, 8 * h : 8 * h + 8 : 2, :],
                            xc[:, 8 * h + 1 : 8 * h + 8 : 2, :], SUB
                        )
                        nc.vector.tensor_tensor(
                            xn[:, 8 * h : 8 * h + 8, :],
                            xbig[:, 16 * h : 16 * h + 16 : 2, :],
                            xbig[:, 16 * h + 1 : 16 * h + 16 : 2, :], SUB
                        )
                fa = dict(xn=xn, xcn=xcn, xc4=xc4, xn4=xn4, xccn=xccn)
                return emit_folds_b(fa)

            def mms_cn(f):
                """cyc256+nega256 (shared psum bank) and nega512 leaves."""
                pCN25 = pp.tile([128, 512], f32, tag="pcn25", name="pcn25")
                for a in range(2):
                    v0 = 128 * (1 - a)
                    nc.tensor.matmul(
                        pCN25[:, 0:256], f["xc4"][:, a, :],
                        strip["sC25"][:, v0 : v0 + 256],
                        start=(a == 0), stop=(a == 1),
                    )
                for a in range(2):
                    v0 = 128 * (1 - a)
                    nc.tensor.matmul(
                        pCN25[:, 256:512], f["xn4"][:, a, :],
                        strip["sN25"][:, v0 : v0 + 256],
                        start=(a == 0), stop=(a == 1),
                    )
                pN5 = pp.tile([128, 512], f32, tag="pn5", name="pn5")
                for a in range(4):
                    v0 = 128 * (3 - a)
                    nc.tensor.matmul(
                        pN5[:], f["xccn"][:, BR2[a], :],
                        strip["sN5"][:, v0 : v0 + 512],
                        start=(a == 0), stop=(a == 3),
                    )
                rCN = ep.tile([128, 1024], bf16, tag="rcn", name="rcn")
                nc.scalar.mul(rCN[:, 0:512], pCN25[:], 0.125)
                nc.scalar.mul(rCN[:, 512:1024], pN5[:], 0.125)
                return rCN

            def mms_n10(f):
                """nega1024 in two half-PSUM tiles (tail streams per half)."""
                pN10a = pp.tile([128, 512], f32, tag="pn10a", name="pn10a")
                pN10b = pp.tile([128, 512], f32, tag="pn10b", name="pn10b")
                sN10 = strip["sN10"]
                rN10 = ep.tile([128, 1024], bf16, tag="rn10", name="rn10")
                for j, ps in ((0, pN10a), (1, pN10b)):
                    for a in range(8):
                        v0 = 896 - 128 * a + 512 * j
                        nc.tensor.matmul(
                            ps[:], f["xcn"][:, BR3[a], :], sN10[:, v0 : v0 + 512],
                            start=(a == 0), stop=(a == 7),
                        )
                    nc.scalar.mul(rN10[:, 512 * j : 512 * j + 512], ps[:], 0.25)
                return rN10

            def mm_t_ring(psum, xres, sname):
                s = strip[sname]
                for a in range(8):
                    v0 = 896 - 128 * a
                    pos = BR2[a] if a < 4 else 4 + BR2[a - 4]
                    nc.tensor.matmul(
                        psum[:], xres[:, pos, :], s[:, v0 : v0 + 512],
                        start=(a == 0), stop=(a == 7),
                    )

            def mms_t(f):
                rTp = ep.tile([128, 1024], bf16, tag="rtp", name="rtp")
                rTm = ep.tile([128, 1024], bf16, tag="rtm", name="rtm")
                for xres, rr, lo_name, hi_name in (
                    (f["xTp"], rTp, "sLp", "sHp"),
                    (f["xTm"], rTm, "sLm", "sHm"),
                ):
                    for j, sname in ((0, lo_name), (1, hi_name)):
                        ph = pp.tile(
                            [128, 512], f32, tag=f"pt{sname}", name=f"pt{sname}"
                        )
                        mm_t_ring(ph, xres, sname)
                        nc.scalar.mul(
                            rr[:, 512 * j : 512 * j + 512], ph[:], SC_T
                        )
                return rTp, rTm

            def emit_rn_chain(rTp, rTm):
                """nega2048 reconstruction from T1024+/- (DVE + ACT scale)."""
                rn = up.tile([128, 2048], bf16, tag="rn", name="rn")
                tt01 = up.tile([128, 1024], bf16, tag="tt01", name="tt01")
                st01 = up.tile([128, 1024], bf16, tag="st01", name="st01")
                nc.vector.tensor_tensor(  # o3
                    rn[:, 1536:2048], rTp[:, 0:512], rTm[:, 0:512], SUB
                )
                nc.vector.tensor_tensor(  # t0
                    tt01[:, 0:512], rTp[:, 0:512], rTm[:, 0:512], ADD
                )
                nc.vector.tensor_tensor(  # o2
                    rn[:, 1024:1536], rTm[:, 512:1024], rTp[:, 512:1024], SUB
                )
                nc.vector.tensor_tensor(  # t1
                    tt01[:, 512:1024], rTp[:, 512:1024], rTm[:, 512:1024], ADD
                )
                nc.scalar.mul(st01[:], tt01[:], R2)
                nc.vector.tensor_tensor(  # o0
                    rn[:, 0:512], st01[:, 0:512], rn[:, 1024:1536], ADD
                )
                nc.vector.tensor_tensor(  # o1
                    rn[:, 512:1024], st01[:, 512:1024], rn[:, 1536:2048], SUB
                )
                return rn

            def emit_rc_part(rCN, rN10):
                rc0 = up.tile([128, 512], bf16, tag="rc0", name="rc0")
                nc.vector.tensor_tensor(
                    rc0[:, 0:256], rCN[:, 0:256], rCN[:, 256:512], ADD
                )
                nc.vector.tensor_tensor(
                    rc0[:, 256:512], rCN[:, 0:256], rCN[:, 256:512], SUB
                )
                rc1 = up.tile([128, 1024], bf16, tag="rc1", name="rc1")
                nc.vector.tensor_tensor(
                    rc1[:, 0:512], rc0[:], rCN[:, 512:1024], ADD
                )
                nc.vector.tensor_tensor(
                    rc1[:, 512:1024], rc0[:], rCN[:, 512:1024], SUB
                )
                rc2 = up.tile([128, 2048], bf16, tag="rc2", name="rc2")
                nc.vector.tensor_tensor(rc2[:, 0:1024], rc1[:], rN10[:], ADD)
                nc.vector.tensor_tensor(rc2[:, 1024:2048], rc1[:], rN10[:], SUB)
                return rc2

            def emit_finals(bt, rc2, rn):
                b0 = 128 * bt
                os01 = op.tile([128, 2048], bf16, tag="os01", name="os01")
                os23 = op.tile([128, 2048], bf16, tag="os23", name="os23")
                nc.vector.tensor_tensor(os01[:], rc2[:], rn[:], ADD)
                nc.gpsimd.tensor_tensor(os23[:], rc2[:], rn[:], SUB)
                nc.sync.dma_start(out_d[b0 : b0 + 128, 0:2048], os01[:])
                nc.sync.dma_start(out_d[b0 : b0 + 128, 2048:4096], os23[:])

            def emit_unfold(bt, rCN, rN10, rTp, rTm):
                rn = emit_rn_chain(rTp, rTm)
                rc2 = emit_rc_part(rCN, rN10)
                emit_finals(bt, rc2, rn)

            def emit_tail_block(f, bt):
                """Last block: T matmuls first (longest reconstruction chain),
                CN next, nega1024 last in two halves; preN = rc1 +- rN10h
                streams each output half out as its rN10 half evacuates.
                All tail combines on DVE (Pool's 2 us ops would pad the end)."""
                b0 = 128 * bt
                rTp, rTm = mms_t(f)
                rCN = mms_cn(f)
                rn = emit_rn_chain(rTp, rTm)
                rc0 = up.tile([128, 512], bf16, tag="rc0", name="rc0")
                nc.vector.tensor_tensor(
                    rc0[:, 0:256], rCN[:, 0:256], rCN[:, 256:512], ADD
                )
                nc.vector.tensor_tensor(
                    rc0[:, 256:512], rCN[:, 0:256], rCN[:, 256:512], SUB
                )
                rc1 = up.tile([128, 1024], bf16, tag="rc1", name="rc1")
                nc.vector.tensor_tensor(
                    rc1[:, 0:512], rc0[:], rCN[:, 512:1024], ADD
                )
                nc.vector.tensor_tensor(
                    rc1[:, 512:1024], rc0[:], rCN[:, 512:1024], SUB
                )
                pN10a = pp.tile([128, 512], f32, tag="pn10a", name="pn10a")
                pN10b = pp.tile([128, 512], f32, tag="pn10b", name="pn10b")
                sN10 = strip["sN10"]
                for j, ps in ((0, pN10a), (1, pN10b)):
                    for a in range(8):
                        v0 = 896 - 128 * a + 512 * j
                        nc.tensor.matmul(
                            ps[:], f["xcn"][:, BR3[a], :], sN10[:, v0 : v0 + 512],
                            start=(a == 0), stop=(a == 7),
                        )
                preN = up.tile([128, 2, 1024], bf16, tag="preN", name="preN")
                rN10 = ep.tile([128, 1024], bf16, tag="rn10", name="rn10")
                os01 = op.tile([128, 2048], bf16, tag="os01", name="os01")
                os23 = op.tile([128, 2048], bf16, tag="os23", name="os23")
                for h, ps in ((0, pN10a), (1, pN10b)):
                    c0 = 512 * h
                    rh = rN10[:, c0 : c0 + 512]
                    nc.scalar.mul(rh, ps[:], 0.25)
                    nc.vector.tensor_tensor(  # preN0 half
                        preN[:, 0, c0 : c0 + 512], rc1[:, c0 : c0 + 512], rh, ADD
                    )
                    nc.vector.tensor_tensor(  # preN1 half
                        preN[:, 1, c0 : c0 + 512], rc1[:, c0 : c0 + 512], rh, SUB
                    )
                    for seg in range(4):
                        alu = ADD if seg in (0, 1) else SUB
                        rn_half = rn[:, 1024 * (seg % 2) + c0 : 1024 * (seg % 2) + c0 + 512]
                        ot = os01 if seg < 2 else os23
                        oh = ot[:, 1024 * (seg % 2) + c0 : 1024 * (seg % 2) + c0 + 512]
                        nc.vector.tensor_tensor(
                            oh, preN[:, seg % 2, c0 : c0 + 512], rn_half, alu
                        )
                        nc.sync.dma_start(
                            out_d[b0 : b0 + 128, 1024 * seg + c0 : 1024 * seg + c0 + 512],
                            oh,
                        )

            # ---------------- preamble ----------------
            nc.gpsimd.memset(warm[:], 0.0)
            xbig = xbigp.tile([128, 32, 128], bf16, tag="xbig", name="xbig")
            # DMA order: x quarters interleaved with strip pieces so the fold
            # chain, C/N strips and N10/T strips all land just in time.
            nc.sync.dma_start(xbig[:, 0:8, :], xtr_d[:, 0, 0:8, :])
            nc.sync.dma_start(xbig[:, 8:16, :], xtr_d[:, 0, 8:16, :])
            nc.sync.dma_start(SB[:, 0:SB_CN], sb_d[:, 0:SB_CN])
            nc.sync.dma_start(xbig[:, 16:24, :], xtr_d[:, 0, 16:24, :])
            nc.sync.dma_start(xbig[:, 24:32, :], xtr_d[:, 0, 24:32, :])
            nc.sync.dma_start(SB[:, SB_CN:SB_N10], sb_d[:, SB_CN:SB_N10])
            for s0 in range(SB_N10, SBW, 1408):
                nc.sync.dma_start(SB[:, s0 : s0 + 1408], sb_d[:, s0 : s0 + 1408])
            # PE clock warmup: HAM releases 2.4 GHz after ~3us of activity;
            # burn dummies while the first DMAs/folds land.
            pW = pp.tile([128, 512], f32, tag="pcn25", name="pwarm")
            for _ in range(N_WARM):
                nc.tensor.matmul(
                    pW[:], warm[:, 0:128], warm[:], start=True, stop=True
                )

            # ---------------- main pipeline ----------------
            # Iteration bt interleaves emissions so each engine's in-order
            # stream matches when its work becomes runnable:
            #   mms+evacs(bt) | rn-chain(bt-1) | early folds(bt+1) |
            #   rc-part(bt-1) | T-folds(bt+1) | finals(bt-1)
            f_cur = emit_folds_first(xbig)
            xbig_next = emit_dma_in(1)
            r_prev = None
            for bt in range(NB - 1):
                xbig = xbig_next
                if bt + 2 < NB:
                    xbig_next = emit_dma_in(bt + 2)
                rCN = mms_cn(f_cur)
                rN10 = mms_n10(f_cur)
                rTp, rTm = mms_t(f_cur)
                if r_prev is not None:
                    rn = emit_rn_chain(r_prev[2], r_prev[3])
                fa = emit_folds_a(xbig, swap=(bt == 0))
                if r_prev is not None:
                    rc2 = emit_rc_part(r_prev[0], r_prev[1])
                f_cur = emit_folds_b(fa)
                if r_prev is not None:
                    emit_finals(bt - 1, rc2, rn)
                r_prev = (rCN, rN10, rTp, rTm)
            emit_unfold(NB - 2, *r_prev)
            emit_tail_block(f_cur, NB - 1)

    nc.compile()
    return nc


def _get_nc():
    if "nc" not in _STATE:
        _STATE["nc"] = _build()
    return _STATE["nc"]


def _make_strip_table(w):
    """All 8 leaf band strips, packed [128, SBW] bf16.

    Strip tiles are shears: tile[p, v] = S[OFF + v - p]. Sequences (t any int,
    Ecyc = wt cyclic):
      Ep(t)   = Ecyc(t) + Ecyc(t+2048)          cyc2048
      En(t)   = Ecyc(t) - Ecyc(t+2048)          nega2048
      Epp(t)  = Ep(t) + Ep(t+1024)              cyc1024
      En10(t) = Ep(t) - Ep(t+1024)              nega1024 leaf
      C5(t)   = Epp(t) + Epp(t+512)             cyc512
      N5(t)   = Epp(t) - Epp(t+512)             nega512 leaf
      C25(t)  = C5(t) + C5(t+256)               cyc256 leaf (x0.5 baked)
      N25(t)  = C5(t) - C5(t+256)               nega256 leaf (x0.5 baked)
      D(t) = En(t) - En(t+1024); Bt(t) = En(t+512) + En(t+1536)
      L+/-(t) = D(t) +- sqrt2 En(t+1536)        T1024 low strips
      H+/-(t) = Bt(t) -+ sqrt2 En(t+1024)       T1024 high strips
    CRT scales (1/8 with the extra 1/2 baked for C25/N25, 1/8, 1/4,
    1/(4sqrt2)) are applied in the ACT PSUM->SBUF evacuations.
    """
    import ml_dtypes

    wt = np.roll(w[::-1], 1).astype(np.float64)
    Ecyc = lambda t: wt[np.mod(t, N)]
    Ep = lambda t: Ecyc(t) + Ecyc(t + 2048)
    En = lambda t: Ecyc(t) - Ecyc(t + 2048)
    Epp = lambda t: Ep(t) + Ep(t + 1024)
    C5 = lambda t: Epp(t) + Epp(t + 512)
    seqs = {
        "sC25": lambda t: 0.5 * (C5(t) + C5(t + 256)),
        "sN25": lambda t: 0.5 * (C5(t) - C5(t + 256)),
        "sN5": lambda t: Epp(t) - Epp(t + 512),
        "sN10": lambda t: Ep(t) - Ep(t + 1024),
        "sLp": lambda t: (En(t) - En(t + 1024)) + R2 * En(t + 1536),
        "sLm": lambda t: (En(t) - En(t + 1024)) - R2 * En(t + 1536),
        "sHp": lambda t: (En(t + 512) + En(t + 1536)) - R2 * En(t + 1024),
        "sHm": lambda t: (En(t + 512) + En(t + 1536)) + R2 * En(t + 1024),
    }
    p = np.arange(128)[:, None]
    tab = np.zeros((128, SBW), dtype=np.float64)
    for name, (c0, width, off) in STRIPS.items():
        v = np.arange(width)[None, :]
        tab[:, c0 : c0 + width] = seqs[name](off + v - p)
    return np.ascontiguousarray(tab.astype(ml_dtypes.bfloat16))


def _prep_inputs(x, w):
    import ml_dtypes

    x = np.ascontiguousarray(x, dtype=np.float32)
    w = np.ascontiguousarray(w, dtype=np.float32)
    sb = _make_strip_table(w)
    in_maps = []
    for i in range(N_CORES):
        xtr = x[i * B_SHARD : (i + 1) * B_SHARD].T  # [N, B_SHARD]
        xtr = xtr.reshape(32, 128, B_SHARD)[BITREV5]  # [a(pos), p, b]
        # [p, bblk, a, bwithin]: per-(p, bblk) an 8KB contiguous bf16 run
        xtr = np.ascontiguousarray(
            xtr.reshape(32, 128, NB, 128).transpose(1, 2, 0, 3).astype(
                ml_dtypes.bfloat16
            )
        )
        in_maps.append({"xtr": xtr, "sb": sb})
    return in_maps


def kernel(x, w, _trace=False):
    from concourse.bass_utils import run_bass_kernel_spmd

    nc = _get_nc()
    in_maps = _prep_inputs(x, w)
    res = run_bass_kernel_spmd(nc, in_maps, list(range(N_CORES)), trace=_trace)
    out = np.concatenate(
        [np.asarray(res.results[i]["out"]).astype(np.float32) for i in range(N_CORES)],
        axis=0,
    )
    if _trace:
        _STATE["last_result"] = res
    return out


# revision 13
# speedup vs baseline: 1.1166x; 1.0037x over previous
"""Circulant matmul for TRN2: 4.5-level CRT with trinomial split, bf16 matmuls.

out[b, r] = sum_c x[b,c] w[(c-r) mod N] = (x (*) wt)[r], wt = roll(w[::-1],1)
(cyclic convolution along c) — no input flip or output reversal needed.

CRT tree on z^4096-1 (per 128-row block of x):
  cyc4096 -> cyc2048 + nega2048             (binomial, band scale 1/2 each)
  cyc2048 -> cyc1024 + nega1024[LEAF]       (1/2)
  cyc1024 -> cyc512 + nega512[LEAF]         (1/2)
  cyc512  -> cyc256[LEAF] + nega256[LEAF]   (1/2)
  nega2048 -> T1024+ , T1024- [LEAVES]      (z^1024 -+ sqrt2 z^512 + 1, 1/(2sqrt2))
Leaf matmuls: 54 x bf16 = 27648 PE cycles/block.  x chunks are stored in
bit-reversed order so EVERY fold level is a stride-2 slice and any quarter
of block 0 folds to completion as its DMA lands (fast pipeline fill).
I/O is bf16 both ways (host converts); per-block HBM traffic = 2 MiB.
Finals are two [128,2048] ops (DVE + Pool) feeding two output DMAs.
"""

import sys

sys.path.insert(0, "/opt/trn_rl_repo")

import numpy as np

N = 4096
B = 8192
N_CORES = 8
B_SHARD = B // N_CORES  # 1024
NB = B_SHARD // 128     # 8 row-tiles per core
R2 = float(np.sqrt(2.0))
SC_T = float(1.0 / (4.0 * np.sqrt(2.0)))   # T1024 leaves: 1/2 * 1/(2sqrt2)

# bit-reversal position maps: leaf-residue chunk a lives at fold-output
# position BRk[a] (k = log2 nchunks)
BR2 = [0, 2, 1, 3]
BR3 = [0, 4, 2, 6, 1, 5, 3, 7]
# x chunk stored at position i is BITREV5[i]
BITREV5 = [
    ((i & 1) << 4) | ((i & 2) << 2) | (i & 4) | ((i & 8) >> 2) | ((i & 16) >> 4)
    for i in range(32)
]

# strip layout in the packed [128, SBW] host table (name: col0, width, OFF)
STRIPS = {
    "sC25": (0, 384, -128),
    "sN25": (384, 384, -128),
    "sN5": (768, 896, -384),
    "sN10": (1664, 1920, -896),
    "sLp": (3584, 1408, -896),
    "sHp": (4992, 1408, -896),
    "sLm": (6400, 1408, -896),
    "sHm": (7808, 1408, -896),
}
SBW = 9216
SB_CN = 1664    # sC25+sN25+sN5
SB_N10 = 3584   # ..sN10

_STATE = {}

N_WARM = 11


def _build():
    import concourse.bacc as bacc
    import concourse.mybir as mybir
    import concourse.tile as tile

    f32 = mybir.dt.float32
    bf16 = mybir.dt.bfloat16
    ADD = mybir.AluOpType.add
    SUB = mybir.AluOpType.subtract

    nc = bacc.Bacc("TRN2", target_bir_lowering=False, debug=False)
    # x transposed, chunk-bit-reversed, bf16 on host; laid out
    # [p, bblk, a, bwithin] so each block is an 8KB contiguous run/partition.
    xtr_d = nc.declare_dram_parameter("xtr", [128, NB, 32, 128], bf16, isOutput=False)
    sb_d = nc.declare_dram_parameter("sb", [128, SBW], bf16, isOutput=False)
    out_d = nc.declare_dram_parameter("out", [B_SHARD, N], bf16, isOutput=True)

    with tile.TileContext(nc) as tc:
        with (
            tc.tile_pool(name="const", bufs=1) as constp,
            tc.tile_pool(name="xbig", bufs=3) as xbigp,
            tc.tile_pool(name="fold", bufs=2) as fp,
            tc.tile_pool(name="evac", bufs=3) as ep,
            tc.tile_pool(name="unf", bufs=2) as up,
            tc.tile_pool(name="outp", bufs=2) as op,
            tc.tile_pool(name="psum", bufs=1, space="PSUM") as pp,
        ):
            SB = constp.tile([128, SBW], bf16, name="SB")
            warm = constp.tile([128, 512], bf16, name="warm")
            strip = {k: SB[:, c0 : c0 + w] for k, (c0, w, _) in STRIPS.items()}

            def emit_dma_in(bt):
                xbig = xbigp.tile([128, 32, 128], bf16, tag="xbig", name="xbig")
                nc.sync.dma_start(xbig[:], xtr_d[:, bt, :, :])
                return xbig

            def emit_folds_a(xbig, swap=False):
                """Early folds: xc (Pool), cyc chain (DVE) — these feed the
                first matmuls of the next block. swap=True runs xc on DVE."""
                xc = fp.tile([128, 16, 128], bf16, tag="xc", name="xc")
                xn = fp.tile([128, 16, 128], bf16, tag="xn", name="xn")
                if swap:
                    nc.vector.tensor_tensor(
                        xc[:], xbig[:, 0:32:2, :], xbig[:, 1:32:2, :], ADD
                    )
                    nc.gpsimd.tensor_tensor(
                        xn[:], xbig[:, 0:32:2, :], xbig[:, 1:32:2, :], SUB
                    )
                else:
                    nc.gpsimd.tensor_tensor(
                        xc[:], xbig[:, 0:32:2, :], xbig[:, 1:32:2, :], ADD
                    )
                    nc.vector.tensor_tensor(
                        xn[:], xbig[:, 0:32:2, :], xbig[:, 1:32:2, :], SUB
                    )
                xcc = fp.tile([128, 8, 128], bf16, tag="xcc", name="xcc")
                nc.vector.tensor_tensor(
                    xcc[:], xc[:, 0:16:2, :], xc[:, 1:16:2, :], ADD
                )
                xccc = fp.tile([128, 4, 128], bf16, tag="xccc", name="xccc")
                nc.vector.tensor_tensor(
                    xccc[:], xcc[:, 0:8:2, :], xcc[:, 1:8:2, :], ADD
                )
                xccn = fp.tile([128, 4, 128], bf16, tag="xccn", name="xccn")
                nc.vector.tensor_tensor(
                    xccn[:], xcc[:, 0:8:2, :], xcc[:, 1:8:2, :], SUB
                )
                xc4 = fp.tile([128, 2, 128], bf16, tag="xc4", name="xc4")
                nc.vector.tensor_tensor(
                    xc4[:], xccc[:, 0:4:2, :], xccc[:, 1:4:2, :], ADD
                )
                xn4 = fp.tile([128, 2, 128], bf16, tag="xn4", name="xn4")
                nc.vector.tensor_tensor(
                    xn4[:], xccc[:, 0:4:2, :], xccc[:, 1:4:2, :], SUB
                )
                xcn = fp.tile([128, 8, 128], bf16, tag="xcn", name="xcn")
                nc.vector.tensor_tensor(
                    xcn[:], xc[:, 0:16:2, :], xc[:, 1:16:2, :], SUB
                )
                return dict(xn=xn, xcn=xcn, xc4=xc4, xn4=xn4, xccn=xccn)

            def emit_folds_b(fa):
                """Late folds: trinomial xTp/xTm (needed only by the T
                matmuls, mid-block). sqrt2 scales on ACT."""
                xn = fa["xn"]
                xA = fp.tile([128, 4, 128], bf16, tag="xA", name="xA")
                nc.vector.tensor_tensor(
                    xA[:], xn[:, 0:16:4, :], xn[:, 1:16:4, :], SUB
                )
                xB = fp.tile([128, 4, 128], bf16, tag="xB", name="xB")
                nc.vector.tensor_tensor(
                    xB[:], xn[:, 2:16:4, :], xn[:, 3:16:4, :], ADD
                )
                s10 = fp.tile([128, 4, 128], bf16, tag="s10", name="s10")
                nc.scalar.mul(s10[:], xn[:, 1:16:4, :], R2)
                s11 = fp.tile([128, 4, 128], bf16, tag="s11", name="s11")
                nc.scalar.mul(s11[:], xn[:, 3:16:4, :], R2)
                xTp = fp.tile([128, 8, 128], bf16, tag="xTp", name="xTp")
                nc.vector.tensor_tensor(xTp[:, 0:4, :], xA[:], s11[:], ADD)
                nc.vector.tensor_tensor(xTp[:, 4:8, :], xB[:], s10[:], SUB)
                xTm = fp.tile([128, 8, 128], bf16, tag="xTm", name="xTm")
                nc.vector.tensor_tensor(xTm[:, 0:4, :], xA[:], s11[:], SUB)
                nc.vector.tensor_tensor(xTm[:, 4:8, :], xB[:], s10[:], ADD)
                fa["xTp"] = xTp
                fa["xTm"] = xTm
                return fa

            def emit_folds_first(xbig):
                """Block 0: fold each quarter's cyc chain as its DMA lands;
                xn/xcn/xc4 pieces follow. All on DVE (Pool too slow to gate)."""
                xc = fp.tile([128, 16, 128], bf16, tag="xc", name="xc")
                xn = fp.tile([128, 16, 128], bf16, tag="xn", name="xn")
                xcc = fp.tile([128, 8, 128], bf16, tag="xcc", name="xcc")
                xccc = fp.tile([128, 4, 128], bf16, tag="xccc", name="xccc")
                xccn = fp.tile([128, 4, 128], bf16, tag="xccn", name="xccn")
                xc4 = fp.tile([128, 2, 128], bf16, tag="xc4", name="xc4")
                xn4 = fp.tile([128, 2, 128], bf16, tag="xn4", name="xn4")
                xcn = fp.tile([128, 8, 128], bf16, tag="xcn", name="xcn")
                for q in range(4):
                    s = 8 * q
                    nc.vector.tensor_tensor(
                        xc[:, 4 * q : 4 * q + 4, :],
                        xbig[:, s : s + 8 : 2, :],
                        xbig[:, s + 1 : s + 8 : 2, :], ADD
                    )
                    nc.vector.tensor_tensor(
                        xcc[:, 2 * q : 2 * q + 2, :],
                        xc[:, 4 * q : 4 * q + 4 : 2, :],
                        xc[:, 4 * q + 1 : 4 * q + 4 : 2, :], ADD
                    )
                    nc.vector.tensor_tensor(
                        xccc[:, q : q + 1, :],
                        xcc[:, 2 * q : 2 * q + 1, :],
                        xcc[:, 2 * q + 1 : 2 * q + 2, :], ADD
                    )
                    nc.vector.tensor_tensor(
                        xccn[:, q : q + 1, :],
                        xcc[:, 2 * q : 2 * q + 1, :],
                        xcc[:, 2 * q + 1 : 2 * q + 2, :], SUB
                    )
                    if q == 1 or q == 3:
                        h = q // 2  # xc4/xn4 entry h from xccc pos 2h, 2h+1
                        nc.vector.tensor_tensor(
                            xc4[:, h : h + 1, :],
                            xccc[:, 2 * h : 2 * h + 1, :],
                            xccc[:, 2 * h + 1 : 2 * h + 2, :], ADD
                        )
                        nc.vector.tensor_tensor(
                            xn4[:, h : h + 1, :],
                            xccc[:, 2 * h : 2 * h + 1, :],
                            xccc[:, 2 * h + 1 : 2 * h + 2, :], SUB
                        )
                        nc.vector.tensor_tensor(
                            xcn[:, 4 * h : 4 * h + 4, :],
                            xc[:, 8 * h : 8 * h + 8 : 2, :],
                            xc[:, 8 * h + 1 : 8 * h + 8 : 2, :], SUB
                        )
                        nc.vector.tensor_tensor(
                            xn[:, 8 * h : 8 * h + 8, :],
                            xbig[:, 16 * h : 16 * h + 16 : 2, :],
                            xbig[:, 16 * h + 1 : 16 * h + 16 : 2, :], SUB
                        )
                fa = dict(xn=xn, xcn=xcn, xc4=xc4, xn4=xn4, xccn=xccn)
                return emit_folds_b(fa)

            def mms_cn(f):
                """cyc256+nega256 (shared psum bank) and nega512 leaves."""
                pCN25 = pp.tile([128, 512], f32, tag="pcn25", name="pcn25")
                for a in range(2):
                    v0 = 128 * (1 - a)
                    nc.tensor.matmul(
                        pCN25[:, 0:256], f["xc4"][:, a, :],
                        strip["sC25"][:, v0 : v0 + 256],
                        start=(a == 0), stop=(a == 1),
                    )
                for a in range(2):
                    v0 = 128 * (1 - a)
                    nc.tensor.matmul(
                        pCN25[:, 256:512], f["xn4"][:, a, :],
                        strip["sN25"][:, v0 : v0 + 256],
                        start=(a == 0), stop=(a == 1),
                    )
                pN5 = pp.tile([128, 512], f32, tag="pn5", name="pn5")
                for i, a in enumerate((0, 2, 1, 3)):  # feed order: quarter BR2[a]
                    v0 = 128 * (3 - a)
                    nc.tensor.matmul(
                        pN5[:], f["xccn"][:, BR2[a], :],
                        strip["sN5"][:, v0 : v0 + 512],
                        start=(i == 0), stop=(i == 3),
                    )
                rCN = ep.tile([128, 1024], bf16, tag="rcn", name="rcn")
                nc.scalar.mul(rCN[:, 0:512], pCN25[:], 0.125)
                nc.scalar.mul(rCN[:, 512:1024], pN5[:], 0.125)
                return rCN

            def mms_n10(f):
                """nega1024 in two half-PSUM tiles (tail streams per half)."""
                pN10a = pp.tile([128, 512], f32, tag="pn10a", name="pn10a")
                pN10b = pp.tile([128, 512], f32, tag="pn10b", name="pn10b")
                sN10 = strip["sN10"]
                rN10 = ep.tile([128, 1024], bf16, tag="rn10", name="rn10")
                for j, ps in ((0, pN10a), (1, pN10b)):
                    for i, a in enumerate((0, 4, 2, 6, 1, 5, 3, 7)):
                        v0 = 896 - 128 * a + 512 * j
                        nc.tensor.matmul(
                            ps[:], f["xcn"][:, BR3[a], :], sN10[:, v0 : v0 + 512],
                            start=(i == 0), stop=(i == 7),
                        )
                    nc.scalar.mul(rN10[:, 512 * j : 512 * j + 512], ps[:], 0.25)
                return rN10

            def mm_t_ring(psum, xres, sname):
                s = strip[sname]
                for a in range(8):
                    v0 = 896 - 128 * a
                    pos = BR2[a] if a < 4 else 4 + BR2[a - 4]
                    nc.tensor.matmul(
                        psum[:], xres[:, pos, :], s[:, v0 : v0 + 512],
                        start=(a == 0), stop=(a == 7),
                    )

            def mms_t(f):
                rTp = ep.tile([128, 1024], bf16, tag="rtp", name="rtp")
                rTm = ep.tile([128, 1024], bf16, tag="rtm", name="rtm")
                for xres, rr, lo_name, hi_name in (
                    (f["xTp"], rTp, "sLp", "sHp"),
                    (f["xTm"], rTm, "sLm", "sHm"),
                ):
                    for j, sname in ((0, lo_name), (1, hi_name)):
                        ph = pp.tile(
                            [128, 512], f32, tag=f"pt{sname}", name=f"pt{sname}"
                        )
                        mm_t_ring(ph, xres, sname)
                        nc.scalar.mul(
                            rr[:, 512 * j : 512 * j + 512], ph[:], SC_T
                        )
                return rTp, rTm

            def emit_rn_chain(rTp, rTm):
                """nega2048 reconstruction from T1024+/- (DVE + ACT scale)."""
                rn = up.tile([128, 2048], bf16, tag="rn", name="rn")
                tt01 = up.tile([128, 1024], bf16, tag="tt01", name="tt01")
                st01 = up.tile([128, 1024], bf16, tag="st01", name="st01")
                nc.vector.tensor_tensor(  # o3
                    rn[:, 1536:2048], rTp[:, 0:512], rTm[:, 0:512], SUB
                )
                nc.vector.tensor_tensor(  # t0
                    tt01[:, 0:512], rTp[:, 0:512], rTm[:, 0:512], ADD
                )
                nc.vector.tensor_tensor(  # o2
                    rn[:, 1024:1536], rTm[:, 512:1024], rTp[:, 512:1024], SUB
                )
                nc.vector.tensor_tensor(  # t1
                    tt01[:, 512:1024], rTp[:, 512:1024], rTm[:, 512:1024], ADD
                )
                nc.scalar.mul(st01[:], tt01[:], R2)
                nc.vector.tensor_tensor(  # o0
                    rn[:, 0:512], st01[:, 0:512], rn[:, 1024:1536], ADD
                )
                nc.vector.tensor_tensor(  # o1
                    rn[:, 512:1024], st01[:, 512:1024], rn[:, 1536:2048], SUB
                )
                return rn

            def emit_rc_part(rCN, rN10):
                rc0 = up.tile([128, 512], bf16, tag="rc0", name="rc0")
                nc.vector.tensor_tensor(
                    rc0[:, 0:256], rCN[:, 0:256], rCN[:, 256:512], ADD
                )
                nc.vector.tensor_tensor(
                    rc0[:, 256:512], rCN[:, 0:256], rCN[:, 256:512], SUB
                )
                rc1 = up.tile([128, 1024], bf16, tag="rc1", name="rc1")
                nc.vector.tensor_tensor(
                    rc1[:, 0:512], rc0[:], rCN[:, 512:1024], ADD
                )
                nc.vector.tensor_tensor(
                    rc1[:, 512:1024], rc0[:], rCN[:, 512:1024], SUB
                )
                rc2 = up.tile([128, 2048], bf16, tag="rc2", name="rc2")
                nc.vector.tensor_tensor(rc2[:, 0:1024], rc1[:], rN10[:], ADD)
                nc.vector.tensor_tensor(rc2[:, 1024:2048], rc1[:], rN10[:], SUB)
                return rc2

            def emit_finals(bt, rc2, rn):
                b0 = 128 * bt
                os01 = op.tile([128, 2048], bf16, tag="os01", name="os01")
                os23 = op.tile([128, 2048], bf16, tag="os23", name="os23")
                nc.vector.tensor_tensor(os01[:], rc2[:], rn[:], ADD)
                nc.gpsimd.tensor_tensor(os23[:], rc2[:], rn[:], SUB)
                nc.sync.dma_start(out_d[b0 : b0 + 128, 0:2048], os01[:])
                nc.sync.dma_start(out_d[b0 : b0 + 128, 2048:4096], os23[:])

            def emit_unfold(bt, rCN, rN10, rTp, rTm):
                rn = emit_rn_chain(rTp, rTm)
                rc2 = emit_rc_part(rCN, rN10)
                emit_finals(bt, rc2, rn)

            def emit_tail_block(f, bt):
                """Last block: T matmuls first (longest reconstruction chain),
                CN next, nega1024 last in two halves; preN = rc1 +- rN10h
                streams each output half out as its rN10 half evacuates.
                All tail combines on DVE (Pool's 2 us ops would pad the end)."""
                b0 = 128 * bt
                rTp, rTm = mms_t(f)
                rCN = mms_cn(f)
                rn = emit_rn_chain(rTp, rTm)
                rc0 = up.tile([128, 512], bf16, tag="rc0", name="rc0")
                nc.vector.tensor_tensor(
                    rc0[:, 0:256], rCN[:, 0:256], rCN[:, 256:512], ADD
                )
                nc.vector.tensor_tensor(
                    rc0[:, 256:512], rCN[:, 0:256], rCN[:, 256:512], SUB
                )
                rc1 = up.tile([128, 1024], bf16, tag="rc1", name="rc1")
                nc.vector.tensor_tensor(
                    rc1[:, 0:512], rc0[:], rCN[:, 512:1024], ADD
                )
                nc.vector.tensor_tensor(
                    rc1[:, 512:1024], rc0[:], rCN[:, 512:1024], SUB
                )
                # P[s] = rc1 +- rn slice: everything except the rN10 term,
                # precomputed while the N10 matmuls run.
                P = up.tile([128, 4, 1024], bf16, tag="P", name="P")
                nc.vector.tensor_tensor(P[:, 0, :], rc1[:], rn[:, 0:1024], ADD)
                nc.vector.tensor_tensor(P[:, 1, :], rc1[:], rn[:, 1024:2048], ADD)
                nc.vector.tensor_tensor(P[:, 2, :], rc1[:], rn[:, 0:1024], SUB)
                nc.vector.tensor_tensor(P[:, 3, :], rc1[:], rn[:, 1024:2048], SUB)
                pN10a = pp.tile([128, 512], f32, tag="pn10a", name="pn10a")
                pN10b = pp.tile([128, 512], f32, tag="pn10b", name="pn10b")
                sN10 = strip["sN10"]
                for j, ps in ((0, pN10a), (1, pN10b)):
                    for i, a in enumerate((0, 4, 2, 6, 1, 5, 3, 7)):
                        v0 = 896 - 128 * a + 512 * j
                        nc.tensor.matmul(
                            ps[:], f["xcn"][:, BR3[a], :], sN10[:, v0 : v0 + 512],
                            start=(i == 0), stop=(i == 7),
                        )
                rN10 = ep.tile([128, 1024], bf16, tag="rn10", name="rn10")
                os_ = op.tile([128, 2, 4, 512], bf16, tag="ost", name="ost")
                # out seg s cols [1024s+c0, +512) = P[s] +- rN10 half
                outv = out_d[:].rearrange("b (s c) -> b s c", c=512)[b0 : b0 + 128]
                for h, ps in ((0, pN10a), (1, pN10b)):
                    c0 = 512 * h
                    rh = rN10[:, c0 : c0 + 512]
                    nc.scalar.mul(rh, ps[:], 0.25)
                    for seg in range(4):
                        nc.vector.tensor_tensor(
                            os_[:, h, seg, :], P[:, seg, c0 : c0 + 512], rh,
                            ADD if seg in (0, 2) else SUB,
                        )
                        if seg % 2 == 1:
                            nc.sync.dma_start(
                                outv[:, 2 * (seg - 1) + h : 2 * seg + h + 1 : 2, :],
                                os_[:, h, seg - 1 : seg + 1, :],
                            )

            # ---------------- preamble ----------------
            nc.gpsimd.memset(warm[:], 0.0)
            xbig = xbigp.tile([128, 32, 128], bf16, tag="xbig", name="xbig")
            # DMA order: x quarters interleaved with strip pieces so the fold
            # chain, C/N strips and N10/T strips all land just in time.
            nc.sync.dma_start(xbig[:, 0:8, :], xtr_d[:, 0, 0:8, :])
            nc.sync.dma_start(xbig[:, 8:16, :], xtr_d[:, 0, 8:16, :])
            nc.sync.dma_start(SB[:, 0:SB_CN], sb_d[:, 0:SB_CN])
            nc.sync.dma_start(xbig[:, 16:24, :], xtr_d[:, 0, 16:24, :])
            nc.sync.dma_start(xbig[:, 24:32, :], xtr_d[:, 0, 24:32, :])
            nc.sync.dma_start(SB[:, SB_CN:SB_N10], sb_d[:, SB_CN:SB_N10])
            for s0 in range(SB_N10, SBW, 1408):
                nc.sync.dma_start(SB[:, s0 : s0 + 1408], sb_d[:, s0 : s0 + 1408])
            # PE clock warmup: HAM releases 2.4 GHz after ~3us of activity;
            # burn dummies while the first DMAs/folds land.
            pW = pp.tile([128, 512], f32, tag="pcn25", name="pwarm")
            for _ in range(N_WARM):
                nc.tensor.matmul(
                    pW[:], warm[:, 0:128], warm[:], start=True, stop=True
                )

            # ---------------- main pipeline ----------------
            # Iteration bt interleaves emissions so each engine's in-order
            # stream matches when its work becomes runnable:
            #   mms+evacs(bt) | rn-chain(bt-1) | early folds(bt+1) |
            #   rc-part(bt-1) | T-folds(bt+1) | finals(bt-1)
            f_cur = emit_folds_first(xbig)
            xbig_next = emit_dma_in(1)
            r_prev = None
            for bt in range(NB - 1):
                xbig = xbig_next
                if bt + 2 < NB:
                    xbig_next = emit_dma_in(bt + 2)
                rCN = mms_cn(f_cur)
                rN10 = mms_n10(f_cur)
                rTp, rTm = mms_t(f_cur)
                if r_prev is not None:
                    rn = emit_rn_chain(r_prev[2], r_prev[3])
                fa = emit_folds_a(xbig, swap=(bt == 0))
                if r_prev is not None:
                    rc2 = emit_rc_part(r_prev[0], r_prev[1])
                f_cur = emit_folds_b(fa)
                if r_prev is not None:
                    emit_finals(bt - 1, rc2, rn)
                r_prev = (rCN, rN10, rTp, rTm)
            emit_unfold(NB - 2, *r_prev)
            emit_tail_block(f_cur, NB - 1)

    nc.compile()
    return nc


def _get_nc():
    if "nc" not in _STATE:
        _STATE["nc"] = _build()
    return _STATE["nc"]


def _make_strip_table(w):
    """All 8 leaf band strips, packed [128, SBW] bf16.

    Strip tiles are shears: tile[p, v] = S[OFF + v - p]. Sequences (t any int,
    Ecyc = wt cyclic):
      Ep(t)   = Ecyc(t) + Ecyc(t+2048)          cyc2048
      En(t)   = Ecyc(t) - Ecyc(t+2048)          nega2048
      Epp(t)  = Ep(t) + Ep(t+1024)              cyc1024
      En10(t) = Ep(t) - Ep(t+1024)              nega1024 leaf
      C5(t)   = Epp(t) + Epp(t+512)             cyc512
      N5(t)   = Epp(t) - Epp(t+512)             nega512 leaf
      C25(t)  = C5(t) + C5(t+256)               cyc256 leaf (x0.5 baked)
      N25(t)  = C5(t) - C5(t+256)               nega256 leaf (x0.5 baked)
      D(t) = En(t) - En(t+1024); Bt(t) = En(t+512) + En(t+1536)
      L+/-(t) = D(t) +- sqrt2 En(t+1536)        T1024 low strips
      H+/-(t) = Bt(t) -+ sqrt2 En(t+1024)       T1024 high strips
    CRT scales (1/8 with the extra 1/2 baked for C25/N25, 1/8, 1/4,
    1/(4sqrt2)) are applied in the ACT PSUM->SBUF evacuations.
    """
    import ml_dtypes

    wt = np.roll(w[::-1], 1).astype(np.float64)
    Ecyc = lambda t: wt[np.mod(t, N)]
    Ep = lambda t: Ecyc(t) + Ecyc(t + 2048)
    En = lambda t: Ecyc(t) - Ecyc(t + 2048)
    Epp = lambda t: Ep(t) + Ep(t + 1024)
    C5 = lambda t: Epp(t) + Epp(t + 512)
    seqs = {
        "sC25": lambda t: 0.5 * (C5(t) + C5(t + 256)),
        "sN25": lambda t: 0.5 * (C5(t) - C5(t + 256)),
        "sN5": lambda t: Epp(t) - Epp(t + 512),
        "sN10": lambda t: Ep(t) - Ep(t + 1024),
        "sLp": lambda t: (En(t) - En(t + 1024)) + R2 * En(t + 1536),
        "sLm": lambda t: (En(t) - En(t + 1024)) - R2 * En(t + 1536),
        "sHp": lambda t: (En(t + 512) + En(t + 1536)) - R2 * En(t + 1024),
        "sHm": lambda t: (En(t + 512) + En(t + 1536)) + R2 * En(t + 1024),
    }
    p = np.arange(128)[:, None]
    tab = np.zeros((128, SBW), dtype=np.float64)
    for name, (c0, width, off) in STRIPS.items():
        v = np.arange(width)[None, :]
        tab[:, c0 : c0 + width] = seqs[name](off + v - p)
    return np.ascontiguousarray(tab.astype(ml_dtypes.bfloat16))


def _prep_inputs(x, w):
    import ml_dtypes

    x = np.ascontiguousarray(x, dtype=np.float32)
    w = np.ascontiguousarray(w, dtype=np.float32)
    sb = _make_strip_table(w)
    in_maps = []
    for i in range(N_CORES):
        xtr = x[i * B_SHARD : (i + 1) * B_SHARD].T  # [N, B_SHARD]
        xtr = xtr.reshape(32, 128, B_SHARD)[BITREV5]  # [a(pos), p, b]
        # [p, bblk, a, bwithin]: per-(p, bblk) an 8KB contiguous bf16 run
        xtr = np.ascontiguousarray(
            xtr.reshape(32, 128, NB, 128).transpose(1, 2, 0, 3).astype(
                ml_dtypes.bfloat16
            )
        )
        in_maps.append({"xtr": xtr, "sb": sb})
    return in_maps


def kernel(x, w, _trace=False):
    from concourse.bass_utils import run_bass_kernel_spmd

    nc = _get_nc()
    in_maps = _prep_inputs(x, w)
    res = run_bass_kernel_spmd(nc, in_maps, list(range(N_CORES)), trace=_trace)
    out = np.concatenate(
        [np.asarray(res.results[i]["out"]).astype(np.float32) for i in range(N_CORES)],
        axis=0,
    )
    if _trace:
        _STATE["last_result"] = res
    return out


# revision 38
# speedup vs baseline: 1.1175x; 1.0008x over previous
"""Circulant matmul for TRN2: 4.5-level CRT with trinomial split, bf16 matmuls.

out[b, r] = sum_c x[b,c] w[(c-r) mod N] = (x (*) wt)[r], wt = roll(w[::-1],1)
(cyclic convolution along c) — no input flip or output reversal needed.

CRT tree on z^4096-1 (per 128-row block of x):
  cyc4096 -> cyc2048 + nega2048             (binomial, band scale 1/2 each)
  cyc2048 -> cyc1024 + nega1024[LEAF]       (1/2)
  cyc1024 -> cyc512 + nega512[LEAF]         (1/2)
  cyc512  -> cyc256[LEAF] + nega256[LEAF]   (1/2)
  nega2048 -> T1024+ , T1024- [LEAVES]      (z^1024 -+ sqrt2 z^512 + 1, 1/(2sqrt2))
Leaf matmuls: 54 x bf16 = 27648 PE cycles/block.  x chunks are stored in
bit-reversed order so EVERY fold level is a stride-2 slice and any quarter
of block 0 folds to completion as its DMA lands (fast pipeline fill).
I/O is bf16 both ways (host converts); per-block HBM traffic = 2 MiB.
Finals are two [128,2048] ops (DVE + Pool) feeding two output DMAs.
"""

import sys

sys.path.insert(0, "/opt/trn_rl_repo")

import numpy as np

N = 4096
B = 8192
N_CORES = 8
B_SHARD = B // N_CORES  # 1024
NB = B_SHARD // 128     # 8 row-tiles per core
R2 = float(np.sqrt(2.0))
SC_T = float(1.0 / (4.0 * np.sqrt(2.0)))   # T1024 leaves: 1/2 * 1/(2sqrt2)

# bit-reversal position maps: leaf-residue chunk a lives at fold-output
# position BRk[a] (k = log2 nchunks)
BR2 = [0, 2, 1, 3]
BR3 = [0, 4, 2, 6, 1, 5, 3, 7]
# x chunk stored at position i is BITREV5[i]
BITREV5 = [
    ((i & 1) << 4) | ((i & 2) << 2) | (i & 4) | ((i & 8) >> 2) | ((i & 16) >> 4)
    for i in range(32)
]

# strip layout in the packed [128, SBW] host table (name: col0, width, OFF)
STRIPS = {
    "sC25": (0, 384, -128),
    "sN25": (384, 384, -128),
    "sN5": (768, 896, -384),
    "sN10": (1664, 1920, -896),
    "sLp": (3584, 1408, -896),
    "sHp": (4992, 1408, -896),
    "sLm": (6400, 1408, -896),
    "sHm": (7808, 1408, -896),
}
SBW = 9216
SB_CN = 1664    # sC25+sN25+sN5
SB_N10 = 3584   # ..sN10

_STATE = {}

N_WARM = 1


def _build():
    import concourse.bacc as bacc
    import concourse.mybir as mybir
    import concourse.tile as tile

    f32 = mybir.dt.float32
    bf16 = mybir.dt.bfloat16
    ADD = mybir.AluOpType.add
    SUB = mybir.AluOpType.subtract

    nc = bacc.Bacc("TRN2", target_bir_lowering=False, debug=False)
    # x transposed, chunk-bit-reversed, bf16 on host; laid out
    # [p, bblk, a, bwithin] so each block is an 8KB contiguous run/partition.
    xtr_d = nc.declare_dram_parameter("xtr", [128, NB, 32, 128], bf16, isOutput=False)
    sb_d = nc.declare_dram_parameter("sb", [128, SBW], bf16, isOutput=False)
    out_d = nc.declare_dram_parameter("out", [B_SHARD, N], bf16, isOutput=True)

    with tile.TileContext(nc) as tc:
        with (
            tc.tile_pool(name="const", bufs=1) as constp,
            tc.tile_pool(name="xbig", bufs=3) as xbigp,
            tc.tile_pool(name="fold", bufs=2) as fp,
            tc.tile_pool(name="evac", bufs=3) as ep,
            tc.tile_pool(name="unf", bufs=2) as up,
            tc.tile_pool(name="outp", bufs=2) as op,
            tc.tile_pool(name="psum", bufs=1, space="PSUM") as pp,
        ):
            SB = constp.tile([128, SBW], bf16, name="SB")
            warm = constp.tile([128, 512], bf16, name="warm")
            strip = {k: SB[:, c0 : c0 + w] for k, (c0, w, _) in STRIPS.items()}

            def emit_dma_in(bt):
                xbig = xbigp.tile([128, 32, 128], bf16, tag="xbig", name="xbig")
                nc.sync.dma_start(xbig[:], xtr_d[:, bt, :, :])
                return xbig

            def emit_folds_a(xbig, swap=False):
                """Early folds: xc (Pool), cyc chain (DVE) — these feed the
                first matmuls of the next block. swap=True runs xc on DVE."""
                xc = fp.tile([128, 16, 128], bf16, tag="xc", name="xc")
                xn = fp.tile([128, 16, 128], bf16, tag="xn", name="xn")
                if swap:
                    nc.vector.tensor_tensor(
                        xc[:], xbig[:, 0:32:2, :], xbig[:, 1:32:2, :], ADD
                    )
                    nc.gpsimd.tensor_tensor(
                        xn[:], xbig[:, 0:32:2, :], xbig[:, 1:32:2, :], SUB
                    )
                else:
                    nc.gpsimd.tensor_tensor(
                        xc[:], xbig[:, 0:32:2, :], xbig[:, 1:32:2, :], ADD
                    )
                    nc.vector.tensor_tensor(
                        xn[:], xbig[:, 0:32:2, :], xbig[:, 1:32:2, :], SUB
                    )
                xcc = fp.tile([128, 8, 128], bf16, tag="xcc", name="xcc")
                nc.vector.tensor_tensor(
                    xcc[:], xc[:, 0:16:2, :], xc[:, 1:16:2, :], ADD
                )
                xccc = fp.tile([128, 4, 128], bf16, tag="xccc", name="xccc")
                nc.vector.tensor_tensor(
                    xccc[:], xcc[:, 0:8:2, :], xcc[:, 1:8:2, :], ADD
                )
                xccn = fp.tile([128, 4, 128], bf16, tag="xccn", name="xccn")
                nc.vector.tensor_tensor(
                    xccn[:], xcc[:, 0:8:2, :], xcc[:, 1:8:2, :], SUB
                )
                xc4 = fp.tile([128, 2, 128], bf16, tag="xc4", name="xc4")
                nc.vector.tensor_tensor(
                    xc4[:], xccc[:, 0:4:2, :], xccc[:, 1:4:2, :], ADD
                )
                xn4 = fp.tile([128, 2, 128], bf16, tag="xn4", name="xn4")
                nc.vector.tensor_tensor(
                    xn4[:], xccc[:, 0:4:2, :], xccc[:, 1:4:2, :], SUB
                )
                xcn = fp.tile([128, 8, 128], bf16, tag="xcn", name="xcn")
                nc.vector.tensor_tensor(
                    xcn[:], xc[:, 0:16:2, :], xc[:, 1:16:2, :], SUB
                )
                return dict(xn=xn, xcn=xcn, xc4=xc4, xn4=xn4, xccn=xccn)

            def emit_folds_b(fa):
                """Late folds: trinomial xTp/xTm (needed only by the T
                matmuls, mid-block). sqrt2 scales on ACT."""
                xn = fa["xn"]
                xA = fp.tile([128, 4, 128], bf16, tag="xA", name="xA")
                nc.vector.tensor_tensor(
                    xA[:], xn[:, 0:16:4, :], xn[:, 1:16:4, :], SUB
                )
                xB = fp.tile([128, 4, 128], bf16, tag="xB", name="xB")
                nc.vector.tensor_tensor(
                    xB[:], xn[:, 2:16:4, :], xn[:, 3:16:4, :], ADD
                )
                s10 = fp.tile([128, 4, 128], bf16, tag="s10", name="s10")
                nc.scalar.mul(s10[:], xn[:, 1:16:4, :], R2)
                s11 = fp.tile([128, 4, 128], bf16, tag="s11", name="s11")
                nc.scalar.mul(s11[:], xn[:, 3:16:4, :], R2)
                xTp = fp.tile([128, 8, 128], bf16, tag="xTp", name="xTp")
                nc.vector.tensor_tensor(xTp[:, 0:4, :], xA[:], s11[:], ADD)
                nc.vector.tensor_tensor(xTp[:, 4:8, :], xB[:], s10[:], SUB)
                xTm = fp.tile([128, 8, 128], bf16, tag="xTm", name="xTm")
                nc.vector.tensor_tensor(xTm[:, 0:4, :], xA[:], s11[:], SUB)
                nc.vector.tensor_tensor(xTm[:, 4:8, :], xB[:], s10[:], ADD)
                fa["xTp"] = xTp
                fa["xTm"] = xTm
                return fa

            def emit_folds_first(xbig):
                """Block 0: fold each quarter's cyc chain as its DMA lands;
                xn/xcn/xc4 pieces follow. All on DVE (Pool too slow to gate)."""
                xc = fp.tile([128, 16, 128], bf16, tag="xc", name="xc")
                xn = fp.tile([128, 16, 128], bf16, tag="xn", name="xn")
                xcc = fp.tile([128, 8, 128], bf16, tag="xcc", name="xcc")
                xccc = fp.tile([128, 4, 128], bf16, tag="xccc", name="xccc")
                xccn = fp.tile([128, 4, 128], bf16, tag="xccn", name="xccn")
                xc4 = fp.tile([128, 2, 128], bf16, tag="xc4", name="xc4")
                xn4 = fp.tile([128, 2, 128], bf16, tag="xn4", name="xn4")
                xcn = fp.tile([128, 8, 128], bf16, tag="xcn", name="xcn")
                for q in range(4):
                    s = 8 * q
                    nc.vector.tensor_tensor(
                        xc[:, 4 * q : 4 * q + 4, :],
                        xbig[:, s : s + 8 : 2, :],
                        xbig[:, s + 1 : s + 8 : 2, :], ADD
                    )
                    nc.vector.tensor_tensor(
                        xcc[:, 2 * q : 2 * q + 2, :],
                        xc[:, 4 * q : 4 * q + 4 : 2, :],
                        xc[:, 4 * q + 1 : 4 * q + 4 : 2, :], ADD
                    )
                    nc.vector.tensor_tensor(
                        xccc[:, q : q + 1, :],
                        xcc[:, 2 * q : 2 * q + 1, :],
                        xcc[:, 2 * q + 1 : 2 * q + 2, :], ADD
                    )
                    nc.vector.tensor_tensor(
                        xccn[:, q : q + 1, :],
                        xcc[:, 2 * q : 2 * q + 1, :],
                        xcc[:, 2 * q + 1 : 2 * q + 2, :], SUB
                    )
                    if q == 1 or q == 3:
                        h = q // 2  # xc4/xn4 entry h from xccc pos 2h, 2h+1
                        nc.vector.tensor_tensor(
                            xc4[:, h : h + 1, :],
                            xccc[:, 2 * h : 2 * h + 1, :],
                            xccc[:, 2 * h + 1 : 2 * h + 2, :], ADD
                        )
                        nc.vector.tensor_tensor(
                            xn4[:, h : h + 1, :],
                            xccc[:, 2 * h : 2 * h + 1, :],
                            xccc[:, 2 * h + 1 : 2 * h + 2, :], SUB
                        )
                # xcn/xn (needed only by the later N10/T matmuls) after the
                # cyc chains so they don't delay the q2/q3 fold chains
                for h in range(2):
                    nc.vector.tensor_tensor(
                        xcn[:, 4 * h : 4 * h + 4, :],
                        xc[:, 8 * h : 8 * h + 8 : 2, :],
                        xc[:, 8 * h + 1 : 8 * h + 8 : 2, :], SUB
                    )
                    nc.vector.tensor_tensor(
                        xn[:, 8 * h : 8 * h + 8, :],
                        xbig[:, 16 * h : 16 * h + 16 : 2, :],
                        xbig[:, 16 * h + 1 : 16 * h + 16 : 2, :], SUB
                    )
                fa = dict(xn=xn, xcn=xcn, xc4=xc4, xn4=xn4, xccn=xccn)
                return emit_folds_b(fa)

            def mms_cn(f):
                """cyc256+nega256 (shared psum bank) and nega512 leaves."""
                pCN25 = pp.tile([128, 512], f32, tag="pcn25", name="pcn25")
                for a in range(2):
                    v0 = 128 * (1 - a)
                    nc.tensor.matmul(
                        pCN25[:, 0:256], f["xc4"][:, a, :],
                        strip["sC25"][:, v0 : v0 + 256],
                        start=(a == 0), stop=(a == 1),
                    )
                for a in range(2):
                    v0 = 128 * (1 - a)
                    nc.tensor.matmul(
                        pCN25[:, 256:512], f["xn4"][:, a, :],
                        strip["sN25"][:, v0 : v0 + 256],
                        start=(a == 0), stop=(a == 1),
                    )
                pN5 = pp.tile([128, 512], f32, tag="pn5", name="pn5")
                for i, a in enumerate((0, 2, 1, 3)):  # feed order: quarter BR2[a]
                    v0 = 128 * (3 - a)
                    nc.tensor.matmul(
                        pN5[:], f["xccn"][:, BR2[a], :],
                        strip["sN5"][:, v0 : v0 + 512],
                        start=(i == 0), stop=(i == 3),
                    )
                rCN = ep.tile([128, 1024], bf16, tag="rcn", name="rcn")
                nc.scalar.mul(rCN[:, 0:512], pCN25[:], 0.125)
                nc.scalar.mul(rCN[:, 512:1024], pN5[:], 0.125)
                return rCN

            def mms_n10(f):
                """nega1024 in two half-PSUM tiles (tail streams per half)."""
                pN10a = pp.tile([128, 512], f32, tag="pn10a", name="pn10a")
                pN10b = pp.tile([128, 512], f32, tag="pn10b", name="pn10b")
                sN10 = strip["sN10"]
                rN10 = ep.tile([128, 1024], bf16, tag="rn10", name="rn10")
                for j, ps in ((0, pN10a), (1, pN10b)):
                    for i, a in enumerate((0, 4, 2, 6, 1, 5, 3, 7)):
                        v0 = 896 - 128 * a + 512 * j
                        nc.tensor.matmul(
                            ps[:], f["xcn"][:, BR3[a], :], sN10[:, v0 : v0 + 512],
                            start=(i == 0), stop=(i == 7),
                        )
                    nc.scalar.mul(rN10[:, 512 * j : 512 * j + 512], ps[:], 0.25)
                return rN10

            def mm_t_ring(psum, xres, sname):
                s = strip[sname]
                for a in range(8):
                    v0 = 896 - 128 * a
                    pos = BR2[a] if a < 4 else 4 + BR2[a - 4]
                    nc.tensor.matmul(
                        psum[:], xres[:, pos, :], s[:, v0 : v0 + 512],
                        start=(a == 0), stop=(a == 7),
                    )

            def mms_t(f, lo_first=False):
                """lo_first runs both low strips before the high strips so
                the rn chain's o3/t0 diffs can start ~1.7us earlier (tail)."""
                rTp = ep.tile([128, 1024], bf16, tag="rtp", name="rtp")
                rTm = ep.tile([128, 1024], bf16, tag="rtm", name="rtm")
                work = [
                    (f["xTp"], rTp, 0, "sLp"), (f["xTp"], rTp, 1, "sHp"),
                    (f["xTm"], rTm, 0, "sLm"), (f["xTm"], rTm, 1, "sHm"),
                ]
                if lo_first:
                    work = [work[0], work[2], work[1], work[3]]
                for xres, rr, j, sname in work:
                    ph = pp.tile(
                        [128, 512], f32, tag=f"pt{sname}", name=f"pt{sname}"
                    )
                    mm_t_ring(ph, xres, sname)
                    nc.scalar.mul(
                        rr[:, 512 * j : 512 * j + 512], ph[:], SC_T
                    )
                return rTp, rTm

            def emit_rn_chain(rTp, rTm):
                """nega2048 reconstruction from T1024+/- (DVE + ACT scale)."""
                rn = up.tile([128, 2048], bf16, tag="rn", name="rn")
                tt01 = up.tile([128, 1024], bf16, tag="tt01", name="tt01")
                st01 = up.tile([128, 1024], bf16, tag="st01", name="st01")
                nc.vector.tensor_tensor(  # o3
                    rn[:, 1536:2048], rTp[:, 0:512], rTm[:, 0:512], SUB
                )
                nc.vector.tensor_tensor(  # t0
                    tt01[:, 0:512], rTp[:, 0:512], rTm[:, 0:512], ADD
                )
                nc.vector.tensor_tensor(  # o2
                    rn[:, 1024:1536], rTm[:, 512:1024], rTp[:, 512:1024], SUB
                )
                nc.vector.tensor_tensor(  # t1
                    tt01[:, 512:1024], rTp[:, 512:1024], rTm[:, 512:1024], ADD
                )
                nc.scalar.mul(st01[:], tt01[:], R2)
                nc.vector.tensor_tensor(  # o0
                    rn[:, 0:512], st01[:, 0:512], rn[:, 1024:1536], ADD
                )
                nc.vector.tensor_tensor(  # o1
                    rn[:, 512:1024], st01[:, 512:1024], rn[:, 1536:2048], SUB
                )
                return rn

            def emit_rc_part(rCN, rN10):
                rc0 = up.tile([128, 512], bf16, tag="rc0", name="rc0")
                nc.vector.tensor_tensor(
                    rc0[:, 0:256], rCN[:, 0:256], rCN[:, 256:512], ADD
                )
                nc.vector.tensor_tensor(
                    rc0[:, 256:512], rCN[:, 0:256], rCN[:, 256:512], SUB
                )
                rc1 = up.tile([128, 1024], bf16, tag="rc1", name="rc1")
                nc.vector.tensor_tensor(
                    rc1[:, 0:512], rc0[:], rCN[:, 512:1024], ADD
                )
                nc.vector.tensor_tensor(
                    rc1[:, 512:1024], rc0[:], rCN[:, 512:1024], SUB
                )
                rc2 = up.tile([128, 2048], bf16, tag="rc2", name="rc2")
                nc.vector.tensor_tensor(rc2[:, 0:1024], rc1[:], rN10[:], ADD)
                nc.vector.tensor_tensor(rc2[:, 1024:2048], rc1[:], rN10[:], SUB)
                return rc2

            def emit_finals(bt, rc2, rn, pool=True):
                b0 = 128 * bt
                os01 = op.tile([128, 2048], bf16, tag="os01", name="os01")
                os23 = op.tile([128, 2048], bf16, tag="os23", name="os23")
                nc.vector.tensor_tensor(os01[:], rc2[:], rn[:], ADD)
                if pool:
                    nc.gpsimd.tensor_tensor(os23[:], rc2[:], rn[:], SUB)
                else:
                    nc.vector.tensor_tensor(os23[:], rc2[:], rn[:], SUB)
                nc.sync.dma_start(out_d[b0 : b0 + 128, 0:2048], os01[:])
                nc.sync.dma_start(out_d[b0 : b0 + 128, 2048:4096], os23[:])

            def emit_unfold(bt, rCN, rN10, rTp, rTm):
                rn = emit_rn_chain(rTp, rTm)
                rc2 = emit_rc_part(rCN, rN10)
                emit_finals(bt, rc2, rn)

            def emit_tail_rc1(rCN):
                rc0 = up.tile([128, 512], bf16, tag="rc0", name="rc0")
                nc.vector.tensor_tensor(
                    rc0[:, 0:256], rCN[:, 0:256], rCN[:, 256:512], ADD
                )
                nc.vector.tensor_tensor(
                    rc0[:, 256:512], rCN[:, 0:256], rCN[:, 256:512], SUB
                )
                rc1 = up.tile([128, 1024], bf16, tag="rc1", name="rc1")
                nc.vector.tensor_tensor(
                    rc1[:, 0:512], rc0[:], rCN[:, 512:1024], ADD
                )
                nc.vector.tensor_tensor(
                    rc1[:, 512:1024], rc0[:], rCN[:, 512:1024], SUB
                )
                return rc1

            def emit_tail_rn_p(rTp, rTm, rc1):
                """Tail rn chain with P1/P3 = rc1 +- rn-hi interleaved right
                after the o2/o3 diffs (before the st01-gated o0/o1) so they
                are hidden under the N10 matmuls."""
                rn = up.tile([128, 2048], bf16, tag="rn", name="rn")
                tt01 = up.tile([128, 1024], bf16, tag="tt01", name="tt01")
                st01 = up.tile([128, 1024], bf16, tag="st01", name="st01")
                P = up.tile([128, 2, 1024], bf16, tag="P", name="P")
                nc.vector.tensor_tensor(  # o3
                    rn[:, 1536:2048], rTp[:, 0:512], rTm[:, 0:512], SUB
                )
                nc.vector.tensor_tensor(  # t0
                    tt01[:, 0:512], rTp[:, 0:512], rTm[:, 0:512], ADD
                )
                nc.vector.tensor_tensor(  # o2
                    rn[:, 1024:1536], rTm[:, 512:1024], rTp[:, 512:1024], SUB
                )
                nc.vector.tensor_tensor(  # t1
                    tt01[:, 512:1024], rTp[:, 512:1024], rTm[:, 512:1024], ADD
                )
                nc.vector.tensor_tensor(P[:, 0, :], rc1[:], rn[:, 1024:2048], ADD)
                nc.vector.tensor_tensor(P[:, 1, :], rc1[:], rn[:, 1024:2048], SUB)
                nc.vector.tensor_scalar_mul(st01[:], tt01[:], R2)  # DVE 4x mode
                nc.vector.tensor_tensor(  # o0
                    rn[:, 0:512], st01[:, 0:512], rn[:, 1024:1536], ADD
                )
                nc.vector.tensor_tensor(  # o1
                    rn[:, 512:1024], st01[:, 512:1024], rn[:, 1536:2048], SUB
                )
                return rn, P

            def emit_tail_n10(f):
                pN10a = pp.tile([128, 512], f32, tag="pn10a", name="pn10a")
                pN10b = pp.tile([128, 512], f32, tag="pn10b", name="pn10b")
                sN10 = strip["sN10"]
                for j, ps in ((0, pN10a), (1, pN10b)):
                    for i, a in enumerate((0, 4, 2, 6, 1, 5, 3, 7)):
                        v0 = 896 - 128 * a + 512 * j
                        nc.tensor.matmul(
                            ps[:], f["xcn"][:, BR3[a], :], sN10[:, v0 : v0 + 512],
                            start=(i == 0), stop=(i == 7),
                        )
                return pN10a, pN10b

            def emit_tail_finish(pN10a, pN10b, rc1, rn, P, bt, tg):
                """Streamed ending: P holds rc1 +- rn-hi; after each nega1024
                half-psum stops, only evac -> 5 small combines -> 2 strided
                DMAs remain."""
                b0 = 128 * bt
                rN10 = ep.tile([128, 1024], bf16, tag="rn10" + tg, name="rn10")
                preA = up.tile([128, 1024], bf16, tag="preA", name="preA")
                # os_ slot order per half: (s1, s3, s0, s2)
                os_ = op.tile([128, 2, 4, 512], bf16, tag="ost" + tg, name="ost")
                outv = out_d[:].rearrange("b (s c) -> b s c", c=512)[b0 : b0 + 128]
                for h, ps in ((0, pN10a), (1, pN10b)):
                    c0 = 512 * h
                    rh = rN10[:, c0 : c0 + 512]
                    nc.scalar.mul(rh, ps[:], 0.25)
                    nc.vector.tensor_tensor(  # s1 = P1 - rN10
                        os_[:, h, 0, :], P[:, 0, c0 : c0 + 512], rh, SUB
                    )
                    nc.vector.tensor_tensor(  # s3 = P3 - rN10
                        os_[:, h, 1, :], P[:, 1, c0 : c0 + 512], rh, SUB
                    )
                    nc.sync.dma_start(
                        outv[:, 2 + h : 7 + h : 4, :], os_[:, h, 0:2, :]
                    )
                    pa = preA[:, c0 : c0 + 512]
                    nc.vector.tensor_tensor(pa, rc1[:, c0 : c0 + 512], rh, ADD)
                    nc.vector.tensor_tensor(  # s0 = preA + rn-lo
                        os_[:, h, 2, :], pa, rn[:, c0 : c0 + 512], ADD
                    )
                    nc.vector.tensor_tensor(  # s2 = preA - rn-lo
                        os_[:, h, 3, :], pa, rn[:, c0 : c0 + 512], SUB
                    )
                    nc.sync.dma_start(
                        outv[:, h : 5 + h : 4, :], os_[:, h, 2:4, :]
                    )

            # ---------------- preamble ----------------
            nc.gpsimd.memset(warm[:], 0.0)
            xbig = xbigp.tile([128, 32, 128], bf16, tag="xbig", name="xbig")
            # DMA order: x quarters interleaved with strip pieces so the fold
            # chain, C/N strips and N10/T strips all land just in time.
            nc.sync.dma_start(xbig[:, 0:8, :], xtr_d[:, 0, 0:8, :])
            nc.sync.dma_start(xbig[:, 8:16, :], xtr_d[:, 0, 8:16, :])
            nc.sync.dma_start(SB[:, 0:768], sb_d[:, 0:768])  # sC25+sN25
            nc.sync.dma_start(xbig[:, 16:24, :], xtr_d[:, 0, 16:24, :])
            nc.sync.dma_start(SB[:, 768:SB_CN], sb_d[:, 768:SB_CN])  # sN5
            nc.sync.dma_start(xbig[:, 24:32, :], xtr_d[:, 0, 24:32, :])
            nc.sync.dma_start(SB[:, SB_CN:SB_N10], sb_d[:, SB_CN:SB_N10])
            for s0 in range(SB_N10, SBW, 1408):
                nc.sync.dma_start(SB[:, s0 : s0 + 1408], sb_d[:, s0 : s0 + 1408])
            # PE clock warmup: HAM releases 2.4 GHz after ~3us of activity;
            # burn dummies while the first DMAs/folds land.
            pW = pp.tile([128, 512], f32, tag="pcn25", name="pwarm")
            for _ in range(N_WARM):
                nc.tensor.matmul(
                    pW[:], warm[:, 0:128], warm[:], start=True, stop=True
                )

            # ---------------- main pipeline ----------------
            # Iteration bt interleaves emissions so each engine's in-order
            # stream matches when its work becomes runnable:
            #   mms+evacs(bt) | rn-chain(bt-1) | early folds(bt+1) |
            #   rc-part(bt-1) | T-folds(bt+1) | finals(bt-1)
            f_cur = emit_folds_first(xbig)
            xbig_next = emit_dma_in(1)
            r_prev = None
            for bt in range(NB - 2):
                xbig = xbig_next
                if bt + 2 < NB:
                    xbig_next = emit_dma_in(bt + 2)
                rCN = mms_cn(f_cur)
                rN10 = mms_n10(f_cur)
                rTp, rTm = mms_t(f_cur)
                if r_prev is not None:
                    rn = emit_rn_chain(r_prev[2], r_prev[3])
                fa = emit_folds_a(xbig, swap=(bt == 0))
                if r_prev is not None:
                    rc2 = emit_rc_part(r_prev[0], r_prev[1])
                f_cur = emit_folds_b(fa)
                if r_prev is not None:
                    emit_finals(bt - 1, rc2, rn)
                r_prev = (rCN, rN10, rTp, rTm)
            # last two blocks: block 7's T matmuls hoisted before block 6's
            # so the tail's long recon chains overlap remaining PE work
            f6 = f_cur
            rCN6 = mms_cn(f6)
            rN10_6 = mms_n10(f6)
            rn5 = emit_rn_chain(r_prev[2], r_prev[3])
            fa7 = emit_folds_a(xbig_next)
            rc2_5 = emit_rc_part(r_prev[0], r_prev[1])
            f7 = emit_folds_b(fa7)
            emit_finals(NB - 3, rc2_5, rn5)
            rTp6, rTm6 = mms_t(f6)
            rCN7 = mms_cn(f7)
            rn6 = emit_rn_chain(rTp6, rTm6)
            rc2_6 = emit_rc_part(rCN6, rN10_6)
            rTp7, rTm7 = mms_t(f7, lo_first=True)
            rc1_7 = emit_tail_rc1(rCN7)
            emit_finals(NB - 2, rc2_6, rn6, pool=False)
            rn7, P7 = emit_tail_rn_p(rTp7, rTm7, rc1_7)
            pa7, pb7 = emit_tail_n10(f7)
            emit_tail_finish(pa7, pb7, rc1_7, rn7, P7, NB - 1, "")

    nc.compile()
    return nc


def _get_nc():
    if "nc" not in _STATE:
        _STATE["nc"] = _build()
    return _STATE["nc"]


def _make_strip_table(w):
    """All 8 leaf band strips, packed [128, SBW] bf16.

    Strip tiles are shears: tile[p, v] = S[OFF + v - p]. Sequences (t any int,
    Ecyc = wt cyclic):
      Ep(t)   = Ecyc(t) + Ecyc(t+2048)          cyc2048
      En(t)   = Ecyc(t) - Ecyc(t+2048)          nega2048
      Epp(t)  = Ep(t) + Ep(t+1024)              cyc1024
      En10(t) = Ep(t) - Ep(t+1024)              nega1024 leaf
      C5(t)   = Epp(t) + Epp(t+512)             cyc512
      N5(t)   = Epp(t) - Epp(t+512)             nega512 leaf
      C25(t)  = C5(t) + C5(t+256)               cyc256 leaf (x0.5 baked)
      N25(t)  = C5(t) - C5(t+256)               nega256 leaf (x0.5 baked)
      D(t) = En(t) - En(t+1024); Bt(t) = En(t+512) + En(t+1536)
      L+/-(t) = D(t) +- sqrt2 En(t+1536)        T1024 low strips
      H+/-(t) = Bt(t) -+ sqrt2 En(t+1024)       T1024 high strips
    CRT scales (1/8 with the extra 1/2 baked for C25/N25, 1/8, 1/4,
    1/(4sqrt2)) are applied in the ACT PSUM->SBUF evacuations.
    """
    import ml_dtypes

    wt = np.roll(w[::-1], 1).astype(np.float64)
    Ecyc = lambda t: wt[np.mod(t, N)]
    Ep = lambda t: Ecyc(t) + Ecyc(t + 2048)
    En = lambda t: Ecyc(t) - Ecyc(t + 2048)
    Epp = lambda t: Ep(t) + Ep(t + 1024)
    C5 = lambda t: Epp(t) + Epp(t + 512)
    seqs = {
        "sC25": lambda t: 0.5 * (C5(t) + C5(t + 256)),
        "sN25": lambda t: 0.5 * (C5(t) - C5(t + 256)),
        "sN5": lambda t: Epp(t) - Epp(t + 512),
        "sN10": lambda t: Ep(t) - Ep(t + 1024),
        "sLp": lambda t: (En(t) - En(t + 1024)) + R2 * En(t + 1536),
        "sLm": lambda t: (En(t) - En(t + 1024)) - R2 * En(t + 1536),
        "sHp": lambda t: (En(t + 512) + En(t + 1536)) - R2 * En(t + 1024),
        "sHm": lambda t: (En(t + 512) + En(t + 1536)) + R2 * En(t + 1024),
    }
    p = np.arange(128)[:, None]
    tab = np.zeros((128, SBW), dtype=np.float64)
    for name, (c0, width, off) in STRIPS.items():
        v = np.arange(width)[None, :]
        tab[:, c0 : c0 + width] = seqs[name](off + v - p)
    return np.ascontiguousarray(tab.astype(ml_dtypes.bfloat16))


def _prep_inputs(x, w):
    import ml_dtypes

    x = np.ascontiguousarray(x, dtype=np.float32)
    w = np.ascontiguousarray(w, dtype=np.float32)
    sb = _make_strip_table(w)
    in_maps = []
    for i in range(N_CORES):
        xtr = x[i * B_SHARD : (i + 1) * B_SHARD].T  # [N, B_SHARD]
        xtr = xtr.reshape(32, 128, B_SHARD)[BITREV5]  # [a(pos), p, b]
        # [p, bblk, a, bwithin]: per-(p, bblk) an 8KB contiguous bf16 run
        xtr = np.ascontiguousarray(
            xtr.reshape(32, 128, NB, 128).transpose(1, 2, 0, 3).astype(
                ml_dtypes.bfloat16
            )
        )
        in_maps.append({"xtr": xtr, "sb": sb})
    return in_maps


def kernel(x, w, _trace=False):
    from concourse.bass_utils import run_bass_kernel_spmd

    nc = _get_nc()
    in_maps = _prep_inputs(x, w)
    res = run_bass_kernel_spmd(nc, in_maps, list(range(N_CORES)), trace=_trace)
    out = np.concatenate(
        [np.asarray(res.results[i]["out"]).astype(np.float32) for i in range(N_CORES)],
        axis=0,
    )
    if _trace:
        _STATE["last_result"] = res
    return out


# revision 57
# speedup vs baseline: 1.1291x; 1.0104x over previous
"""Circulant matmul for TRN2: 4.5-level CRT with trinomial split, bf16 matmuls.

out[b, r] = sum_c x[b,c] w[(c-r) mod N] = (x (*) wt)[r], wt = roll(w[::-1],1)
(cyclic convolution along c) — no input flip or output reversal needed.

CRT tree on z^4096-1 (per 128-row block of x):
  cyc4096 -> cyc2048 + nega2048             (binomial, band scale 1/2 each)
  cyc2048 -> cyc1024 + nega1024[LEAF]       (1/2)
  cyc1024 -> cyc512 + nega512[LEAF]         (1/2)
  cyc512  -> cyc256[LEAF] + nega256[LEAF]   (1/2)
  nega2048 -> T1024+ , T1024- [LEAVES]      (z^1024 -+ sqrt2 z^512 + 1, 1/(2sqrt2))
Leaf matmuls: 54 x bf16 = 27648 PE cycles/block (11.5us at 2.4GHz; the
f32 baseline's direct matmul would be 54.6us).  x chunks are stored in
bit-reversed order so EVERY fold level is a stride-2 slice and any quarter
of block 0 folds to completion as its DMA lands (fast pipeline fill).
I/O is bf16 both ways (host converts; rel-err budget allows it); per-block
HBM traffic is 2 MiB so the exclusive 360 B/ns DMA engine stays at ~50%.
Steady finals are two [128,2048] ops (DVE + Pool) feeding two output DMAs.
The last block streams out per nega1024 half-psum: P = rc1 +- rn-hi is
precomputed under the matmuls (split so the o3-dependent halves are
emitted early) and each half needs only evac -> 5 combines -> 2 strided
DMAs after its psum stops.  A single warmup matmul at ~1.3us starts the
PE p-state ramp clock so real matmuls from ~4.3us run at full clock; a
few more bridge the idle so the ramp timer never resets.
Per-block engine busy: PE 11.9 (critical), DVE 11.2, Pool 7.4, ACT 7.2,
DMA 6.6.  Makespan = fill ~6.1 + PE stream ~93 + drain ~7.
"""

import sys

sys.path.insert(0, "/opt/trn_rl_repo")

import numpy as np

N = 4096
B = 8192
N_CORES = 8
B_SHARD = B // N_CORES  # 1024
NB = B_SHARD // 128     # 8 row-tiles per core
R2 = float(np.sqrt(2.0))
SC_T = float(1.0 / (4.0 * np.sqrt(2.0)))   # T1024 leaves: 1/2 * 1/(2sqrt2)

# bit-reversal position maps: leaf-residue chunk a lives at fold-output
# position BRk[a] (k = log2 nchunks)
BR2 = [0, 2, 1, 3]
BR3 = [0, 4, 2, 6, 1, 5, 3, 7]
# x chunk stored at position i is BITREV5[i]
BITREV5 = [
    ((i & 1) << 4) | ((i & 2) << 2) | (i & 4) | ((i & 8) >> 2) | ((i & 16) >> 4)
    for i in range(32)
]

# strip layout in the packed [128, SBW] host table (name: col0, width, OFF)
STRIPS = {
    "sC25": (0, 384, -128),
    "sN25": (384, 384, -128),
    "sN5": (768, 896, -384),
    "sN10": (1664, 1920, -896),
    "sLp": (3584, 1408, -896),
    "sHp": (4992, 1408, -896),
    "sLm": (6400, 1408, -896),
    "sHm": (7808, 1408, -896),
}
SBW = 9216
SB_CN = 1664    # sC25+sN25+sN5
SB_N10 = 3584   # ..sN10

_STATE = {}

N_WARM = 10


def _build():
    import concourse.bacc as bacc
    import concourse.mybir as mybir
    import concourse.tile as tile

    f32 = mybir.dt.float32
    bf16 = mybir.dt.bfloat16
    ADD = mybir.AluOpType.add
    SUB = mybir.AluOpType.subtract

    nc = bacc.Bacc("TRN2", target_bir_lowering=False, debug=False)
    # x transposed, chunk-bit-reversed, bf16 on host; laid out
    # [p, bblk, a, bwithin] so each block is an 8KB contiguous run/partition.
    xtr_d = nc.declare_dram_parameter("xtr", [128, NB, 32, 128], bf16, isOutput=False)
    sb_d = nc.declare_dram_parameter("sb", [128, SBW], bf16, isOutput=False)
    out_d = nc.declare_dram_parameter("out", [B_SHARD, N], bf16, isOutput=True)

    with tile.TileContext(nc) as tc:
        with (
            tc.tile_pool(name="const", bufs=1) as constp,
            tc.tile_pool(name="xbig", bufs=3) as xbigp,
            tc.tile_pool(name="fold", bufs=2) as fp,
            tc.tile_pool(name="evac", bufs=3) as ep,
            tc.tile_pool(name="unf", bufs=2) as up,
            tc.tile_pool(name="outp", bufs=2) as op,
            tc.tile_pool(name="psum", bufs=1, space="PSUM") as pp,
        ):
            SB = constp.tile([128, SBW], bf16, name="SB")
            warm = constp.tile([128, 512], bf16, name="warm")
            strip = {k: SB[:, c0 : c0 + w] for k, (c0, w, _) in STRIPS.items()}

            def emit_dma_in(bt):
                xbig = xbigp.tile([128, 32, 128], bf16, tag="xbig", name="xbig")
                nc.sync.dma_start(xbig[:], xtr_d[:, bt, :, :])
                return xbig

            def emit_folds_a(xbig, swap=False):
                """Early folds: xc (Pool), cyc chain (DVE) — these feed the
                first matmuls of the next block. swap=True runs xc on DVE."""
                xc = fp.tile([128, 16, 128], bf16, tag="xc", name="xc")
                xn = fp.tile([128, 16, 128], bf16, tag="xn", name="xn")
                if swap:
                    nc.vector.tensor_tensor(
                        xc[:], xbig[:, 0:32:2, :], xbig[:, 1:32:2, :], ADD
                    )
                    nc.gpsimd.tensor_tensor(
                        xn[:], xbig[:, 0:32:2, :], xbig[:, 1:32:2, :], SUB
                    )
                else:
                    nc.gpsimd.tensor_tensor(
                        xc[:], xbig[:, 0:32:2, :], xbig[:, 1:32:2, :], ADD
                    )
                    nc.vector.tensor_tensor(
                        xn[:], xbig[:, 0:32:2, :], xbig[:, 1:32:2, :], SUB
                    )
                xcc = fp.tile([128, 8, 128], bf16, tag="xcc", name="xcc")
                nc.vector.tensor_tensor(
                    xcc[:], xc[:, 0:16:2, :], xc[:, 1:16:2, :], ADD
                )
                xccc = fp.tile([128, 4, 128], bf16, tag="xccc", name="xccc")
                nc.vector.tensor_tensor(
                    xccc[:], xcc[:, 0:8:2, :], xcc[:, 1:8:2, :], ADD
                )
                xccn = fp.tile([128, 4, 128], bf16, tag="xccn", name="xccn")
                nc.vector.tensor_tensor(
                    xccn[:], xcc[:, 0:8:2, :], xcc[:, 1:8:2, :], SUB
                )
                xc4 = fp.tile([128, 2, 128], bf16, tag="xc4", name="xc4")
                nc.vector.tensor_tensor(
                    xc4[:], xccc[:, 0:4:2, :], xccc[:, 1:4:2, :], ADD
                )
                xn4 = fp.tile([128, 2, 128], bf16, tag="xn4", name="xn4")
                nc.vector.tensor_tensor(
                    xn4[:], xccc[:, 0:4:2, :], xccc[:, 1:4:2, :], SUB
                )
                xcn = fp.tile([128, 8, 128], bf16, tag="xcn", name="xcn")
                nc.vector.tensor_tensor(
                    xcn[:], xc[:, 0:16:2, :], xc[:, 1:16:2, :], SUB
                )
                return dict(xn=xn, xcn=xcn, xc4=xc4, xn4=xn4, xccn=xccn)

            def emit_folds_b(fa):
                """Late folds: trinomial xTp/xTm (needed only by the T
                matmuls, mid-block). sqrt2 scales on ACT."""
                xn = fa["xn"]
                xA = fp.tile([128, 4, 128], bf16, tag="xA", name="xA")
                nc.vector.tensor_tensor(
                    xA[:], xn[:, 0:16:4, :], xn[:, 1:16:4, :], SUB
                )
                xB = fp.tile([128, 4, 128], bf16, tag="xB", name="xB")
                nc.vector.tensor_tensor(
                    xB[:], xn[:, 2:16:4, :], xn[:, 3:16:4, :], ADD
                )
                s10 = fp.tile([128, 4, 128], bf16, tag="s10", name="s10")
                nc.scalar.mul(s10[:], xn[:, 1:16:4, :], R2)
                s11 = fp.tile([128, 4, 128], bf16, tag="s11", name="s11")
                nc.scalar.mul(s11[:], xn[:, 3:16:4, :], R2)
                xTp = fp.tile([128, 8, 128], bf16, tag="xTp", name="xTp")
                nc.vector.tensor_tensor(xTp[:, 0:4, :], xA[:], s11[:], ADD)
                nc.vector.tensor_tensor(xTp[:, 4:8, :], xB[:], s10[:], SUB)
                xTm = fp.tile([128, 8, 128], bf16, tag="xTm", name="xTm")
                nc.vector.tensor_tensor(xTm[:, 0:4, :], xA[:], s11[:], SUB)
                nc.vector.tensor_tensor(xTm[:, 4:8, :], xB[:], s10[:], ADD)
                fa["xTp"] = xTp
                fa["xTm"] = xTm
                return fa

            def emit_folds_first(xbig):
                """Block 0: fold each quarter's cyc chain as its DMA lands;
                xn/xcn/xc4 pieces follow. All on DVE (Pool too slow to gate)."""
                xc = fp.tile([128, 16, 128], bf16, tag="xc", name="xc")
                xn = fp.tile([128, 16, 128], bf16, tag="xn", name="xn")
                xcc = fp.tile([128, 8, 128], bf16, tag="xcc", name="xcc")
                xccc = fp.tile([128, 4, 128], bf16, tag="xccc", name="xccc")
                xccn = fp.tile([128, 4, 128], bf16, tag="xccn", name="xccn")
                xc4 = fp.tile([128, 2, 128], bf16, tag="xc4", name="xc4")
                xn4 = fp.tile([128, 2, 128], bf16, tag="xn4", name="xn4")
                xcn = fp.tile([128, 8, 128], bf16, tag="xcn", name="xcn")
                for q in range(4):
                    s = 8 * q
                    nc.vector.tensor_tensor(
                        xc[:, 4 * q : 4 * q + 4, :],
                        xbig[:, s : s + 8 : 2, :],
                        xbig[:, s + 1 : s + 8 : 2, :], ADD
                    )
                    nc.vector.tensor_tensor(
                        xcc[:, 2 * q : 2 * q + 2, :],
                        xc[:, 4 * q : 4 * q + 4 : 2, :],
                        xc[:, 4 * q + 1 : 4 * q + 4 : 2, :], ADD
                    )
                    nc.vector.tensor_tensor(
                        xccc[:, q : q + 1, :],
                        xcc[:, 2 * q : 2 * q + 1, :],
                        xcc[:, 2 * q + 1 : 2 * q + 2, :], ADD
                    )
                    nc.vector.tensor_tensor(
                        xccn[:, q : q + 1, :],
                        xcc[:, 2 * q : 2 * q + 1, :],
                        xcc[:, 2 * q + 1 : 2 * q + 2, :], SUB
                    )
                    if q == 1 or q == 3:
                        h = q // 2  # xc4/xn4 entry h from xccc pos 2h, 2h+1
                        nc.vector.tensor_tensor(
                            xc4[:, h : h + 1, :],
                            xccc[:, 2 * h : 2 * h + 1, :],
                            xccc[:, 2 * h + 1 : 2 * h + 2, :], ADD
                        )
                        nc.vector.tensor_tensor(
                            xn4[:, h : h + 1, :],
                            xccc[:, 2 * h : 2 * h + 1, :],
                            xccc[:, 2 * h + 1 : 2 * h + 2, :], SUB
                        )
                # xcn/xn (needed only by the later N10/T matmuls) after the
                # cyc chains so they don't delay the q2/q3 fold chains
                for h in range(2):
                    nc.vector.tensor_tensor(
                        xcn[:, 4 * h : 4 * h + 4, :],
                        xc[:, 8 * h : 8 * h + 8 : 2, :],
                        xc[:, 8 * h + 1 : 8 * h + 8 : 2, :], SUB
                    )
                    nc.vector.tensor_tensor(
                        xn[:, 8 * h : 8 * h + 8, :],
                        xbig[:, 16 * h : 16 * h + 16 : 2, :],
                        xbig[:, 16 * h + 1 : 16 * h + 16 : 2, :], SUB
                    )
                fa = dict(xn=xn, xcn=xcn, xc4=xc4, xn4=xn4, xccn=xccn)
                return emit_folds_b(fa)

            def mms_c25n25(f):
                pCN25 = pp.tile([128, 512], f32, tag="pcn25", name="pcn25")
                for a in range(2):
                    v0 = 128 * (1 - a)
                    nc.tensor.matmul(
                        pCN25[:, 0:256], f["xc4"][:, a, :],
                        strip["sC25"][:, v0 : v0 + 256],
                        start=(a == 0), stop=(a == 1),
                    )
                for a in range(2):
                    v0 = 128 * (1 - a)
                    nc.tensor.matmul(
                        pCN25[:, 256:512], f["xn4"][:, a, :],
                        strip["sN25"][:, v0 : v0 + 256],
                        start=(a == 0), stop=(a == 1),
                    )
                return pCN25

            def mms_n5(f):
                pN5 = pp.tile([128, 512], f32, tag="pn5", name="pn5")
                for i, a in enumerate((0, 2, 1, 3)):  # feed order: quarter BR2[a]
                    v0 = 128 * (3 - a)
                    nc.tensor.matmul(
                        pN5[:], f["xccn"][:, BR2[a], :],
                        strip["sN5"][:, v0 : v0 + 512],
                        start=(i == 0), stop=(i == 3),
                    )
                return pN5

            def evac_cn(pCN25, pN5):
                rCN = ep.tile([128, 1024], bf16, tag="rcn", name="rcn")
                nc.scalar.mul(rCN[:, 0:512], pCN25[:], 0.125)
                nc.scalar.mul(rCN[:, 512:1024], pN5[:], 0.125)
                return rCN

            def mms_cn(f):
                """cyc256+nega256 (shared psum bank) and nega512 leaves."""
                pCN25 = mms_c25n25(f)
                pN5 = mms_n5(f)
                return evac_cn(pCN25, pN5)

            def mms_n10(f):
                """nega1024 in two half-PSUM tiles (tail streams per half)."""
                pN10a = pp.tile([128, 512], f32, tag="pn10a", name="pn10a")
                pN10b = pp.tile([128, 512], f32, tag="pn10b", name="pn10b")
                sN10 = strip["sN10"]
                rN10 = ep.tile([128, 1024], bf16, tag="rn10", name="rn10")
                for j, ps in ((0, pN10a), (1, pN10b)):
                    for i, a in enumerate((0, 4, 2, 6, 1, 5, 3, 7)):
                        v0 = 896 - 128 * a + 512 * j
                        nc.tensor.matmul(
                            ps[:], f["xcn"][:, BR3[a], :], sN10[:, v0 : v0 + 512],
                            start=(i == 0), stop=(i == 7),
                        )
                    nc.scalar.mul(rN10[:, 512 * j : 512 * j + 512], ps[:], 0.25)
                return rN10

            def mm_t_ring(psum, xres, sname):
                s = strip[sname]
                for a in range(8):
                    v0 = 896 - 128 * a
                    pos = BR2[a] if a < 4 else 4 + BR2[a - 4]
                    nc.tensor.matmul(
                        psum[:], xres[:, pos, :], s[:, v0 : v0 + 512],
                        start=(a == 0), stop=(a == 7),
                    )

            def mms_t(f, lo_first=False):
                """lo_first runs both low strips before the high strips so
                the rn chain's o3/t0 diffs can start ~1.7us earlier (tail)."""
                rTp = ep.tile([128, 1024], bf16, tag="rtp", name="rtp")
                rTm = ep.tile([128, 1024], bf16, tag="rtm", name="rtm")
                work = [
                    (f["xTp"], rTp, 0, "sLp"), (f["xTp"], rTp, 1, "sHp"),
                    (f["xTm"], rTm, 0, "sLm"), (f["xTm"], rTm, 1, "sHm"),
                ]
                if lo_first:
                    work = [work[0], work[2], work[1], work[3]]
                for xres, rr, j, sname in work:
                    ph = pp.tile(
                        [128, 512], f32, tag=f"pt{sname}", name=f"pt{sname}"
                    )
                    mm_t_ring(ph, xres, sname)
                    nc.scalar.mul(
                        rr[:, 512 * j : 512 * j + 512], ph[:], SC_T
                    )
                return rTp, rTm

            def emit_rn_chain(rTp, rTm):
                """nega2048 reconstruction from T1024+/- (DVE + ACT scale)."""
                rn = up.tile([128, 2048], bf16, tag="rn", name="rn")
                tt01 = up.tile([128, 1024], bf16, tag="tt01", name="tt01")
                st01 = up.tile([128, 1024], bf16, tag="st01", name="st01")
                nc.vector.tensor_tensor(  # o3
                    rn[:, 1536:2048], rTp[:, 0:512], rTm[:, 0:512], SUB
                )
                nc.vector.tensor_tensor(  # t0
                    tt01[:, 0:512], rTp[:, 0:512], rTm[:, 0:512], ADD
                )
                nc.vector.tensor_tensor(  # o2
                    rn[:, 1024:1536], rTm[:, 512:1024], rTp[:, 512:1024], SUB
                )
                nc.vector.tensor_tensor(  # t1
                    tt01[:, 512:1024], rTp[:, 512:1024], rTm[:, 512:1024], ADD
                )
                nc.scalar.mul(st01[:], tt01[:], R2)
                nc.vector.tensor_tensor(  # o0
                    rn[:, 0:512], st01[:, 0:512], rn[:, 1024:1536], ADD
                )
                nc.vector.tensor_tensor(  # o1
                    rn[:, 512:1024], st01[:, 512:1024], rn[:, 1536:2048], SUB
                )
                return rn

            def emit_rc_part(rCN, rN10):
                rc0 = up.tile([128, 512], bf16, tag="rc0", name="rc0")
                nc.vector.tensor_tensor(
                    rc0[:, 0:256], rCN[:, 0:256], rCN[:, 256:512], ADD
                )
                nc.vector.tensor_tensor(
                    rc0[:, 256:512], rCN[:, 0:256], rCN[:, 256:512], SUB
                )
                rc1 = up.tile([128, 1024], bf16, tag="rc1", name="rc1")
                nc.vector.tensor_tensor(
                    rc1[:, 0:512], rc0[:], rCN[:, 512:1024], ADD
                )
                nc.vector.tensor_tensor(
                    rc1[:, 512:1024], rc0[:], rCN[:, 512:1024], SUB
                )
                rc2 = up.tile([128, 2048], bf16, tag="rc2", name="rc2")
                nc.vector.tensor_tensor(rc2[:, 0:1024], rc1[:], rN10[:], ADD)
                nc.vector.tensor_tensor(rc2[:, 1024:2048], rc1[:], rN10[:], SUB)
                return rc2

            def emit_finals(bt, rc2, rn, pool=True):
                b0 = 128 * bt
                os01 = op.tile([128, 2048], bf16, tag="os01", name="os01")
                os23 = op.tile([128, 2048], bf16, tag="os23", name="os23")
                nc.vector.tensor_tensor(os01[:], rc2[:], rn[:], ADD)
                if pool:
                    nc.gpsimd.tensor_tensor(os23[:], rc2[:], rn[:], SUB)
                else:
                    nc.vector.tensor_tensor(os23[:], rc2[:], rn[:], SUB)
                nc.sync.dma_start(out_d[b0 : b0 + 128, 0:2048], os01[:])
                nc.sync.dma_start(out_d[b0 : b0 + 128, 2048:4096], os23[:])

            def emit_unfold(bt, rCN, rN10, rTp, rTm):
                rn = emit_rn_chain(rTp, rTm)
                rc2 = emit_rc_part(rCN, rN10)
                emit_finals(bt, rc2, rn)

            def emit_tail_rc1(rCN):
                rc0 = up.tile([128, 512], bf16, tag="rc0", name="rc0")
                nc.vector.tensor_tensor(
                    rc0[:, 0:256], rCN[:, 0:256], rCN[:, 256:512], ADD
                )
                nc.vector.tensor_tensor(
                    rc0[:, 256:512], rCN[:, 0:256], rCN[:, 256:512], SUB
                )
                rc1 = up.tile([128, 1024], bf16, tag="rc1", name="rc1")
                nc.vector.tensor_tensor(
                    rc1[:, 0:512], rc0[:], rCN[:, 512:1024], ADD
                )
                nc.vector.tensor_tensor(
                    rc1[:, 512:1024], rc0[:], rCN[:, 512:1024], SUB
                )
                return rc1

            def emit_tail_rn_p(rTp, rTm, rc1):
                """Tail rn chain with P1/P3 = rc1 +- rn-hi interleaved right
                after the o2/o3 diffs (before the st01-gated o0/o1) so they
                are hidden under the N10 matmuls."""
                rn = up.tile([128, 2048], bf16, tag="rn", name="rn")
                tt01 = up.tile([128, 1024], bf16, tag="tt01", name="tt01")
                st01 = up.tile([128, 1024], bf16, tag="st01", name="st01")
                P = up.tile([128, 2, 1024], bf16, tag="P", name="P")
                nc.vector.tensor_tensor(  # o3
                    rn[:, 1536:2048], rTp[:, 0:512], rTm[:, 0:512], SUB
                )
                nc.vector.tensor_tensor(  # t0
                    tt01[:, 0:512], rTp[:, 0:512], rTm[:, 0:512], ADD
                )
                # P-high halves depend only on o3 — emit early
                nc.vector.tensor_tensor(
                    P[:, 0, 512:1024], rc1[:, 512:1024], rn[:, 1536:2048], ADD
                )
                nc.vector.tensor_tensor(
                    P[:, 1, 512:1024], rc1[:, 512:1024], rn[:, 1536:2048], SUB
                )
                nc.vector.tensor_tensor(  # o2
                    rn[:, 1024:1536], rTm[:, 512:1024], rTp[:, 512:1024], SUB
                )
                nc.vector.tensor_tensor(  # t1
                    tt01[:, 512:1024], rTp[:, 512:1024], rTm[:, 512:1024], ADD
                )
                nc.vector.tensor_tensor(
                    P[:, 0, 0:512], rc1[:, 0:512], rn[:, 1024:1536], ADD
                )
                nc.vector.tensor_tensor(
                    P[:, 1, 0:512], rc1[:, 0:512], rn[:, 1024:1536], SUB
                )
                nc.vector.tensor_scalar_mul(st01[:], tt01[:], R2)  # DVE 4x mode
                nc.vector.tensor_tensor(  # o0
                    rn[:, 0:512], st01[:, 0:512], rn[:, 1024:1536], ADD
                )
                nc.vector.tensor_tensor(  # o1
                    rn[:, 512:1024], st01[:, 512:1024], rn[:, 1536:2048], SUB
                )
                return rn, P

            def emit_tail_n10(f):
                """nega1024 as four 256-wide quarter-rings: the psum quarters
                stop staggered so the finish streams out per quarter."""
                pN10a = pp.tile([128, 512], f32, tag="pn10a", name="pn10a")
                pN10b = pp.tile([128, 512], f32, tag="pn10b", name="pn10b")
                sN10 = strip["sN10"]
                for j, ps in ((0, pN10a), (1, pN10b)):
                    for i, a in enumerate((0, 4, 2, 6, 1, 5, 3, 7)):
                        v0 = 896 - 128 * a + 512 * j
                        nc.tensor.matmul(
                            ps[:], f["xcn"][:, BR3[a], :], sN10[:, v0 : v0 + 512],
                            start=(i == 0), stop=(i == 7),
                        )
                return pN10a, pN10b

            def emit_tail_finish(pN10a, pN10b, rc1, rn, P, bt, tg):
                """Streamed ending: P holds rc1 +- rn-hi; after each nega1024
                half-psum stops, only evac -> 5 small combines -> 2 strided
                DMAs remain."""
                b0 = 128 * bt
                rN10 = ep.tile([128, 1024], bf16, tag="rn10" + tg, name="rn10")
                preA = up.tile([128, 1024], bf16, tag="preA", name="preA")
                # os_ slot order per half: (s1, s3, s0, s2)
                os_ = op.tile([128, 2, 4, 512], bf16, tag="ost" + tg, name="ost")
                outv = out_d[:].rearrange("b (s c) -> b s c", c=512)[b0 : b0 + 128]
                for h, ps in ((0, pN10a), (1, pN10b)):
                    c0 = 512 * h
                    rh = rN10[:, c0 : c0 + 512]
                    nc.scalar.mul(rh, ps[:], 0.25)
                    nc.vector.tensor_tensor(  # s1 = P1 - rN10
                        os_[:, h, 0, :], P[:, 0, c0 : c0 + 512], rh, SUB
                    )
                    nc.vector.tensor_tensor(  # s3 = P3 - rN10
                        os_[:, h, 1, :], P[:, 1, c0 : c0 + 512], rh, SUB
                    )
                    nc.sync.dma_start(
                        outv[:, 2 + h : 7 + h : 4, :], os_[:, h, 0:2, :]
                    )
                    pa = preA[:, c0 : c0 + 512]
                    nc.vector.tensor_tensor(pa, rc1[:, c0 : c0 + 512], rh, ADD)
                    nc.vector.tensor_tensor(  # s0 = preA + rn-lo
                        os_[:, h, 2, :], pa, rn[:, c0 : c0 + 512], ADD
                    )
                    nc.vector.tensor_tensor(  # s2 = preA - rn-lo
                        os_[:, h, 3, :], pa, rn[:, c0 : c0 + 512], SUB
                    )
                    nc.sync.dma_start(
                        outv[:, h : 5 + h : 4, :], os_[:, h, 2:4, :]
                    )

            # ---------------- preamble ----------------
            nc.gpsimd.memset(warm[:], 0.0)
            xbig = xbigp.tile([128, 32, 128], bf16, tag="xbig", name="xbig")
            # DMA order: x quarters interleaved with strip pieces so the fold
            # chain, C/N strips and N10/T strips all land just in time.
            nc.sync.dma_start(xbig[:, 0:8, :], xtr_d[:, 0, 0:8, :])
            nc.sync.dma_start(xbig[:, 8:16, :], xtr_d[:, 0, 8:16, :])
            nc.sync.dma_start(SB[:, 0:768], sb_d[:, 0:768])  # sC25+sN25
            nc.sync.dma_start(SB[:, SB_CN:SB_N10], sb_d[:, SB_CN:SB_N10])  # sN10
            nc.sync.dma_start(xbig[:, 16:24, :], xtr_d[:, 0, 16:24, :])
            nc.sync.dma_start(SB[:, 768:SB_CN], sb_d[:, 768:SB_CN])  # sN5
            nc.sync.dma_start(xbig[:, 24:32, :], xtr_d[:, 0, 24:32, :])
            for s0 in range(SB_N10, SBW, 1408):
                nc.sync.dma_start(SB[:, s0 : s0 + 1408], sb_d[:, s0 : s0 + 1408])
            # PE clock warmup: HAM releases 2.4 GHz after ~3us of activity;
            # burn dummies while the first DMAs/folds land.
            pW = pp.tile([128, 512], f32, tag="pcn25", name="pwarm")
            for _ in range(N_WARM):
                nc.tensor.matmul(
                    pW[:], warm[:, 0:128], warm[:], start=True, stop=True
                )

            # ---------------- main pipeline ----------------
            # Iteration bt interleaves emissions so each engine's in-order
            # stream matches when its work becomes runnable:
            #   mms+evacs(bt) | rn-chain(bt-1) | early folds(bt+1) |
            #   rc-part(bt-1) | T-folds(bt+1) | finals(bt-1)
            f_cur = emit_folds_first(xbig)
            xbig_next = emit_dma_in(1)
            r_prev = None
            for bt in range(NB - 2):
                xbig = xbig_next
                if bt + 2 < NB:
                    xbig_next = emit_dma_in(bt + 2)
                if bt == 0:
                    # block 0: C25/N25 (earliest strip), then N10 (its first
                    # ring chunks need only quarters q0/q1), N5 last — its
                    # strip and the q2/q3 folds are still in flight
                    pCN25 = mms_c25n25(f_cur)
                    rN10 = mms_n10(f_cur)
                    rCN = evac_cn(pCN25, mms_n5(f_cur))
                else:
                    rCN = mms_cn(f_cur)
                    rN10 = mms_n10(f_cur)
                rTp, rTm = mms_t(f_cur)
                if r_prev is not None:
                    rn = emit_rn_chain(r_prev[2], r_prev[3])
                fa = emit_folds_a(xbig, swap=(bt == 0))
                if r_prev is not None:
                    rc2 = emit_rc_part(r_prev[0], r_prev[1])
                f_cur = emit_folds_b(fa)
                if r_prev is not None:
                    emit_finals(bt - 1, rc2, rn)
                r_prev = (rCN, rN10, rTp, rTm)
            # last two blocks: block 7's T matmuls hoisted before block 6's
            # so the tail's long recon chains overlap remaining PE work
            f6 = f_cur
            rCN6 = mms_cn(f6)
            rN10_6 = mms_n10(f6)
            rn5 = emit_rn_chain(r_prev[2], r_prev[3])
            fa7 = emit_folds_a(xbig_next)
            rc2_5 = emit_rc_part(r_prev[0], r_prev[1])
            f7 = emit_folds_b(fa7)
            emit_finals(NB - 3, rc2_5, rn5)
            rTp6, rTm6 = mms_t(f6)
            rCN7 = mms_cn(f7)
            rn6 = emit_rn_chain(rTp6, rTm6)
            rc2_6 = emit_rc_part(rCN6, rN10_6)
            rTp7, rTm7 = mms_t(f7, lo_first=True)
            rc1_7 = emit_tail_rc1(rCN7)
            emit_finals(NB - 2, rc2_6, rn6)
            rn7, P7 = emit_tail_rn_p(rTp7, rTm7, rc1_7)
            pa7, pb7 = emit_tail_n10(f7)
            emit_tail_finish(pa7, pb7, rc1_7, rn7, P7, NB - 1, "")

    nc.compile()
    return nc


def _get_nc():
    if "nc" not in _STATE:
        _STATE["nc"] = _build()
    return _STATE["nc"]


def _make_strip_table(w):
    """All 8 leaf band strips, packed [128, SBW] bf16.

    Strip tiles are shears: tile[p, v] = S[OFF + v - p]. Sequences (t any int,
    Ecyc = wt cyclic):
      Ep(t)   = Ecyc(t) + Ecyc(t+2048)          cyc2048
      En(t)   = Ecyc(t) - Ecyc(t+2048)          nega2048
      Epp(t)  = Ep(t) + Ep(t+1024)              cyc1024
      En10(t) = Ep(t) - Ep(t+1024)              nega1024 leaf
      C5(t)   = Epp(t) + Epp(t+512)             cyc512
      N5(t)   = Epp(t) - Epp(t+512)             nega512 leaf
      C25(t)  = C5(t) + C5(t+256)               cyc256 leaf (x0.5 baked)
      N25(t)  = C5(t) - C5(t+256)               nega256 leaf (x0.5 baked)
      D(t) = En(t) - En(t+1024); Bt(t) = En(t+512) + En(t+1536)
      L+/-(t) = D(t) +- sqrt2 En(t+1536)        T1024 low strips
      H+/-(t) = Bt(t) -+ sqrt2 En(t+1024)       T1024 high strips
    CRT scales (1/8 with the extra 1/2 baked for C25/N25, 1/8, 1/4,
    1/(4sqrt2)) are applied in the ACT PSUM->SBUF evacuations.
    """
    import ml_dtypes

    wt = np.roll(w[::-1], 1).astype(np.float64)
    Ecyc = lambda t: wt[np.mod(t, N)]
    Ep = lambda t: Ecyc(t) + Ecyc(t + 2048)
    En = lambda t: Ecyc(t) - Ecyc(t + 2048)
    Epp = lambda t: Ep(t) + Ep(t + 1024)
    C5 = lambda t: Epp(t) + Epp(t + 512)
    seqs = {
        "sC25": lambda t: 0.5 * (C5(t) + C5(t + 256)),
        "sN25": lambda t: 0.5 * (C5(t) - C5(t + 256)),
        "sN5": lambda t: Epp(t) - Epp(t + 512),
        "sN10": lambda t: Ep(t) - Ep(t + 1024),
        "sLp": lambda t: (En(t) - En(t + 1024)) + R2 * En(t + 1536),
        "sLm": lambda t: (En(t) - En(t + 1024)) - R2 * En(t + 1536),
        "sHp": lambda t: (En(t + 512) + En(t + 1536)) - R2 * En(t + 1024),
        "sHm": lambda t: (En(t + 512) + En(t + 1536)) + R2 * En(t + 1024),
    }
    p = np.arange(128)[:, None]
    tab = np.zeros((128, SBW), dtype=np.float64)
    for name, (c0, width, off) in STRIPS.items():
        v = np.arange(width)[None, :]
        tab[:, c0 : c0 + width] = seqs[name](off + v - p)
    return np.ascontiguousarray(tab.astype(ml_dtypes.bfloat16))


def _prep_inputs(x, w):
    import ml_dtypes

    x = np.ascontiguousarray(x, dtype=np.float32)
    w = np.ascontiguousarray(w, dtype=np.float32)
    sb = _make_strip_table(w)
    in_maps = []
    for i in range(N_CORES):
        xtr = x[i * B_SHARD : (i + 1) * B_SHARD].T  # [N, B_SHARD]
        xtr = xtr.reshape(32, 128, B_SHARD)[BITREV5]  # [a(pos), p, b]
        # [p, bblk, a, bwithin]: per-(p, bblk) an 8KB contiguous bf16 run
        xtr = np.ascontiguousarray(
            xtr.reshape(32, 128, NB, 128).transpose(1, 2, 0, 3).astype(
                ml_dtypes.bfloat16
            )
        )
        in_maps.append({"xtr": xtr, "sb": sb})
    return in_maps


def kernel(x, w, _trace=False):
    from concourse.bass_utils import run_bass_kernel_spmd

    nc = _get_nc()
    in_maps = _prep_inputs(x, w)
    res = run_bass_kernel_spmd(nc, in_maps, list(range(N_CORES)), trace=_trace)
    out = np.concatenate(
        [np.asarray(res.results[i]["out"]).astype(np.float32) for i in range(N_CORES)],
        axis=0,
    )
    if _trace:
        _STATE["last_result"] = res
    return out


# revision 60
# speedup vs baseline: 1.1322x; 1.0028x over previous
"""Circulant matmul for TRN2: 4.5-level CRT with trinomial split, bf16 matmuls.

out[b, r] = sum_c x[b,c] w[(c-r) mod N] = (x (*) wt)[r], wt = roll(w[::-1],1)
(cyclic convolution along c) — no input flip or output reversal needed.

CRT tree on z^4096-1 (per 128-row block of x):
  cyc4096 -> cyc2048 + nega2048             (binomial, band scale 1/2 each)
  cyc2048 -> cyc1024 + nega1024[LEAF]       (1/2)
  cyc1024 -> cyc512 + nega512[LEAF]         (1/2)
  cyc512  -> cyc256[LEAF] + nega256[LEAF]   (1/2)
  nega2048 -> T1024+ , T1024- [LEAVES]      (z^1024 -+ sqrt2 z^512 + 1, 1/(2sqrt2))
Leaf matmuls: 54 x bf16 = 27648 PE cycles/block (11.5us at 2.4GHz; the
f32 baseline's direct matmul would be 54.6us).  x chunks are stored in
bit-reversed order so EVERY fold level is a stride-2 slice and any quarter
of block 0 folds to completion as its DMA lands (fast pipeline fill).
I/O is bf16 both ways (host converts; rel-err budget allows it); per-block
HBM traffic is 2 MiB so the exclusive 360 B/ns DMA engine stays at ~50%.
Steady finals are two [128,2048] ops (DVE + Pool) feeding two output DMAs.
The last block streams out per nega1024 half-psum: P = rc1 +- rn-hi is
precomputed under the matmuls (split so the o3-dependent halves are
emitted early) and each half needs only evac -> 5 combines -> 2 strided
DMAs after its psum stops.  A single warmup matmul at ~1.3us starts the
PE p-state ramp clock so real matmuls from ~4.3us run at full clock; a
few more bridge the idle so the ramp timer never resets.
Per-block engine busy: PE 11.9 (critical), DVE 11.2, Pool 7.4, ACT 7.2,
DMA 6.6.  Makespan = fill ~6.1 + PE stream ~93 + drain ~7.
"""

import sys

sys.path.insert(0, "/opt/trn_rl_repo")

import numpy as np

N = 4096
B = 8192
N_CORES = 8
B_SHARD = B // N_CORES  # 1024
NB = B_SHARD // 128     # 8 row-tiles per core
R2 = float(np.sqrt(2.0))
SC_T = float(1.0 / (4.0 * np.sqrt(2.0)))   # T1024 leaves: 1/2 * 1/(2sqrt2)

# bit-reversal position maps: leaf-residue chunk a lives at fold-output
# position BRk[a] (k = log2 nchunks)
BR2 = [0, 2, 1, 3]
BR3 = [0, 4, 2, 6, 1, 5, 3, 7]
# x chunk stored at position i is BITREV5[i]
BITREV5 = [
    ((i & 1) << 4) | ((i & 2) << 2) | (i & 4) | ((i & 8) >> 2) | ((i & 16) >> 4)
    for i in range(32)
]

# strip layout in the packed [128, SBW] host table (name: col0, width, OFF)
STRIPS = {
    "sC25": (0, 384, -128),
    "sN25": (384, 384, -128),
    "sN5": (768, 896, -384),
    "sN10": (1664, 1920, -896),
    "sLp": (3584, 1408, -896),
    "sHp": (4992, 1408, -896),
    "sLm": (6400, 1408, -896),
    "sHm": (7808, 1408, -896),
}
SBW = 9216
SB_CN = 1664    # sC25+sN25+sN5
SB_N10 = 3584   # ..sN10

_STATE = {}

N_WARM = 10


def _build():
    import concourse.bacc as bacc
    import concourse.mybir as mybir
    import concourse.tile as tile

    f32 = mybir.dt.float32
    bf16 = mybir.dt.bfloat16
    ADD = mybir.AluOpType.add
    SUB = mybir.AluOpType.subtract

    nc = bacc.Bacc("TRN2", target_bir_lowering=False, debug=False)
    # x transposed, chunk-bit-reversed, bf16 on host; laid out
    # [p, bblk, a, bwithin] so each block is an 8KB contiguous run/partition.
    xtr_d = nc.declare_dram_parameter("xtr", [128, NB, 32, 128], bf16, isOutput=False)
    sb_d = nc.declare_dram_parameter("sb", [128, SBW], bf16, isOutput=False)
    out_d = nc.declare_dram_parameter("out", [B_SHARD, N], bf16, isOutput=True)

    with tile.TileContext(nc) as tc:
        with (
            tc.tile_pool(name="const", bufs=1) as constp,
            tc.tile_pool(name="xbig", bufs=3) as xbigp,
            tc.tile_pool(name="fold", bufs=2) as fp,
            tc.tile_pool(name="evac", bufs=3) as ep,
            tc.tile_pool(name="unf", bufs=2) as up,
            tc.tile_pool(name="outp", bufs=2) as op,
            tc.tile_pool(name="psum", bufs=1, space="PSUM") as pp,
        ):
            SB = constp.tile([128, SBW], bf16, name="SB")
            warm = constp.tile([128, 512], bf16, name="warm")
            strip = {k: SB[:, c0 : c0 + w] for k, (c0, w, _) in STRIPS.items()}

            def emit_dma_in(bt):
                xbig = xbigp.tile([128, 32, 128], bf16, tag="xbig", name="xbig")
                nc.sync.dma_start(xbig[:], xtr_d[:, bt, :, :])
                return xbig

            def emit_folds_a(xbig, swap=False):
                """Early folds: xc (Pool), cyc chain (DVE) — these feed the
                first matmuls of the next block. swap=True runs xc on DVE."""
                xc = fp.tile([128, 16, 128], bf16, tag="xc", name="xc")
                xn = fp.tile([128, 16, 128], bf16, tag="xn", name="xn")
                if swap:
                    nc.vector.tensor_tensor(
                        xc[:], xbig[:, 0:32:2, :], xbig[:, 1:32:2, :], ADD
                    )
                    nc.gpsimd.tensor_tensor(
                        xn[:], xbig[:, 0:32:2, :], xbig[:, 1:32:2, :], SUB
                    )
                else:
                    nc.gpsimd.tensor_tensor(
                        xc[:], xbig[:, 0:32:2, :], xbig[:, 1:32:2, :], ADD
                    )
                    nc.vector.tensor_tensor(
                        xn[:], xbig[:, 0:32:2, :], xbig[:, 1:32:2, :], SUB
                    )
                xcc = fp.tile([128, 8, 128], bf16, tag="xcc", name="xcc")
                nc.vector.tensor_tensor(
                    xcc[:], xc[:, 0:16:2, :], xc[:, 1:16:2, :], ADD
                )
                xccc = fp.tile([128, 4, 128], bf16, tag="xccc", name="xccc")
                nc.vector.tensor_tensor(
                    xccc[:], xcc[:, 0:8:2, :], xcc[:, 1:8:2, :], ADD
                )
                xccn = fp.tile([128, 4, 128], bf16, tag="xccn", name="xccn")
                nc.vector.tensor_tensor(
                    xccn[:], xcc[:, 0:8:2, :], xcc[:, 1:8:2, :], SUB
                )
                xc4 = fp.tile([128, 2, 128], bf16, tag="xc4", name="xc4")
                nc.vector.tensor_tensor(
                    xc4[:], xccc[:, 0:4:2, :], xccc[:, 1:4:2, :], ADD
                )
                xn4 = fp.tile([128, 2, 128], bf16, tag="xn4", name="xn4")
                nc.vector.tensor_tensor(
                    xn4[:], xccc[:, 0:4:2, :], xccc[:, 1:4:2, :], SUB
                )
                xcn = fp.tile([128, 8, 128], bf16, tag="xcn", name="xcn")
                nc.vector.tensor_tensor(
                    xcn[:], xc[:, 0:16:2, :], xc[:, 1:16:2, :], SUB
                )
                return dict(xn=xn, xcn=xcn, xc4=xc4, xn4=xn4, xccn=xccn)

            def emit_folds_b(fa, dve_scales=False):
                """Late folds: trinomial xTp/xTm (needed only by the T
                matmuls, mid-block). sqrt2 scales on ACT (DVE 4x-mode for
                block 0, where the ACT hop would delay the first T matmuls)."""
                xn = fa["xn"]
                xA = fp.tile([128, 4, 128], bf16, tag="xA", name="xA")
                nc.vector.tensor_tensor(
                    xA[:], xn[:, 0:16:4, :], xn[:, 1:16:4, :], SUB
                )
                xB = fp.tile([128, 4, 128], bf16, tag="xB", name="xB")
                nc.vector.tensor_tensor(
                    xB[:], xn[:, 2:16:4, :], xn[:, 3:16:4, :], ADD
                )
                s10 = fp.tile([128, 4, 128], bf16, tag="s10", name="s10")
                s11 = fp.tile([128, 4, 128], bf16, tag="s11", name="s11")
                if dve_scales:
                    nc.vector.tensor_scalar_mul(s10[:], xn[:, 1:16:4, :], R2)
                    nc.vector.tensor_scalar_mul(s11[:], xn[:, 3:16:4, :], R2)
                else:
                    nc.scalar.mul(s10[:], xn[:, 1:16:4, :], R2)
                    nc.scalar.mul(s11[:], xn[:, 3:16:4, :], R2)
                xTp = fp.tile([128, 8, 128], bf16, tag="xTp", name="xTp")
                nc.vector.tensor_tensor(xTp[:, 0:4, :], xA[:], s11[:], ADD)
                nc.vector.tensor_tensor(xTp[:, 4:8, :], xB[:], s10[:], SUB)
                xTm = fp.tile([128, 8, 128], bf16, tag="xTm", name="xTm")
                nc.vector.tensor_tensor(xTm[:, 0:4, :], xA[:], s11[:], SUB)
                nc.vector.tensor_tensor(xTm[:, 4:8, :], xB[:], s10[:], ADD)
                fa["xTp"] = xTp
                fa["xTm"] = xTm
                return fa

            def emit_folds_first(xbig):
                """Block 0: fold each quarter's cyc chain as its DMA lands;
                xn/xcn/xc4 pieces follow. All on DVE (Pool too slow to gate)."""
                xc = fp.tile([128, 16, 128], bf16, tag="xc", name="xc")
                xn = fp.tile([128, 16, 128], bf16, tag="xn", name="xn")
                xcc = fp.tile([128, 8, 128], bf16, tag="xcc", name="xcc")
                xccc = fp.tile([128, 4, 128], bf16, tag="xccc", name="xccc")
                xccn = fp.tile([128, 4, 128], bf16, tag="xccn", name="xccn")
                xc4 = fp.tile([128, 2, 128], bf16, tag="xc4", name="xc4")
                xn4 = fp.tile([128, 2, 128], bf16, tag="xn4", name="xn4")
                xcn = fp.tile([128, 8, 128], bf16, tag="xcn", name="xcn")
                for q in range(4):
                    s = 8 * q
                    # the cyc chain feeds the first matmuls — schedule ahead
                    # of the xn/xcn side ops on DVE
                    with tc.high_priority():
                        nc.vector.tensor_tensor(
                            xc[:, 4 * q : 4 * q + 4, :],
                            xbig[:, s : s + 8 : 2, :],
                            xbig[:, s + 1 : s + 8 : 2, :], ADD
                        )
                        nc.vector.tensor_tensor(
                            xcc[:, 2 * q : 2 * q + 2, :],
                            xc[:, 4 * q : 4 * q + 4 : 2, :],
                            xc[:, 4 * q + 1 : 4 * q + 4 : 2, :], ADD
                        )
                        nc.vector.tensor_tensor(
                            xccc[:, q : q + 1, :],
                            xcc[:, 2 * q : 2 * q + 1, :],
                            xcc[:, 2 * q + 1 : 2 * q + 2, :], ADD
                        )
                        if q == 1 or q == 3:
                            h = q // 2  # xc4/xn4 entry h, xccc pos 2h, 2h+1
                            nc.vector.tensor_tensor(
                                xc4[:, h : h + 1, :],
                                xccc[:, 2 * h : 2 * h + 1, :],
                                xccc[:, 2 * h + 1 : 2 * h + 2, :], ADD
                            )
                            nc.vector.tensor_tensor(
                                xn4[:, h : h + 1, :],
                                xccc[:, 2 * h : 2 * h + 1, :],
                                xccc[:, 2 * h + 1 : 2 * h + 2, :], SUB
                            )
                    nc.vector.tensor_tensor(
                        xccn[:, q : q + 1, :],
                        xcc[:, 2 * q : 2 * q + 1, :],
                        xcc[:, 2 * q + 1 : 2 * q + 2, :], SUB
                    )
                # xcn/xn (needed only by the later N10/T matmuls) after the
                # cyc chains so they don't delay the q2/q3 fold chains
                for h in range(2):
                    nc.vector.tensor_tensor(
                        xcn[:, 4 * h : 4 * h + 4, :],
                        xc[:, 8 * h : 8 * h + 8 : 2, :],
                        xc[:, 8 * h + 1 : 8 * h + 8 : 2, :], SUB
                    )
                    nc.vector.tensor_tensor(
                        xn[:, 8 * h : 8 * h + 8, :],
                        xbig[:, 16 * h : 16 * h + 16 : 2, :],
                        xbig[:, 16 * h + 1 : 16 * h + 16 : 2, :], SUB
                    )
                fa = dict(xn=xn, xcn=xcn, xc4=xc4, xn4=xn4, xccn=xccn)
                return emit_folds_b(fa, dve_scales=True)

            def mms_c25n25(f):
                pCN25 = pp.tile([128, 512], f32, tag="pcn25", name="pcn25")
                for a in range(2):
                    v0 = 128 * (1 - a)
                    nc.tensor.matmul(
                        pCN25[:, 0:256], f["xc4"][:, a, :],
                        strip["sC25"][:, v0 : v0 + 256],
                        start=(a == 0), stop=(a == 1),
                    )
                for a in range(2):
                    v0 = 128 * (1 - a)
                    nc.tensor.matmul(
                        pCN25[:, 256:512], f["xn4"][:, a, :],
                        strip["sN25"][:, v0 : v0 + 256],
                        start=(a == 0), stop=(a == 1),
                    )
                return pCN25

            def mms_n5(f):
                pN5 = pp.tile([128, 512], f32, tag="pn5", name="pn5")
                for i, a in enumerate((0, 2, 1, 3)):  # feed order: quarter BR2[a]
                    v0 = 128 * (3 - a)
                    nc.tensor.matmul(
                        pN5[:], f["xccn"][:, BR2[a], :],
                        strip["sN5"][:, v0 : v0 + 512],
                        start=(i == 0), stop=(i == 3),
                    )
                return pN5

            def evac_cn(pCN25, pN5):
                rCN = ep.tile([128, 1024], bf16, tag="rcn", name="rcn")
                nc.scalar.mul(rCN[:, 0:512], pCN25[:], 0.125)
                nc.scalar.mul(rCN[:, 512:1024], pN5[:], 0.125)
                return rCN

            def mms_cn(f):
                """cyc256+nega256 (shared psum bank) and nega512 leaves."""
                pCN25 = mms_c25n25(f)
                pN5 = mms_n5(f)
                return evac_cn(pCN25, pN5)

            def mms_n10(f):
                """nega1024 in two half-PSUM tiles (tail streams per half)."""
                pN10a = pp.tile([128, 512], f32, tag="pn10a", name="pn10a")
                pN10b = pp.tile([128, 512], f32, tag="pn10b", name="pn10b")
                sN10 = strip["sN10"]
                rN10 = ep.tile([128, 1024], bf16, tag="rn10", name="rn10")
                for j, ps in ((0, pN10a), (1, pN10b)):
                    for i, a in enumerate((0, 4, 2, 6, 1, 5, 3, 7)):
                        v0 = 896 - 128 * a + 512 * j
                        nc.tensor.matmul(
                            ps[:], f["xcn"][:, BR3[a], :], sN10[:, v0 : v0 + 512],
                            start=(i == 0), stop=(i == 7),
                        )
                    nc.scalar.mul(rN10[:, 512 * j : 512 * j + 512], ps[:], 0.25)
                return rN10

            def mm_t_ring(psum, xres, sname):
                s = strip[sname]
                for a in range(8):
                    v0 = 896 - 128 * a
                    pos = BR2[a] if a < 4 else 4 + BR2[a - 4]
                    nc.tensor.matmul(
                        psum[:], xres[:, pos, :], s[:, v0 : v0 + 512],
                        start=(a == 0), stop=(a == 7),
                    )

            def mms_t(f, lo_first=False):
                """lo_first runs both low strips before the high strips so
                the rn chain's o3/t0 diffs can start ~1.7us earlier (tail)."""
                rTp = ep.tile([128, 1024], bf16, tag="rtp", name="rtp")
                rTm = ep.tile([128, 1024], bf16, tag="rtm", name="rtm")
                work = [
                    (f["xTp"], rTp, 0, "sLp"), (f["xTp"], rTp, 1, "sHp"),
                    (f["xTm"], rTm, 0, "sLm"), (f["xTm"], rTm, 1, "sHm"),
                ]
                if lo_first:
                    work = [work[0], work[2], work[1], work[3]]
                for xres, rr, j, sname in work:
                    ph = pp.tile(
                        [128, 512], f32, tag=f"pt{sname}", name=f"pt{sname}"
                    )
                    mm_t_ring(ph, xres, sname)
                    nc.scalar.mul(
                        rr[:, 512 * j : 512 * j + 512], ph[:], SC_T
                    )
                return rTp, rTm

            def emit_rn_chain(rTp, rTm):
                """nega2048 reconstruction from T1024+/- (DVE + ACT scale)."""
                rn = up.tile([128, 2048], bf16, tag="rn", name="rn")
                tt01 = up.tile([128, 1024], bf16, tag="tt01", name="tt01")
                st01 = up.tile([128, 1024], bf16, tag="st01", name="st01")
                nc.vector.tensor_tensor(  # o3
                    rn[:, 1536:2048], rTp[:, 0:512], rTm[:, 0:512], SUB
                )
                nc.vector.tensor_tensor(  # t0
                    tt01[:, 0:512], rTp[:, 0:512], rTm[:, 0:512], ADD
                )
                nc.vector.tensor_tensor(  # o2
                    rn[:, 1024:1536], rTm[:, 512:1024], rTp[:, 512:1024], SUB
                )
                nc.vector.tensor_tensor(  # t1
                    tt01[:, 512:1024], rTp[:, 512:1024], rTm[:, 512:1024], ADD
                )
                nc.scalar.mul(st01[:], tt01[:], R2)
                nc.vector.tensor_tensor(  # o0
                    rn[:, 0:512], st01[:, 0:512], rn[:, 1024:1536], ADD
                )
                nc.vector.tensor_tensor(  # o1
                    rn[:, 512:1024], st01[:, 512:1024], rn[:, 1536:2048], SUB
                )
                return rn

            def emit_rc_part(rCN, rN10):
                rc0 = up.tile([128, 512], bf16, tag="rc0", name="rc0")
                nc.vector.tensor_tensor(
                    rc0[:, 0:256], rCN[:, 0:256], rCN[:, 256:512], ADD
                )
                nc.vector.tensor_tensor(
                    rc0[:, 256:512], rCN[:, 0:256], rCN[:, 256:512], SUB
                )
                rc1 = up.tile([128, 1024], bf16, tag="rc1", name="rc1")
                nc.vector.tensor_tensor(
                    rc1[:, 0:512], rc0[:], rCN[:, 512:1024], ADD
                )
                nc.vector.tensor_tensor(
                    rc1[:, 512:1024], rc0[:], rCN[:, 512:1024], SUB
                )
                rc2 = up.tile([128, 2048], bf16, tag="rc2", name="rc2")
                nc.vector.tensor_tensor(rc2[:, 0:1024], rc1[:], rN10[:], ADD)
                nc.vector.tensor_tensor(rc2[:, 1024:2048], rc1[:], rN10[:], SUB)
                return rc2

            def emit_finals(bt, rc2, rn, pool=True):
                b0 = 128 * bt
                os01 = op.tile([128, 2048], bf16, tag="os01", name="os01")
                os23 = op.tile([128, 2048], bf16, tag="os23", name="os23")
                nc.vector.tensor_tensor(os01[:], rc2[:], rn[:], ADD)
                if pool:
                    nc.gpsimd.tensor_tensor(os23[:], rc2[:], rn[:], SUB)
                else:
                    nc.vector.tensor_tensor(os23[:], rc2[:], rn[:], SUB)
                nc.sync.dma_start(out_d[b0 : b0 + 128, 0:2048], os01[:])
                nc.sync.dma_start(out_d[b0 : b0 + 128, 2048:4096], os23[:])

            def emit_unfold(bt, rCN, rN10, rTp, rTm):
                rn = emit_rn_chain(rTp, rTm)
                rc2 = emit_rc_part(rCN, rN10)
                emit_finals(bt, rc2, rn)

            def emit_tail_rc1(rCN):
                rc0 = up.tile([128, 512], bf16, tag="rc0", name="rc0")
                nc.vector.tensor_tensor(
                    rc0[:, 0:256], rCN[:, 0:256], rCN[:, 256:512], ADD
                )
                nc.vector.tensor_tensor(
                    rc0[:, 256:512], rCN[:, 0:256], rCN[:, 256:512], SUB
                )
                rc1 = up.tile([128, 1024], bf16, tag="rc1", name="rc1")
                nc.vector.tensor_tensor(
                    rc1[:, 0:512], rc0[:], rCN[:, 512:1024], ADD
                )
                nc.vector.tensor_tensor(
                    rc1[:, 512:1024], rc0[:], rCN[:, 512:1024], SUB
                )
                return rc1

            def emit_tail_rn_p(rTp, rTm, rc1):
                """Tail rn chain with P1/P3 = rc1 +- rn-hi interleaved right
                after the o2/o3 diffs (before the st01-gated o0/o1) so they
                are hidden under the N10 matmuls."""
                rn = up.tile([128, 2048], bf16, tag="rn", name="rn")
                tt01 = up.tile([128, 1024], bf16, tag="tt01", name="tt01")
                st01 = up.tile([128, 1024], bf16, tag="st01", name="st01")
                P = up.tile([128, 2, 1024], bf16, tag="P", name="P")
                nc.vector.tensor_tensor(  # o3
                    rn[:, 1536:2048], rTp[:, 0:512], rTm[:, 0:512], SUB
                )
                nc.vector.tensor_tensor(  # t0
                    tt01[:, 0:512], rTp[:, 0:512], rTm[:, 0:512], ADD
                )
                # P-high halves depend only on o3 — emit early
                nc.vector.tensor_tensor(
                    P[:, 0, 512:1024], rc1[:, 512:1024], rn[:, 1536:2048], ADD
                )
                nc.vector.tensor_tensor(
                    P[:, 1, 512:1024], rc1[:, 512:1024], rn[:, 1536:2048], SUB
                )
                nc.vector.tensor_tensor(  # o2
                    rn[:, 1024:1536], rTm[:, 512:1024], rTp[:, 512:1024], SUB
                )
                nc.vector.tensor_tensor(  # t1
                    tt01[:, 512:1024], rTp[:, 512:1024], rTm[:, 512:1024], ADD
                )
                nc.vector.tensor_tensor(
                    P[:, 0, 0:512], rc1[:, 0:512], rn[:, 1024:1536], ADD
                )
                nc.vector.tensor_tensor(
                    P[:, 1, 0:512], rc1[:, 0:512], rn[:, 1024:1536], SUB
                )
                nc.vector.tensor_scalar_mul(st01[:], tt01[:], R2)  # DVE 4x mode
                nc.vector.tensor_tensor(  # o0
                    rn[:, 0:512], st01[:, 0:512], rn[:, 1024:1536], ADD
                )
                nc.vector.tensor_tensor(  # o1
                    rn[:, 512:1024], st01[:, 512:1024], rn[:, 1536:2048], SUB
                )
                return rn, P

            def emit_tail_n10(f):
                """nega1024 as four 256-wide quarter-rings: the psum quarters
                stop staggered so the finish streams out per quarter."""
                pN10a = pp.tile([128, 512], f32, tag="pn10a", name="pn10a")
                pN10b = pp.tile([128, 512], f32, tag="pn10b", name="pn10b")
                sN10 = strip["sN10"]
                for j, ps in ((0, pN10a), (1, pN10b)):
                    for i, a in enumerate((0, 4, 2, 6, 1, 5, 3, 7)):
                        v0 = 896 - 128 * a + 512 * j
                        nc.tensor.matmul(
                            ps[:], f["xcn"][:, BR3[a], :], sN10[:, v0 : v0 + 512],
                            start=(i == 0), stop=(i == 7),
                        )
                return pN10a, pN10b

            def emit_tail_finish(pN10a, pN10b, rc1, rn, P, bt, tg):
                """Streamed ending: P holds rc1 +- rn-hi; after each nega1024
                half-psum stops, only evac -> 5 small combines -> 2 strided
                DMAs remain."""
                b0 = 128 * bt
                rN10 = ep.tile([128, 1024], bf16, tag="rn10" + tg, name="rn10")
                preA = up.tile([128, 1024], bf16, tag="preA", name="preA")
                # os_ slot order per half: (s1, s3, s0, s2)
                os_ = op.tile([128, 2, 4, 512], bf16, tag="ost" + tg, name="ost")
                outv = out_d[:].rearrange("b (s c) -> b s c", c=512)[b0 : b0 + 128]
                for h, ps in ((0, pN10a), (1, pN10b)):
                    c0 = 512 * h
                    rh = rN10[:, c0 : c0 + 512]
                    nc.scalar.mul(rh, ps[:], 0.25)
                    nc.vector.tensor_tensor(  # s1 = P1 - rN10
                        os_[:, h, 0, :], P[:, 0, c0 : c0 + 512], rh, SUB
                    )
                    nc.vector.tensor_tensor(  # s3 = P3 - rN10
                        os_[:, h, 1, :], P[:, 1, c0 : c0 + 512], rh, SUB
                    )
                    nc.sync.dma_start(
                        outv[:, 2 + h : 7 + h : 4, :], os_[:, h, 0:2, :]
                    )
                    pa = preA[:, c0 : c0 + 512]
                    nc.vector.tensor_tensor(pa, rc1[:, c0 : c0 + 512], rh, ADD)
                    nc.vector.tensor_tensor(  # s0 = preA + rn-lo
                        os_[:, h, 2, :], pa, rn[:, c0 : c0 + 512], ADD
                    )
                    nc.vector.tensor_tensor(  # s2 = preA - rn-lo
                        os_[:, h, 3, :], pa, rn[:, c0 : c0 + 512], SUB
                    )
                    nc.sync.dma_start(
                        outv[:, h : 5 + h : 4, :], os_[:, h, 2:4, :]
                    )

            # ---------------- preamble ----------------
            nc.gpsimd.memset(warm[:], 0.0)
            xbig = xbigp.tile([128, 32, 128], bf16, tag="xbig", name="xbig")
            # DMA order: x quarters interleaved with strip pieces so the fold
            # chain, C/N strips and N10/T strips all land just in time.
            nc.sync.dma_start(xbig[:, 0:8, :], xtr_d[:, 0, 0:8, :])
            nc.sync.dma_start(xbig[:, 8:16, :], xtr_d[:, 0, 8:16, :])
            nc.sync.dma_start(SB[:, 0:768], sb_d[:, 0:768])  # sC25+sN25
            nc.sync.dma_start(SB[:, SB_CN:SB_N10], sb_d[:, SB_CN:SB_N10])  # sN10
            nc.sync.dma_start(xbig[:, 16:24, :], xtr_d[:, 0, 16:24, :])
            nc.sync.dma_start(SB[:, 768:SB_CN], sb_d[:, 768:SB_CN])  # sN5
            nc.sync.dma_start(xbig[:, 24:32, :], xtr_d[:, 0, 24:32, :])
            for s0 in range(SB_N10, SBW, 1408):
                nc.sync.dma_start(
                    SB[:, s0 + 512 : s0 + 1408], sb_d[:, s0 + 512 : s0 + 1408]
                )
                nc.sync.dma_start(SB[:, s0 : s0 + 512], sb_d[:, s0 : s0 + 512])
            # PE clock warmup: HAM releases 2.4 GHz after ~3us of activity;
            # burn dummies while the first DMAs/folds land.
            pW = pp.tile([128, 512], f32, tag="pcn25", name="pwarm")
            for _ in range(N_WARM):
                nc.tensor.matmul(
                    pW[:], warm[:, 0:128], warm[:], start=True, stop=True
                )

            # ---------------- main pipeline ----------------
            # Iteration bt interleaves emissions so each engine's in-order
            # stream matches when its work becomes runnable:
            #   mms+evacs(bt) | rn-chain(bt-1) | early folds(bt+1) |
            #   rc-part(bt-1) | T-folds(bt+1) | finals(bt-1)
            f_cur = emit_folds_first(xbig)
            xbig_next = emit_dma_in(1)
            r_prev = None
            for bt in range(NB - 2):
                xbig = xbig_next
                if bt + 2 < NB:
                    xbig_next = emit_dma_in(bt + 2)
                if bt == 0:
                    # block 0: C25/N25 (earliest strip), then N10 (its first
                    # ring chunks need only quarters q0/q1), N5 last — its
                    # strip and the q2/q3 folds are still in flight
                    pCN25 = mms_c25n25(f_cur)
                    rN10 = mms_n10(f_cur)
                    rCN = evac_cn(pCN25, mms_n5(f_cur))
                else:
                    rCN = mms_cn(f_cur)
                    rN10 = mms_n10(f_cur)
                rTp, rTm = mms_t(f_cur)
                if r_prev is not None:
                    rn = emit_rn_chain(r_prev[2], r_prev[3])
                fa = emit_folds_a(xbig, swap=(bt == 0))
                if r_prev is not None:
                    rc2 = emit_rc_part(r_prev[0], r_prev[1])
                f_cur = emit_folds_b(fa)
                if r_prev is not None:
                    emit_finals(bt - 1, rc2, rn)
                r_prev = (rCN, rN10, rTp, rTm)
            # last two blocks: block 7's T matmuls hoisted before block 6's
            # so the tail's long recon chains overlap remaining PE work
            f6 = f_cur
            rCN6 = mms_cn(f6)
            rN10_6 = mms_n10(f6)
            rn5 = emit_rn_chain(r_prev[2], r_prev[3])
            fa7 = emit_folds_a(xbig_next)
            rc2_5 = emit_rc_part(r_prev[0], r_prev[1])
            f7 = emit_folds_b(fa7)
            emit_finals(NB - 3, rc2_5, rn5)
            rTp6, rTm6 = mms_t(f6)
            rCN7 = mms_cn(f7)
            rn6 = emit_rn_chain(rTp6, rTm6)
            rc2_6 = emit_rc_part(rCN6, rN10_6)
            rTp7, rTm7 = mms_t(f7, lo_first=True)
            rc1_7 = emit_tail_rc1(rCN7)
            emit_finals(NB - 2, rc2_6, rn6)
            rn7, P7 = emit_tail_rn_p(rTp7, rTm7, rc1_7)
            pa7, pb7 = emit_tail_n10(f7)
            emit_tail_finish(pa7, pb7, rc1_7, rn7, P7, NB - 1, "")

    nc.compile()
    return nc


def _get_nc():
    if "nc" not in _STATE:
        _STATE["nc"] = _build()
    return _STATE["nc"]


def _make_strip_table(w):
    """All 8 leaf band strips, packed [128, SBW] bf16.

    Strip tiles are shears: tile[p, v] = S[OFF + v - p]. Sequences (t any int,
    Ecyc = wt cyclic):
      Ep(t)   = Ecyc(t) + Ecyc(t+2048)          cyc2048
      En(t)   = Ecyc(t) - Ecyc(t+2048)          nega2048
      Epp(t)  = Ep(t) + Ep(t+1024)              cyc1024
      En10(t) = Ep(t) - Ep(t+1024)              nega1024 leaf
      C5(t)   = Epp(t) + Epp(t+512)             cyc512
      N5(t)   = Epp(t) - Epp(t+512)             nega512 leaf
      C25(t)  = C5(t) + C5(t+256)               cyc256 leaf (x0.5 baked)
      N25(t)  = C5(t) - C5(t+256)               nega256 leaf (x0.5 baked)
      D(t) = En(t) - En(t+1024); Bt(t) = En(t+512) + En(t+1536)
      L+/-(t) = D(t) +- sqrt2 En(t+1536)        T1024 low strips
      H+/-(t) = Bt(t) -+ sqrt2 En(t+1024)       T1024 high strips
    CRT scales (1/8 with the extra 1/2 baked for C25/N25, 1/8, 1/4,
    1/(4sqrt2)) are applied in the ACT PSUM->SBUF evacuations.
    """
    import ml_dtypes

    wt = np.roll(w[::-1], 1).astype(np.float64)
    Ecyc = lambda t: wt[np.mod(t, N)]
    Ep = lambda t: Ecyc(t) + Ecyc(t + 2048)
    En = lambda t: Ecyc(t) - Ecyc(t + 2048)
    Epp = lambda t: Ep(t) + Ep(t + 1024)
    C5 = lambda t: Epp(t) + Epp(t + 512)
    seqs = {
        "sC25": lambda t: 0.5 * (C5(t) + C5(t + 256)),
        "sN25": lambda t: 0.5 * (C5(t) - C5(t + 256)),
        "sN5": lambda t: Epp(t) - Epp(t + 512),
        "sN10": lambda t: Ep(t) - Ep(t + 1024),
        "sLp": lambda t: (En(t) - En(t + 1024)) + R2 * En(t + 1536),
        "sLm": lambda t: (En(t) - En(t + 1024)) - R2 * En(t + 1536),
        "sHp": lambda t: (En(t + 512) + En(t + 1536)) - R2 * En(t + 1024),
        "sHm": lambda t: (En(t + 512) + En(t + 1536)) + R2 * En(t + 1024),
    }
    p = np.arange(128)[:, None]
    tab = np.zeros((128, SBW), dtype=np.float64)
    for name, (c0, width, off) in STRIPS.items():
        v = np.arange(width)[None, :]
        tab[:, c0 : c0 + width] = seqs[name](off + v - p)
    return np.ascontiguousarray(tab.astype(ml_dtypes.bfloat16))


def _prep_inputs(x, w):
    import ml_dtypes

    x = np.ascontiguousarray(x, dtype=np.float32)
    w = np.ascontiguousarray(w, dtype=np.float32)
    sb = _make_strip_table(w)
    in_maps = []
    for i in range(N_CORES):
        xtr = x[i * B_SHARD : (i + 1) * B_SHARD].T  # [N, B_SHARD]
        xtr = xtr.reshape(32, 128, B_SHARD)[BITREV5]  # [a(pos), p, b]
        # [p, bblk, a, bwithin]: per-(p, bblk) an 8KB contiguous bf16 run
        xtr = np.ascontiguousarray(
            xtr.reshape(32, 128, NB, 128).transpose(1, 2, 0, 3).astype(
                ml_dtypes.bfloat16
            )
        )
        in_maps.append({"xtr": xtr, "sb": sb})
    return in_maps


def kernel(x, w, _trace=False):
    from concourse.bass_utils import run_bass_kernel_spmd

    nc = _get_nc()
    in_maps = _prep_inputs(x, w)
    res = run_bass_kernel_spmd(nc, in_maps, list(range(N_CORES)), trace=_trace)
    out = np.concatenate(
        [np.asarray(res.results[i]["out"]).astype(np.float32) for i in range(N_CORES)],
        axis=0,
    )
    if _trace:
        _STATE["last_result"] = res
    return out


# revision 64
# speedup vs baseline: 1.1359x; 1.0032x over previous
"""Circulant matmul for TRN2: 4.5-level CRT with trinomial split, bf16 matmuls.

out[b, r] = sum_c x[b,c] w[(c-r) mod N] = (x (*) wt)[r], wt = roll(w[::-1],1)
(cyclic convolution along c) — no input flip or output reversal needed.

CRT tree on z^4096-1 (per 128-row block of x):
  cyc4096 -> cyc2048 + nega2048             (binomial, band scale 1/2 each)
  cyc2048 -> cyc1024 + nega1024[LEAF]       (1/2)
  cyc1024 -> cyc512 + nega512[LEAF]         (1/2)
  cyc512  -> cyc256[LEAF] + nega256[LEAF]   (1/2)
  nega2048 -> T1024+ , T1024- [LEAVES]      (z^1024 -+ sqrt2 z^512 + 1, 1/(2sqrt2))
Leaf matmuls: 54 x bf16 = 27648 PE cycles/block (11.5us at 2.4GHz; the
f32 baseline's direct matmul would be 54.6us).  x chunks are stored in
bit-reversed order so EVERY fold level is a stride-2 slice and any quarter
of block 0 folds to completion as its DMA lands (fast pipeline fill).
I/O is bf16 both ways (host converts; rel-err budget allows it); per-block
HBM traffic is 2 MiB so the exclusive 360 B/ns DMA engine stays at ~50%.
Steady finals are two [128,2048] ops (DVE + Pool) feeding two output DMAs.
The last block streams out per nega1024 half-psum: P = rc1 +- rn-hi is
precomputed under the matmuls (split so the o3-dependent halves are
emitted early) and each half needs only evac -> 5 combines -> 2 strided
DMAs after its psum stops.  A single warmup matmul at ~1.3us starts the
PE p-state ramp clock so real matmuls from ~4.3us run at full clock; a
few more bridge the idle so the ramp timer never resets.
Per-block engine busy: PE 11.9 (critical), DVE 11.2, Pool 7.4, ACT 7.2,
DMA 6.6.  Makespan = fill ~6.1 + PE stream ~93 + drain ~7.
"""

import sys

sys.path.insert(0, "/opt/trn_rl_repo")

import numpy as np

N = 4096
B = 8192
N_CORES = 8
B_SHARD = B // N_CORES  # 1024
NB = B_SHARD // 128     # 8 row-tiles per core
R2 = float(np.sqrt(2.0))
SC_T = float(1.0 / (4.0 * np.sqrt(2.0)))   # T1024 leaves: 1/2 * 1/(2sqrt2)

# bit-reversal position maps: leaf-residue chunk a lives at fold-output
# position BRk[a] (k = log2 nchunks)
BR2 = [0, 2, 1, 3]
BR3 = [0, 4, 2, 6, 1, 5, 3, 7]
# x chunk stored at position i is BITREV5[i]
BITREV5 = [
    ((i & 1) << 4) | ((i & 2) << 2) | (i & 4) | ((i & 8) >> 2) | ((i & 16) >> 4)
    for i in range(32)
]

# strip layout in the packed [128, SBW] host table (name: col0, width, OFF)
STRIPS = {
    "sC25": (0, 384, -128),
    "sN25": (384, 384, -128),
    "sN5": (768, 896, -384),
    "sN10": (1664, 1920, -896),
    "sLp": (3584, 1408, -896),
    "sHp": (4992, 1408, -896),
    "sLm": (6400, 1408, -896),
    "sHm": (7808, 1408, -896),
}
SBW = 9216
SB_CN = 1664    # sC25+sN25+sN5
SB_N10 = 3584   # ..sN10

_STATE = {}

N_WARM = 10


def _build():
    import concourse.bacc as bacc
    import concourse.mybir as mybir
    import concourse.tile as tile

    f32 = mybir.dt.float32
    bf16 = mybir.dt.bfloat16
    ADD = mybir.AluOpType.add
    SUB = mybir.AluOpType.subtract

    nc = bacc.Bacc("TRN2", target_bir_lowering=False, debug=False)
    # x transposed, chunk-bit-reversed, bf16 on host; laid out
    # [p, bblk, a, bwithin] so each block is an 8KB contiguous run/partition.
    xtr_d = nc.declare_dram_parameter("xtr", [128, NB, 32, 128], bf16, isOutput=False)
    sb_d = nc.declare_dram_parameter("sb", [128, SBW], bf16, isOutput=False)
    out_d = nc.declare_dram_parameter("out", [B_SHARD, N], bf16, isOutput=True)

    with tile.TileContext(nc) as tc:
        with (
            tc.tile_pool(name="const", bufs=1) as constp,
            tc.tile_pool(name="xbig", bufs=3) as xbigp,
            tc.tile_pool(name="fold", bufs=2) as fp,
            tc.tile_pool(name="evac", bufs=3) as ep,
            tc.tile_pool(name="unf", bufs=2) as up,
            tc.tile_pool(name="outp", bufs=2) as op,
            tc.tile_pool(name="psum", bufs=1, space="PSUM") as pp,
        ):
            SB = constp.tile([128, SBW], bf16, name="SB")
            warm = constp.tile([128, 512], bf16, name="warm")
            strip = {k: SB[:, c0 : c0 + w] for k, (c0, w, _) in STRIPS.items()}

            def emit_dma_in(bt):
                xbig = xbigp.tile([128, 32, 128], bf16, tag="xbig", name="xbig")
                nc.sync.dma_start(xbig[:], xtr_d[:, bt, :, :])
                return xbig

            def emit_folds_a(xbig, swap=False):
                """Early folds: xc (Pool), cyc chain (DVE) — these feed the
                first matmuls of the next block. swap=True runs xc on DVE."""
                xc = fp.tile([128, 16, 128], bf16, tag="xc", name="xc")
                xn = fp.tile([128, 16, 128], bf16, tag="xn", name="xn")
                if swap:
                    nc.vector.tensor_tensor(
                        xc[:], xbig[:, 0:32:2, :], xbig[:, 1:32:2, :], ADD
                    )
                    nc.gpsimd.tensor_tensor(
                        xn[:], xbig[:, 0:32:2, :], xbig[:, 1:32:2, :], SUB
                    )
                else:
                    nc.gpsimd.tensor_tensor(
                        xc[:], xbig[:, 0:32:2, :], xbig[:, 1:32:2, :], ADD
                    )
                    nc.vector.tensor_tensor(
                        xn[:], xbig[:, 0:32:2, :], xbig[:, 1:32:2, :], SUB
                    )
                xcc = fp.tile([128, 8, 128], bf16, tag="xcc", name="xcc")
                nc.vector.tensor_tensor(
                    xcc[:], xc[:, 0:16:2, :], xc[:, 1:16:2, :], ADD
                )
                xccc = fp.tile([128, 4, 128], bf16, tag="xccc", name="xccc")
                nc.vector.tensor_tensor(
                    xccc[:], xcc[:, 0:8:2, :], xcc[:, 1:8:2, :], ADD
                )
                xccn = fp.tile([128, 4, 128], bf16, tag="xccn", name="xccn")
                nc.vector.tensor_tensor(
                    xccn[:], xcc[:, 0:8:2, :], xcc[:, 1:8:2, :], SUB
                )
                xc4 = fp.tile([128, 2, 128], bf16, tag="xc4", name="xc4")
                nc.vector.tensor_tensor(
                    xc4[:], xccc[:, 0:4:2, :], xccc[:, 1:4:2, :], ADD
                )
                xn4 = fp.tile([128, 2, 128], bf16, tag="xn4", name="xn4")
                nc.vector.tensor_tensor(
                    xn4[:], xccc[:, 0:4:2, :], xccc[:, 1:4:2, :], SUB
                )
                xcn = fp.tile([128, 8, 128], bf16, tag="xcn", name="xcn")
                nc.vector.tensor_tensor(
                    xcn[:], xc[:, 0:16:2, :], xc[:, 1:16:2, :], SUB
                )
                return dict(xn=xn, xcn=xcn, xc4=xc4, xn4=xn4, xccn=xccn)

            def emit_folds_b(fa, dve_scales=False):
                """Late folds: trinomial xTp/xTm (needed only by the T
                matmuls, mid-block). sqrt2 scales on ACT (DVE 4x-mode for
                block 0, where the ACT hop would delay the first T matmuls)."""
                xn = fa["xn"]
                xA = fp.tile([128, 4, 128], bf16, tag="xA", name="xA")
                nc.vector.tensor_tensor(
                    xA[:], xn[:, 0:16:4, :], xn[:, 1:16:4, :], SUB
                )
                xB = fp.tile([128, 4, 128], bf16, tag="xB", name="xB")
                nc.vector.tensor_tensor(
                    xB[:], xn[:, 2:16:4, :], xn[:, 3:16:4, :], ADD
                )
                s10 = fp.tile([128, 4, 128], bf16, tag="s10", name="s10")
                s11 = fp.tile([128, 4, 128], bf16, tag="s11", name="s11")
                if dve_scales:
                    nc.vector.tensor_scalar_mul(s10[:], xn[:, 1:16:4, :], R2)
                    nc.vector.tensor_scalar_mul(s11[:], xn[:, 3:16:4, :], R2)
                else:
                    nc.scalar.mul(s10[:], xn[:, 1:16:4, :], R2)
                    nc.scalar.mul(s11[:], xn[:, 3:16:4, :], R2)
                xTp = fp.tile([128, 8, 128], bf16, tag="xTp", name="xTp")
                nc.vector.tensor_tensor(xTp[:, 0:4, :], xA[:], s11[:], ADD)
                nc.vector.tensor_tensor(xTp[:, 4:8, :], xB[:], s10[:], SUB)
                xTm = fp.tile([128, 8, 128], bf16, tag="xTm", name="xTm")
                nc.vector.tensor_tensor(xTm[:, 0:4, :], xA[:], s11[:], SUB)
                nc.vector.tensor_tensor(xTm[:, 4:8, :], xB[:], s10[:], ADD)
                fa["xTp"] = xTp
                fa["xTm"] = xTm
                return fa

            def emit_folds_first(xbig):
                """Block 0: fold each quarter's cyc chain as its DMA lands;
                xn/xcn/xc4 pieces follow. All on DVE (Pool too slow to gate)."""
                xc = fp.tile([128, 16, 128], bf16, tag="xc", name="xc")
                xn = fp.tile([128, 16, 128], bf16, tag="xn", name="xn")
                xcc = fp.tile([128, 8, 128], bf16, tag="xcc", name="xcc")
                xccc = fp.tile([128, 4, 128], bf16, tag="xccc", name="xccc")
                xccn = fp.tile([128, 4, 128], bf16, tag="xccn", name="xccn")
                xc4 = fp.tile([128, 2, 128], bf16, tag="xc4", name="xc4")
                xn4 = fp.tile([128, 2, 128], bf16, tag="xn4", name="xn4")
                xcn = fp.tile([128, 8, 128], bf16, tag="xcn", name="xcn")
                for q in range(4):
                    s = 8 * q
                    # the cyc chain feeds the first matmuls — schedule ahead
                    # of the xn/xcn side ops on DVE
                    with tc.high_priority():
                        nc.vector.tensor_tensor(
                            xc[:, 4 * q : 4 * q + 4, :],
                            xbig[:, s : s + 8 : 2, :],
                            xbig[:, s + 1 : s + 8 : 2, :], ADD
                        )
                        nc.vector.tensor_tensor(
                            xcc[:, 2 * q : 2 * q + 2, :],
                            xc[:, 4 * q : 4 * q + 4 : 2, :],
                            xc[:, 4 * q + 1 : 4 * q + 4 : 2, :], ADD
                        )
                        nc.vector.tensor_tensor(
                            xccc[:, q : q + 1, :],
                            xcc[:, 2 * q : 2 * q + 1, :],
                            xcc[:, 2 * q + 1 : 2 * q + 2, :], ADD
                        )
                        if q == 1 or q == 3:
                            h = q // 2  # xc4/xn4 entry h, xccc pos 2h, 2h+1
                            nc.vector.tensor_tensor(
                                xc4[:, h : h + 1, :],
                                xccc[:, 2 * h : 2 * h + 1, :],
                                xccc[:, 2 * h + 1 : 2 * h + 2, :], ADD
                            )
                            nc.vector.tensor_tensor(
                                xn4[:, h : h + 1, :],
                                xccc[:, 2 * h : 2 * h + 1, :],
                                xccc[:, 2 * h + 1 : 2 * h + 2, :], SUB
                            )
                    nc.vector.tensor_tensor(
                        xccn[:, q : q + 1, :],
                        xcc[:, 2 * q : 2 * q + 1, :],
                        xcc[:, 2 * q + 1 : 2 * q + 2, :], SUB
                    )
                # xcn/xn (needed only by the later N10/T matmuls) after the
                # cyc chains so they don't delay the q2/q3 fold chains
                for h in range(2):
                    nc.vector.tensor_tensor(
                        xcn[:, 4 * h : 4 * h + 4, :],
                        xc[:, 8 * h : 8 * h + 8 : 2, :],
                        xc[:, 8 * h + 1 : 8 * h + 8 : 2, :], SUB
                    )
                    nc.vector.tensor_tensor(
                        xn[:, 8 * h : 8 * h + 8, :],
                        xbig[:, 16 * h : 16 * h + 16 : 2, :],
                        xbig[:, 16 * h + 1 : 16 * h + 16 : 2, :], SUB
                    )
                fa = dict(xn=xn, xcn=xcn, xc4=xc4, xn4=xn4, xccn=xccn)
                return emit_folds_b(fa, dve_scales=True)

            def mms_c25n25(f):
                pCN25 = pp.tile([128, 512], f32, tag="pcn25", name="pcn25")
                for a in range(2):
                    v0 = 128 * (1 - a)
                    nc.tensor.matmul(
                        pCN25[:, 0:256], f["xc4"][:, a, :],
                        strip["sC25"][:, v0 : v0 + 256],
                        start=(a == 0), stop=(a == 1),
                    )
                for a in range(2):
                    v0 = 128 * (1 - a)
                    nc.tensor.matmul(
                        pCN25[:, 256:512], f["xn4"][:, a, :],
                        strip["sN25"][:, v0 : v0 + 256],
                        start=(a == 0), stop=(a == 1),
                    )
                return pCN25

            def mms_n5(f):
                pN5 = pp.tile([128, 512], f32, tag="pn5", name="pn5")
                for i, a in enumerate((0, 2, 1, 3)):  # feed order: quarter BR2[a]
                    v0 = 128 * (3 - a)
                    nc.tensor.matmul(
                        pN5[:], f["xccn"][:, BR2[a], :],
                        strip["sN5"][:, v0 : v0 + 512],
                        start=(i == 0), stop=(i == 3),
                    )
                return pN5

            def evac_cn(pCN25, pN5):
                rCN = ep.tile([128, 1024], bf16, tag="rcn", name="rcn")
                nc.scalar.mul(rCN[:, 0:512], pCN25[:], 0.125)
                nc.scalar.mul(rCN[:, 512:1024], pN5[:], 0.125)
                return rCN

            def mms_cn(f):
                """cyc256+nega256 (shared psum bank) and nega512 leaves."""
                pCN25 = mms_c25n25(f)
                pN5 = mms_n5(f)
                return evac_cn(pCN25, pN5)

            def mms_n10(f):
                """nega1024 in two half-PSUM tiles (tail streams per half)."""
                pN10a = pp.tile([128, 512], f32, tag="pn10a", name="pn10a")
                pN10b = pp.tile([128, 512], f32, tag="pn10b", name="pn10b")
                sN10 = strip["sN10"]
                rN10 = ep.tile([128, 1024], bf16, tag="rn10", name="rn10")
                for j, ps in ((0, pN10a), (1, pN10b)):
                    for i, a in enumerate((0, 4, 2, 6, 1, 5, 3, 7)):
                        v0 = 896 - 128 * a + 512 * j
                        nc.tensor.matmul(
                            ps[:], f["xcn"][:, BR3[a], :], sN10[:, v0 : v0 + 512],
                            start=(i == 0), stop=(i == 7),
                        )
                    nc.scalar.mul(rN10[:, 512 * j : 512 * j + 512], ps[:], 0.25)
                return rN10

            def mm_t_ring(psum, xres, sname):
                s = strip[sname]
                for a in range(8):
                    v0 = 896 - 128 * a
                    pos = BR2[a] if a < 4 else 4 + BR2[a - 4]
                    nc.tensor.matmul(
                        psum[:], xres[:, pos, :], s[:, v0 : v0 + 512],
                        start=(a == 0), stop=(a == 7),
                    )

            def mms_t(f, lo_first=False):
                """lo_first runs both low strips before the high strips so
                the rn chain's o3/t0 diffs can start ~1.7us earlier (tail)."""
                rTp = ep.tile([128, 1024], bf16, tag="rtp", name="rtp")
                rTm = ep.tile([128, 1024], bf16, tag="rtm", name="rtm")
                work = [
                    (f["xTp"], rTp, 0, "sLp"), (f["xTp"], rTp, 1, "sHp"),
                    (f["xTm"], rTm, 0, "sLm"), (f["xTm"], rTm, 1, "sHm"),
                ]
                if lo_first:
                    work = [work[0], work[2], work[1], work[3]]
                for xres, rr, j, sname in work:
                    ph = pp.tile(
                        [128, 512], f32, tag=f"pt{sname}", name=f"pt{sname}"
                    )
                    mm_t_ring(ph, xres, sname)
                    nc.scalar.mul(
                        rr[:, 512 * j : 512 * j + 512], ph[:], SC_T
                    )
                return rTp, rTm

            def emit_rn_chain(rTp, rTm):
                """nega2048 reconstruction from T1024+/- (DVE + ACT scale)."""
                rn = up.tile([128, 2048], bf16, tag="rn", name="rn")
                tt01 = up.tile([128, 1024], bf16, tag="tt01", name="tt01")
                st01 = up.tile([128, 1024], bf16, tag="st01", name="st01")
                nc.vector.tensor_tensor(  # o3
                    rn[:, 1536:2048], rTp[:, 0:512], rTm[:, 0:512], SUB
                )
                nc.vector.tensor_tensor(  # t0
                    tt01[:, 0:512], rTp[:, 0:512], rTm[:, 0:512], ADD
                )
                nc.vector.tensor_tensor(  # o2
                    rn[:, 1024:1536], rTm[:, 512:1024], rTp[:, 512:1024], SUB
                )
                nc.vector.tensor_tensor(  # t1
                    tt01[:, 512:1024], rTp[:, 512:1024], rTm[:, 512:1024], ADD
                )
                nc.scalar.mul(st01[:], tt01[:], R2)
                nc.vector.tensor_tensor(  # o0
                    rn[:, 0:512], st01[:, 0:512], rn[:, 1024:1536], ADD
                )
                nc.vector.tensor_tensor(  # o1
                    rn[:, 512:1024], st01[:, 512:1024], rn[:, 1536:2048], SUB
                )
                return rn

            def emit_rc_part(rCN, rN10):
                rc0 = up.tile([128, 512], bf16, tag="rc0", name="rc0")
                nc.vector.tensor_tensor(
                    rc0[:, 0:256], rCN[:, 0:256], rCN[:, 256:512], ADD
                )
                nc.vector.tensor_tensor(
                    rc0[:, 256:512], rCN[:, 0:256], rCN[:, 256:512], SUB
                )
                rc1 = up.tile([128, 1024], bf16, tag="rc1", name="rc1")
                nc.vector.tensor_tensor(
                    rc1[:, 0:512], rc0[:], rCN[:, 512:1024], ADD
                )
                nc.vector.tensor_tensor(
                    rc1[:, 512:1024], rc0[:], rCN[:, 512:1024], SUB
                )
                rc2 = up.tile([128, 2048], bf16, tag="rc2", name="rc2")
                nc.vector.tensor_tensor(rc2[:, 0:1024], rc1[:], rN10[:], ADD)
                nc.vector.tensor_tensor(rc2[:, 1024:2048], rc1[:], rN10[:], SUB)
                return rc2

            def emit_finals(bt, rc2, rn, pool=True):
                b0 = 128 * bt
                os01 = op.tile([128, 2048], bf16, tag="os01", name="os01")
                os23 = op.tile([128, 2048], bf16, tag="os23", name="os23")
                nc.vector.tensor_tensor(os01[:], rc2[:], rn[:], ADD)
                if pool:
                    nc.gpsimd.tensor_tensor(os23[:], rc2[:], rn[:], SUB)
                else:
                    nc.vector.tensor_tensor(os23[:], rc2[:], rn[:], SUB)
                nc.sync.dma_start(out_d[b0 : b0 + 128, 0:2048], os01[:])
                nc.sync.dma_start(out_d[b0 : b0 + 128, 2048:4096], os23[:])

            def emit_unfold(bt, rCN, rN10, rTp, rTm):
                rn = emit_rn_chain(rTp, rTm)
                rc2 = emit_rc_part(rCN, rN10)
                emit_finals(bt, rc2, rn)

            def emit_tail_rc1(rCN):
                rc0 = up.tile([128, 512], bf16, tag="rc0", name="rc0")
                nc.vector.tensor_tensor(
                    rc0[:, 0:256], rCN[:, 0:256], rCN[:, 256:512], ADD
                )
                nc.vector.tensor_tensor(
                    rc0[:, 256:512], rCN[:, 0:256], rCN[:, 256:512], SUB
                )
                rc1 = up.tile([128, 1024], bf16, tag="rc1", name="rc1")
                nc.vector.tensor_tensor(
                    rc1[:, 0:512], rc0[:], rCN[:, 512:1024], ADD
                )
                nc.vector.tensor_tensor(
                    rc1[:, 512:1024], rc0[:], rCN[:, 512:1024], SUB
                )
                return rc1

            def emit_tail_rn_p(rTp, rTm, rc1):
                """Tail rn chain with P1/P3 = rc1 +- rn-hi interleaved right
                after the o2/o3 diffs (before the st01-gated o0/o1) so they
                are hidden under the N10 matmuls."""
                rn = up.tile([128, 2048], bf16, tag="rn", name="rn")
                tt01 = up.tile([128, 1024], bf16, tag="tt01", name="tt01")
                st01 = up.tile([128, 1024], bf16, tag="st01", name="st01")
                P = up.tile([128, 2, 1024], bf16, tag="P", name="P")
                nc.vector.tensor_tensor(  # o3
                    rn[:, 1536:2048], rTp[:, 0:512], rTm[:, 0:512], SUB
                )
                nc.vector.tensor_tensor(  # t0
                    tt01[:, 0:512], rTp[:, 0:512], rTm[:, 0:512], ADD
                )
                # P-high halves depend only on o3 — emit early
                nc.vector.tensor_tensor(
                    P[:, 0, 512:1024], rc1[:, 512:1024], rn[:, 1536:2048], ADD
                )
                nc.vector.tensor_tensor(
                    P[:, 1, 512:1024], rc1[:, 512:1024], rn[:, 1536:2048], SUB
                )
                nc.vector.tensor_tensor(  # o2
                    rn[:, 1024:1536], rTm[:, 512:1024], rTp[:, 512:1024], SUB
                )
                nc.vector.tensor_tensor(  # t1
                    tt01[:, 512:1024], rTp[:, 512:1024], rTm[:, 512:1024], ADD
                )
                nc.vector.tensor_tensor(
                    P[:, 0, 0:512], rc1[:, 0:512], rn[:, 1024:1536], ADD
                )
                nc.vector.tensor_tensor(
                    P[:, 1, 0:512], rc1[:, 0:512], rn[:, 1024:1536], SUB
                )
                nc.vector.tensor_scalar_mul(st01[:], tt01[:], R2)  # DVE 4x mode
                nc.vector.tensor_tensor(  # o0
                    rn[:, 0:512], st01[:, 0:512], rn[:, 1024:1536], ADD
                )
                nc.vector.tensor_tensor(  # o1
                    rn[:, 512:1024], st01[:, 512:1024], rn[:, 1536:2048], SUB
                )
                return rn, P

            def emit_tail_n10(f):
                """nega1024 as four 256-wide quarter-rings: the psum quarters
                stop staggered so the finish streams out per quarter."""
                pN10a = pp.tile([128, 512], f32, tag="pn10a", name="pn10a")
                pN10b = pp.tile([128, 512], f32, tag="pn10b", name="pn10b")
                sN10 = strip["sN10"]
                for j, ps in ((0, pN10a), (1, pN10b)):
                    for i, a in enumerate((0, 4, 2, 6, 1, 5, 3, 7)):
                        v0 = 896 - 128 * a + 512 * j
                        nc.tensor.matmul(
                            ps[:], f["xcn"][:, BR3[a], :], sN10[:, v0 : v0 + 512],
                            start=(i == 0), stop=(i == 7),
                        )
                return pN10a, pN10b

            def emit_tail_finish(pN10a, pN10b, rc1, rn, P, bt, tg):
                """Streamed ending: P holds rc1 +- rn-hi; after each nega1024
                half-psum stops, only evac -> 5 small combines -> 2 strided
                DMAs remain."""
                b0 = 128 * bt
                rN10 = ep.tile([128, 1024], bf16, tag="rn10" + tg, name="rn10")
                preA = up.tile([128, 1024], bf16, tag="preA", name="preA")
                # os_ slot order per half: (s1, s3, s0, s2)
                os_ = op.tile([128, 2, 4, 512], bf16, tag="ost" + tg, name="ost")
                outv = out_d[:].rearrange("b (s c) -> b s c", c=512)[b0 : b0 + 128]
                for h, ps in ((0, pN10a), (1, pN10b)):
                    c0 = 512 * h
                    rh = rN10[:, c0 : c0 + 512]
                    nc.scalar.mul(rh, ps[:], 0.25)
                    nc.vector.tensor_tensor(  # s1 = P1 - rN10
                        os_[:, h, 0, :], P[:, 0, c0 : c0 + 512], rh, SUB
                    )
                    nc.vector.tensor_tensor(  # s3 = P3 - rN10
                        os_[:, h, 1, :], P[:, 1, c0 : c0 + 512], rh, SUB
                    )
                    nc.sync.dma_start(
                        outv[:, 2 + h : 7 + h : 4, :], os_[:, h, 0:2, :]
                    )
                    pa = preA[:, c0 : c0 + 512]
                    nc.vector.tensor_tensor(pa, rc1[:, c0 : c0 + 512], rh, ADD)
                    nc.vector.tensor_tensor(  # s0 = preA + rn-lo
                        os_[:, h, 2, :], pa, rn[:, c0 : c0 + 512], ADD
                    )
                    nc.vector.tensor_tensor(  # s2 = preA - rn-lo
                        os_[:, h, 3, :], pa, rn[:, c0 : c0 + 512], SUB
                    )
                    nc.sync.dma_start(
                        outv[:, h : 5 + h : 4, :], os_[:, h, 2:4, :]
                    )

            # ---------------- preamble ----------------
            nc.gpsimd.memset(warm[:], 0.0)
            xbig = xbigp.tile([128, 32, 128], bf16, tag="xbig", name="xbig")
            # DMA order: x quarters interleaved with strip pieces so the fold
            # chain, C/N strips and N10/T strips all land just in time.
            nc.sync.dma_start(xbig[:, 0:8, :], xtr_d[:, 0, 0:8, :])
            nc.sync.dma_start(xbig[:, 8:16, :], xtr_d[:, 0, 8:16, :])
            nc.sync.dma_start(SB[:, 0:768], sb_d[:, 0:768])  # sC25+sN25
            nc.sync.dma_start(SB[:, SB_CN:SB_N10], sb_d[:, SB_CN:SB_N10])  # sN10
            nc.sync.dma_start(xbig[:, 16:24, :], xtr_d[:, 0, 16:24, :])
            nc.sync.dma_start(xbig[:, 24:32, :], xtr_d[:, 0, 24:32, :])
            nc.sync.dma_start(SB[:, 768:SB_CN], sb_d[:, 768:SB_CN])  # sN5
            for s0 in range(SB_N10, SBW, 1408):
                nc.sync.dma_start(
                    SB[:, s0 + 512 : s0 + 1408], sb_d[:, s0 + 512 : s0 + 1408]
                )
                nc.sync.dma_start(SB[:, s0 : s0 + 512], sb_d[:, s0 : s0 + 512])
            # PE clock warmup: HAM releases 2.4 GHz after ~3us of activity;
            # burn dummies while the first DMAs/folds land.
            pW = pp.tile([128, 512], f32, tag="pcn25", name="pwarm")
            for _ in range(N_WARM):
                nc.tensor.matmul(
                    pW[:], warm[:, 0:128], warm[:], start=True, stop=True
                )

            # ---------------- main pipeline ----------------
            # Iteration bt interleaves emissions so each engine's in-order
            # stream matches when its work becomes runnable:
            #   mms+evacs(bt) | rn-chain(bt-1) | early folds(bt+1) |
            #   rc-part(bt-1) | T-folds(bt+1) | finals(bt-1)
            f_cur = emit_folds_first(xbig)
            xbig_next = emit_dma_in(1)
            r_prev = None
            for bt in range(NB - 2):
                xbig = xbig_next
                if bt + 2 < NB:
                    xbig_next = emit_dma_in(bt + 2)
                if bt == 0:
                    # block 0: C25/N25 (earliest strip), then N10 (its first
                    # ring chunks need only quarters q0/q1), N5 last — its
                    # strip and the q2/q3 folds are still in flight
                    pCN25 = mms_c25n25(f_cur)
                    rN10 = mms_n10(f_cur)
                    rCN = evac_cn(pCN25, mms_n5(f_cur))
                else:
                    rCN = mms_cn(f_cur)
                    rN10 = mms_n10(f_cur)
                rTp, rTm = mms_t(f_cur)
                if r_prev is not None:
                    rn = emit_rn_chain(r_prev[2], r_prev[3])
                fa = emit_folds_a(xbig, swap=(bt == 0))
                if r_prev is not None:
                    rc2 = emit_rc_part(r_prev[0], r_prev[1])
                f_cur = emit_folds_b(fa)
                if r_prev is not None:
                    emit_finals(bt - 1, rc2, rn)
                r_prev = (rCN, rN10, rTp, rTm)
            # last two blocks: block 7's T matmuls hoisted before block 6's
            # so the tail's long recon chains overlap remaining PE work
            f6 = f_cur
            rCN6 = mms_cn(f6)
            rN10_6 = mms_n10(f6)
            rn5 = emit_rn_chain(r_prev[2], r_prev[3])
            fa7 = emit_folds_a(xbig_next)
            rc2_5 = emit_rc_part(r_prev[0], r_prev[1])
            f7 = emit_folds_b(fa7)
            emit_finals(NB - 3, rc2_5, rn5)
            rTp6, rTm6 = mms_t(f6)
            rCN7 = mms_cn(f7)
            rn6 = emit_rn_chain(rTp6, rTm6)
            rc2_6 = emit_rc_part(rCN6, rN10_6)
            rTp7, rTm7 = mms_t(f7, lo_first=True)
            rc1_7 = emit_tail_rc1(rCN7)
            emit_finals(NB - 2, rc2_6, rn6)
            rn7, P7 = emit_tail_rn_p(rTp7, rTm7, rc1_7)
            pa7, pb7 = emit_tail_n10(f7)
            emit_tail_finish(pa7, pb7, rc1_7, rn7, P7, NB - 1, "")

    nc.compile()
    return nc


def _get_nc():
    if "nc" not in _STATE:
        _STATE["nc"] = _build()
    return _STATE["nc"]


def _make_strip_table(w):
    """All 8 leaf band strips, packed [128, SBW] bf16.

    Strip tiles are shears: tile[p, v] = S[OFF + v - p]. Sequences (t any int,
    Ecyc = wt cyclic):
      Ep(t)   = Ecyc(t) + Ecyc(t+2048)          cyc2048
      En(t)   = Ecyc(t) - Ecyc(t+2048)          nega2048
      Epp(t)  = Ep(t) + Ep(t+1024)              cyc1024
      En10(t) = Ep(t) - Ep(t+1024)              nega1024 leaf
      C5(t)   = Epp(t) + Epp(t+512)             cyc512
      N5(t)   = Epp(t) - Epp(t+512)             nega512 leaf
      C25(t)  = C5(t) + C5(t+256)               cyc256 leaf (x0.5 baked)
      N25(t)  = C5(t) - C5(t+256)               nega256 leaf (x0.5 baked)
      D(t) = En(t) - En(t+1024); Bt(t) = En(t+512) + En(t+1536)
      L+/-(t) = D(t) +- sqrt2 En(t+1536)        T1024 low strips
      H+/-(t) = Bt(t) -+ sqrt2 En(t+1024)       T1024 high strips
    CRT scales (1/8 with the extra 1/2 baked for C25/N25, 1/8, 1/4,
    1/(4sqrt2)) are applied in the ACT PSUM->SBUF evacuations.
    """
    import ml_dtypes

    wt = np.roll(w[::-1], 1).astype(np.float64)
    Ecyc = lambda t: wt[np.mod(t, N)]
    Ep = lambda t: Ecyc(t) + Ecyc(t + 2048)
    En = lambda t: Ecyc(t) - Ecyc(t + 2048)
    Epp = lambda t: Ep(t) + Ep(t + 1024)
    C5 = lambda t: Epp(t) + Epp(t + 512)
    seqs = {
        "sC25": lambda t: 0.5 * (C5(t) + C5(t + 256)),
        "sN25": lambda t: 0.5 * (C5(t) - C5(t + 256)),
        "sN5": lambda t: Epp(t) - Epp(t + 512),
        "sN10": lambda t: Ep(t) - Ep(t + 1024),
        "sLp": lambda t: (En(t) - En(t + 1024)) + R2 * En(t + 1536),
        "sLm": lambda t: (En(t) - En(t + 1024)) - R2 * En(t + 1536),
        "sHp": lambda t: (En(t + 512) + En(t + 1536)) - R2 * En(t + 1024),
        "sHm": lambda t: (En(t + 512) + En(t + 1536)) + R2 * En(t + 1024),
    }
    p = np.arange(128)[:, None]
    tab = np.zeros((128, SBW), dtype=np.float64)
    for name, (c0, width, off) in STRIPS.items():
        v = np.arange(width)[None, :]
        tab[:, c0 : c0 + width] = seqs[name](off + v - p)
    return np.ascontiguousarray(tab.astype(ml_dtypes.bfloat16))


def _prep_inputs(x, w):
    import ml_dtypes

    x = np.ascontiguousarray(x, dtype=np.float32)
    w = np.ascontiguousarray(w, dtype=np.float32)
    sb = _make_strip_table(w)
    in_maps = []
    for i in range(N_CORES):
        xtr = x[i * B_SHARD : (i + 1) * B_SHARD].T  # [N, B_SHARD]
        xtr = xtr.reshape(32, 128, B_SHARD)[BITREV5]  # [a(pos), p, b]
        # [p, bblk, a, bwithin]: per-(p, bblk) an 8KB contiguous bf16 run
        xtr = np.ascontiguousarray(
            xtr.reshape(32, 128, NB, 128).transpose(1, 2, 0, 3).astype(
                ml_dtypes.bfloat16
            )
        )
        in_maps.append({"xtr": xtr, "sb": sb})
    return in_maps


def kernel(x, w, _trace=False):
    from concourse.bass_utils import run_bass_kernel_spmd

    nc = _get_nc()
    in_maps = _prep_inputs(x, w)
    res = run_bass_kernel_spmd(nc, in_maps, list(range(N_CORES)), trace=_trace)
    out = np.concatenate(
        [np.asarray(res.results[i]["out"]).astype(np.float32) for i in range(N_CORES)],
        axis=0,
    )
    if _trace:
        _STATE["last_result"] = res
    return out
